# revision 16
# baseline (speedup 1.0000x reference)
"""Trainium2 Bass kernel for the CLIT-style sparse local attention module.

Strategy (8 NeuronCores, SPMD, no collectives):
  - core c = (batch bi = c // 4, strip qc = c % 4) OWNS the queries whose
    nearest-pixel center row falls in image rows [16qc, 16qc+16). Each core
    computes the 5 convs locally for the 24-row band [16qc-3, 16qc+21)
    (own 16 rows + 3-row halo each side + 2 spare), so every window/bilinear
    read its queries need is produced locally -- the AllGather disappears.
  - qkv conv outputs are PE-transposed to pixel-major entries in local DRAM
    with a 4x row-shifted duplication: entry (r, x) holds rows r..r+3 of
    column x back to back. A 7x7 window then needs only TWO dma_gather
    descriptors per query (4+3 window rows each), and the bilinear q read
    needs ONE (2x2 patch via 2x row duplication).
  - v is stored as fp8e4 (k stays bf16): halves the v gather traffic; the
    wv multiply runs at DVE 1x anyway (broadcast attn operand), so fp8
    costs nothing there. rel-err budget is ~2e-2; bf16 baseline is ~8e-5.
  - Attention per 128-query tile: QK logits as DVE mul + binary-tree adds
    (tensor_reduce is 1x-mode and slow), softmax, attention-weighted v
    (split DVE/GpSimd), PE-transposed into the K-major layout for the
    9410x256 MLP0 (bf16, fp32 PSUM), then MLP1-3 + head per tile group.
  - Host precomputes ownership, gather indices, bilinear weights, masks;
    kernel() scatters per-core outputs back to the original query order.
"""

import sys

sys.path.insert(0, "/opt/trn_rl_repo")

import numpy as np
import ml_dtypes

# ---------------- problem constants (hardcoded per contract) ----------------
B, CH_IN, H, W = 2, 3, 64, 64
Q = 4096
DIM, HEAD, R = 192, 8, 3
RR = 2 * R + 1
RA = RR * RR          # 49
HD = DIM // HEAD      # 24
ENC = 64
HID = 256
N_CORES = 8

NROWS = 24            # local conv band rows (strip 16 + 3 halo + 2 spare)
NPX = NROWS * W       # 1536 pixel entries
SUB_E = 288           # bf16 slots per kv sub-entry: 192 k bf16 + 192 v fp8
KV_ENT = 4 * SUB_E    # 1152: entry holds rows r..r+3
HEADKV = 195          # 192 write-shift room + 3 px guard
KVROWS = HEADKV + NPX # 1731
Q_ENT = 2 * 256       # q entry holds rows r..r+1
HEADQ = 64
QROWS = HEADQ + NPX   # 1600

DYW = RR * DIM                    # 1344 columns per window-row chunk
DYW_P = 1408                      # padded to 11 x 128
DY_BLOCKS = DYW_P // 128          # 11
KBLK = RR * DY_BLOCKS             # 77 K-blocks for MLP layer 0

f32 = np.float32
bf16 = ml_dtypes.bfloat16

_PROGRAMS = {}  # cached compiled Bass programs keyed by NT


def _groups(nt):
    gs = []
    t = 0
    while t < nt:
        gs.append(tuple(range(t, min(t + 2, nt))))
        t += 2
    return gs


# ============================ device program ================================

def build_program(NT):
    import concourse.bass as bass
    import concourse.tile as tile
    from concourse import bacc, mybir

    dt = mybir.dt

    nc = bacc.Bacc("TRN2", target_bir_lowering=False, debug=False,
                   enable_asserts=False, num_devices=N_CORES)

    def din(name, shape, dtype):
        return nc.dram_tensor(name, list(shape), dtype, kind="ExternalInput").ap()

    inp_col = din("inp_col", [27, 6 * 512], dt.bfloat16)
    inp_hilo = din("inp_hilo", [128, 12, 6], dt.bfloat16)
    w_enc = din("w_enc", [27, ENC], dt.bfloat16)
    w_chp = din("w_chp", [128, 3, DIM], dt.bfloat16)
    w_ch2 = din("w_ch2", [ENC, 3, DIM], dt.bfloat16)
    w_qkv0 = din("w_qkv0", [128, 9, 576], dt.bfloat16)
    w_qkv1p = din("w_qkv1p", [128, 3, 576], dt.bfloat16)
    w_qkv1k2 = din("w_qkv1k2", [64, 3, 576], dt.bfloat16)
    qkvb = din("qkvb", [128, 5], dt.float32)
    maskfp = din("maskfp", [128, 26], dt.bfloat16)
    enc_b = din("enc_b", [ENC, 1], dt.float32)
    ch_b = din("ch_b", [128, 2], dt.float32)
    m0w = din("m0w", [128, KBLK, HID], dt.bfloat16)
    m13w = din("m13w", [128, 6, HID], dt.bfloat16)
    m4w = din("m4w", [128, 2, 3], dt.bfloat16)
    bmlp = din("bmlp", [128, 8], dt.float32)
    b4 = din("b4", [128, 3], dt.float32)
    ident = din("ident", [128, 128], dt.bfloat16)
    kvidx = din("kvidx", [128, NT, 16], dt.int16)
    qidx = din("qidx", [128, NT, 8], dt.int16)
    maskt = din("maskt", [128, NT, 56], dt.float32)
    qwt = din("qwt", [128, NT, 4], dt.bfloat16)
    qwbt = din("qwbt", [128, NT, 4], dt.float32)
    out = nc.dram_tensor("out", [NT * 128, 3], dt.float32,
                         kind="ExternalOutput").ap()

    with tile.TileContext(nc) as tc:
        with tc.tile_pool(name="dram", bufs=1, space="DRAM") as dp:
            kv4 = dp.tile([KVROWS, KV_ENT], dt.bfloat16)
            q2 = dp.tile([QROWS, Q_ENT], dt.bfloat16)

            _convs(nc, tc, mybir, locals())
            _attention(nc, tc, mybir, NT, locals())

    nc.compile()
    return nc


def _convs(nc, tc, mybir, env):
    dt = mybir.dt
    AF = mybir.ActivationFunctionType

    inp_col, w_enc = env["inp_col"], env["w_enc"]
    w_chp, w_ch2 = env["w_chp"], env["w_ch2"]
    w_qkv0, w_qkv1p, w_qkv1k2 = env["w_qkv0"], env["w_qkv1p"], env["w_qkv1k2"]
    qkvb = env["qkvb"]
    enc_b, ch_b, inp_hilo = env["enc_b"], env["ch_b"], env["inp_hilo"]
    kv4, q2 = env["kv4"], env["q2"]

    with (
        tc.tile_pool(name="cw", bufs=1) as cw,
        tc.tile_pool(name="cfeat", bufs=1) as cf,
        tc.tile_pool(name="cpsum", bufs=2, space="PSUM") as cp,
        tc.tile_pool(name="qpsum", bufs=2, space="PSUM") as cpq,
        tc.tile_pool(name="ctpsum", bufs=2, space="PSUM") as cpt,
        tc.tile_pool(name="cstage", bufs=3) as cs,
    ):
        col_sb = cw.tile([27, 6 * 512], dt.bfloat16)
        nc.sync.dma_start(col_sb[:], inp_col)
        wenc_sb = cw.tile([27, ENC], dt.bfloat16)
        nc.sync.dma_start(wenc_sb[:], w_enc)
        wchp_sb = cw.tile([128, 3, DIM], dt.bfloat16)
        nc.sync.dma_start(wchp_sb[:], w_chp)
        wch2_sb = cw.tile([ENC, 3, DIM], dt.bfloat16)
        nc.sync.dma_start(wch2_sb[:], w_ch2)
        wq0 = cw.tile([128, 9, 576], dt.bfloat16)
        nc.sync.dma_start(wq0[:], w_qkv0)
        wq1p = cw.tile([128, 3, 576], dt.bfloat16)
        nc.sync.dma_start(wq1p[:], w_qkv1p)
        wq1k2 = cw.tile([64, 3, 576], dt.bfloat16)
        nc.sync.dma_start(wq1k2[:], w_qkv1k2)
        qkvb_sb = cw.tile([128, 5], dt.float32)
        nc.sync.dma_start(qkvb_sb[:], qkvb)
        encb_sb = cw.tile([ENC, 1], dt.float32)
        nc.sync.dma_start(encb_sb[:], enc_b)
        chb_sb = cw.tile([128, 2], dt.float32)
        nc.sync.dma_start(chb_sb[:], ch_b)
        hilo_sb = cw.tile([128, 12, 6], dt.bfloat16)
        nc.sync.dma_start(hilo_sb[:], inp_hilo)
        id_c = cw.tile([128, 128], dt.bfloat16)
        nc.sync.dma_start(id_c[:], env["ident"])
        maskfp_sb = cw.tile([128, 26], dt.bfloat16)
        nc.sync.dma_start(maskfp_sb[:], env["maskfp"])

        # zero the kv head region (write-shift room + guard entries): rows
        # 0..HEADKV of kv4; shifted writes partially overwrite it afterwards.
        zt = cw.tile([128, 1755], dt.bfloat16)
        nc.vector.memset(zt[:], 0.0)
        kvf = kv4[:, :].flatten()
        nc.sync.dma_start(kvf[0: HEADKV * KV_ENT]
                          .rearrange("(p a) -> p a", p=128), zt[:])

        encp = cf.tile([ENC, 66, 66], dt.bfloat16)
        nc.vector.memset(encp[:], 0.0)
        fp0 = cf.tile([128, 66, 66], dt.bfloat16)
        nc.vector.memset(fp0[:], 0.0)
        fp1 = cf.tile([64, 66, 66], dt.bfloat16)
        nc.vector.memset(fp1[:], 0.0)
        encb2 = cf.tile([128, 66, 66], dt.bfloat16)
        fp1b = cf.tile([128, 66, 66], dt.bfloat16)

        # ---- enc conv (strips 2-7) ----
        for t in range(2, 8):
            ps = cp.tile([128, 512], dt.float32, tag="cps")
            nc.tensor.matmul(ps[:ENC, :], wenc_sb[:],
                             col_sb[:, (t - 2) * 512:(t - 1) * 512],
                             start=True, stop=True)
            dst = encp[:, 1 + t * 8: 1 + t * 8 + 8, 1:65]
            nc.scalar.activation(dst, ps[:ENC, :].rearrange("p (a b) -> p a b", a=8),
                                 AF.Identity, bias=encb_sb[:, 0:1])

        # encb2: enc features with a one-column-shifted copy in partitions 64:
        nc.vector.tensor_copy(encb2[0:64, 24:66, :], encp[:, 24:66, :])
        nc.vector.tensor_copy(encb2[64:128, 24:66, 0:65], encp[:, 24:66, 1:66])

        # ---- ch conv (strips 3-7; kx 0/1 paired into K=128, kx=2 single) ----
        for t in range(3, 8):
            for m, msz in ((0, 128), (1, 64)):
                ps = cp.tile([128, 512], dt.float32, tag="cps")
                for ky in range(3):
                    rhs = encb2[:, t * 8 + ky: t * 8 + ky + 8, 0:64]
                    nc.tensor.matmul(ps[:msz, :],
                                     wchp_sb[:, ky, m * 128: m * 128 + msz],
                                     rhs, start=(ky == 0), stop=False)
                for ky in range(3):
                    rhs = encp[:, t * 8 + ky: t * 8 + ky + 8, 2:66]
                    nc.tensor.matmul(ps[:msz, :],
                                     wch2_sb[:, ky, m * 128: m * 128 + msz],
                                     rhs, start=False, stop=(ky == 2))
                dstp = (fp0 if m == 0 else fp1)
                dst = dstp[:msz, 1 + t * 8: 1 + t * 8 + 8, 1:65]
                nc.scalar.activation(dst,
                                     ps[:msz, :].rearrange("p (a b) -> p a b", a=8),
                                     AF.Identity, bias=chb_sb[:msz, m: m + 1])

        # zero ch features of out-of-image rows (qkv convs read fp rows 32..57)
        mbc = maskfp_sb[:].unsqueeze(2).broadcast_to((128, 26, 66))
        nc.vector.tensor_mul(fp0[:, 32:58, :], fp0[:, 32:58, :], mbc)
        nc.vector.tensor_mul(fp1[:, 32:58, :], fp1[:, 32:58, :],
                             maskfp_sb[0:64, :].unsqueeze(2)
                             .broadcast_to((64, 26, 66)))

        # fp1b: channel-chunk-1 features with one-column-shifted copy
        nc.vector.tensor_copy(fp1b[0:64, 32:58, :], fp1[:, 32:58, :])
        nc.vector.tensor_copy(fp1b[64:128, 32:58, 0:65], fp1[:, 32:58, 1:66])

        # ---- q/k/v convs (strips 4-6; M-packed 576 = 4x128+64) + transpose ----
        # 576 cols = [q 0:192 | k 192:384 | v 384:576]
        # per group: list of (dst_kind, src_off, dst_off, n)
        #   kind 0 = qstage bf16, 1 = kvstage k bf16, 2 = kvstage v fp8 view
        gdst = [
            [(0, 0, 0, 128)],
            [(0, 0, 128, 64), (1, 64, 0, 64)],
            [(1, 0, 64, 128)],
            [(2, 0, 0, 128)],
            [(2, 0, 128, 64)],
        ]
        for t in (4, 5, 6):
            kvstage = cs.tile([128, 4, SUB_E], dt.bfloat16, tag="kvstage")
            qstage = cs.tile([128, 4, 256], dt.bfloat16, tag="qstage")
            vview = kvstage[:, :, 192:SUB_E].bitcast(dt.float8e4)
            nc.vector.memset(qstage[:, :, 198:], 0.0)
            nc.vector.tensor_copy(qstage[:, :, 192:198],
                                  hilo_sb[:, (t - 4) * 4:(t - 3) * 4, :])
            for g in range(5):
                msz = 128 if g < 4 else 64
                ps = cpq.tile([128, 512], dt.float32, tag="qkvps")
                for off in range(9):
                    ky, kx = off // 3, off % 3
                    rhs0 = fp0[:, t * 8 + ky: t * 8 + ky + 8, kx: kx + 64]
                    nc.tensor.matmul(ps[:msz, :],
                                     wq0[:, off, g * 128: g * 128 + msz],
                                     rhs0, start=(off == 0), stop=False)
                for ky in range(3):
                    rhs1 = fp1b[:, t * 8 + ky: t * 8 + ky + 8, 0:64]
                    nc.tensor.matmul(ps[:msz, :],
                                     wq1p[:, ky, g * 128: g * 128 + msz],
                                     rhs1, start=False, stop=False)
                for ky in range(3):
                    rhs1 = fp1[:, t * 8 + ky: t * 8 + ky + 8, 2:66]
                    nc.tensor.matmul(ps[:msz, :],
                                     wq1k2[:, ky, g * 128: g * 128 + msz],
                                     rhs1, start=False, stop=(ky == 2))
                csb = cs.tile([128, 512], dt.bfloat16, tag="convsb")
                nc.scalar.activation(csb[:msz, :], ps[:msz, :], AF.Identity,
                                     bias=qkvb_sb[:msz, g: g + 1])
                tps = cpt.tile([128, 512], dt.bfloat16, tag="ctps")
                for blk in range(4):
                    nc.tensor.transpose(
                        tps[:, blk * 128: blk * 128 + msz],
                        csb[:msz, blk * 128:(blk + 1) * 128],
                        id_c[:msz, :msz])
                tview = tps[:].rearrange("p (a b) -> p a b", a=4)
                for kind, so, do, n in gdst[g]:
                    if kind == 0:
                        nc.scalar.copy(qstage[:, :, do: do + n],
                                       tview[:, :, so: so + n])
                    elif kind == 1:
                        nc.scalar.copy(kvstage[:, :, do: do + n],
                                       tview[:, :, so: so + n])
                    else:
                        nc.scalar.copy(vview[:, :, do: do + n],
                                       tview[:, :, so: so + n])
            # 4x / 2x row-shifted duplicated writes: entry (r, x) sub s holds
            # local row r+s. Strip t covers local rows LT..LT+7.
            LT = (t - 4) * 8
            for s in range(4):
                r0 = HEADKV + (LT - s) * 64
                nc.sync.dma_start(
                    kv4[r0: r0 + 512, s * SUB_E: (s + 1) * SUB_E]
                    .rearrange("(b p) e -> p b e", p=128), kvstage[:])
            for s in range(2):
                r0 = HEADQ + (LT - s) * 64
                nc.sync.dma_start(
                    q2[r0: r0 + 512, s * 256: (s + 1) * 256]
                    .rearrange("(b p) e -> p b e", p=128), qstage[:])


def _attention(nc, tc, mybir, NT, env):
    import concourse.bass as bass
    dt = mybir.dt
    AX = mybir.AxisListType
    AF = mybir.ActivationFunctionType

    kv4, q2 = env["kv4"], env["q2"]
    m0w, m13w, m4w = env["m0w"], env["m13w"], env["m4w"]
    bmlp, b4, ident = env["bmlp"], env["b4"], env["ident"]
    kvidx, qidx = env["kvidx"], env["qidx"]
    maskt, qwt, qwbt = env["maskt"], env["qwt"], env["qwbt"]
    out = env["out"]

    with (
        tc.tile_pool(name="aw", bufs=1) as aw,
        tc.tile_pool(name="gath", bufs=2) as gp,
        tc.tile_pool(name="attn", bufs=3) as ap_,
        tc.tile_pool(name="attn3", bufs=3) as ap3,
        tc.tile_pool(name="wvp", bufs=4) as wvp,
        tc.tile_pool(name="xkp", bufs=1) as xkp,
        tc.tile_pool(name="hp", bufs=2) as hp,
        tc.tile_pool(name="outp", bufs=1) as op_,
        tc.tile_pool(name="tpsum", bufs=2, space="PSUM") as tp_,
        tc.tile_pool(name="mpsum", bufs=2, space="PSUM") as mp_,
        tc.tile_pool(name="m0psum", bufs=2, space="PSUM") as mp0,
    ):
        m0w_sb = aw.tile([128, KBLK, HID], dt.bfloat16)
        nc.sync.dma_start(m0w_sb[:], m0w)
        m13_sb = aw.tile([128, 6, HID], dt.bfloat16)
        nc.sync.dma_start(m13_sb[:], m13w)
        m4_sb = aw.tile([128, 2, 3], dt.bfloat16)
        nc.sync.dma_start(m4_sb[:], m4w)
        bm_sb = aw.tile([128, 8], dt.float32)
        nc.sync.dma_start(bm_sb[:], bmlp)
        b4_sb = aw.tile([128, 3], dt.float32)
        nc.sync.dma_start(b4_sb[:], b4)
        id_sb = aw.tile([128, 128], dt.bfloat16)
        nc.sync.dma_start(id_sb[:], ident)
        kvi_sb = aw.tile([128, NT, 16], dt.int16)
        nc.sync.dma_start(kvi_sb[:], kvidx)
        qi_sb = aw.tile([128, NT, 8], dt.int16)
        nc.sync.dma_start(qi_sb[:], qidx)
        mk_sb = aw.tile([128, NT, 56], dt.float32)
        nc.sync.dma_start(mk_sb[:], maskt)
        qw_sb = aw.tile([128, NT, 4], dt.bfloat16)
        nc.sync.dma_start(qw_sb[:], qwt)
        qwb_sb = aw.tile([128, NT, 4], dt.float32)
        nc.sync.dma_start(qwb_sb[:], qwbt)

        out_sb = op_.tile([128, NT, 3], dt.float32)

        kv_ap = kv4[:, :]
        kv_ap = bass.AP(kv_ap.tensor, kv_ap.offset,
                        [[KV_ENT, KVROWS - 6], [1, RR * KV_ENT]])
        q_ap = q2[:, :]
        q_ap = bass.AP(q_ap.tensor, q_ap.offset,
                       [[Q_ENT, QROWS - 2], [1, 2 * Q_ENT]])

        def issue_gathers(t):
            kvg = gp.tile([128, 2, RR * KV_ENT], dt.bfloat16, tag="kvg")
            nc.gpsimd.dma_gather(kvg[:], kv_ap, kvi_sb[:, t, :],
                                 num_idxs=256, num_idxs_reg=256,
                                 elem_size=RR * KV_ENT, elem_step=KV_ENT,
                                 single_packet=False)
            qg = gp.tile([128, 1, 2 * Q_ENT], dt.bfloat16, tag="qg")
            nc.gpsimd.dma_gather(qg[:], q_ap, qi_sb[:, t, :],
                                 num_idxs=128, num_idxs_reg=128,
                                 elem_size=2 * Q_ENT, elem_step=Q_ENT,
                                 single_packet=False)
            return qg, kvg

        groups = _groups(NT)
        base_tiles = [None, None]
        mlp_ps = None
        xchunks = None

        pending = issue_gathers(0)
        for gi, grp in enumerate(groups):
            gw = len(grp)  # tiles in this MLP group (2 or trailing 1)
            for ti, t in enumerate(grp):
                qg, kvg = pending
                if t + 1 < NT:
                    # desc-gen first in the gpsimd queue so the next tile's
                    # gather DMA overlaps this tile's compute fully
                    pending = issue_gathers(t + 1)
                kvv = kvg[:].rearrange("p g (x s e) -> p g x s e", x=RR, s=4)
                qg4 = qg[:].rearrange("p o (x y e) -> p o x y e", x=2, y=2)

                def kslice(dy):
                    g, s = (0, dy) if dy < 4 else (1, dy - 4)
                    return kvv[:, g, :, s, 0:DIM]

                def vslice(dy):
                    g, s = (0, dy) if dy < 4 else (1, dy - 4)
                    return kvv[:, g, :, s, DIM:SUB_E].bitcast(dt.float8e4)

                # ---------------- q vector (bilinear blend) + base ----------
                qprod = ap_.tile([128, 2, 2, DIM], dt.bfloat16, tag="qprod")
                nc.vector.tensor_mul(
                    qprod[:], qg4[:, 0, :, :, 0:DIM],
                    qw_sb[:, t, :].rearrange("p (a b) -> p a b", a=2)
                    .unsqueeze(3).broadcast_to((128, 2, 2, DIM)))
                qt1 = ap_.tile([128, 2, DIM], dt.bfloat16, tag="qt1")
                nc.vector.tensor_add(qt1[:], qprod[:, 0], qprod[:, 1])
                qb = ap_.tile([128, DIM], dt.bfloat16, tag="qb")
                nc.vector.tensor_add(qb[:], qt1[:, 0], qt1[:, 1])

                badd = ap_.tile([128, 2, 2, 3], dt.float32, tag="badd")
                nc.vector.tensor_add(badd[:], qg4[:, 0, :, :, 192:195],
                                     qg4[:, 0, :, :, 195:198])
                bprod = ap_.tile([128, 2, 2, 3], dt.float32, tag="bprod")
                nc.vector.tensor_mul(
                    bprod[:], badd[:],
                    qwb_sb[:, t, :].rearrange("p (a b) -> p a b", a=2)
                    .unsqueeze(3).broadcast_to((128, 2, 2, 3)))
                bt1 = ap_.tile([128, 2, 3], dt.float32, tag="bt1")
                nc.vector.tensor_add(bt1[:], bprod[:, 0], bprod[:, 1])
                base_t = ap_.tile([128, 3], dt.float32, tag="base")
                nc.vector.tensor_add(base_t[:], bt1[:, 0], bt1[:, 1])
                base_tiles[ti] = base_t

                # -------- QK logits: 2 bulk muls + per-group binary trees ----
                # logits layout [128, dy 8 (slot 7 = junk), dx 7, h 8]
                logits = ap3.tile([128, 8, RR, HEAD], dt.float32, tag="logits")
                for g in range(2):
                    e = ap3.tile([128, RR, 4, DIM], dt.bfloat16, tag="emul",
                                 bufs=1)
                    nc.vector.tensor_mul(
                        e[:], kvv[:, g, :, :, 0:DIM],
                        qb[:].unsqueeze(1).unsqueeze(2)
                        .broadcast_to((128, RR, 4, DIM)))
                    ev = e[:].rearrange("p x s (h d) -> p (x s) h d", h=HEAD)
                    l1 = ap3.tile([128, 28, HEAD, 12], dt.bfloat16, tag="l1",
                                  bufs=1)
                    nc.vector.tensor_add(l1[:], ev[:, :, :, 0:12], ev[:, :, :, 12:24])
                    l2 = ap3.tile([128, 28, HEAD, 6], dt.bfloat16, tag="l2",
                                  bufs=1)
                    nc.vector.tensor_add(l2[:], l1[:, :, :, 0:6], l1[:, :, :, 6:12])
                    l3 = ap3.tile([128, 28, HEAD, 3], dt.bfloat16, tag="l3",
                                  bufs=1)
                    nc.vector.tensor_add(l3[:], l2[:, :, :, 0:3], l2[:, :, :, 3:6])
                    l4 = ap3.tile([128, 28, HEAD, 1], dt.bfloat16, tag="l4",
                                  bufs=1)
                    nc.vector.tensor_add(l4[:], l3[:, :, :, 0:1], l3[:, :, :, 1:2])
                    nc.vector.tensor_add(
                        logits[:, g * 4:(g + 1) * 4, :, :]
                        .rearrange("p s x h -> p x s h"),
                        l4[:].rearrange("p (x s) h d -> p x s (h d)", x=RR),
                        l3[:, :, :, 2:3].rearrange("p (x s) h d -> p x s (h d)", x=RR))

                mask_bc = mk_sb[:, t, :].rearrange("p (d x) -> p d x", d=8) \
                    .unsqueeze(3).broadcast_to((128, 8, RR, HEAD))
                nc.vector.tensor_mul(logits[:], logits[:], mask_bc)
                nc.scalar.activation(logits[:], logits[:], AF.Exp)
                ssum = ap_.tile([128, HEAD], dt.float32, tag="ssum")
                nc.vector.reduce_sum(
                    ssum[:], logits[:, 0:RR, :, :].rearrange("p d x h -> p h (d x)"),
                    axis=AX.X)
                rec = ap_.tile([128, HEAD], dt.float32, tag="rec")
                nc.vector.reciprocal(rec[:], ssum[:])
                nc.vector.tensor_mul(
                    logits[:], logits[:],
                    rec[:].unsqueeze(1).unsqueeze(2).broadcast_to((128, 8, RR, HEAD)))
                attnb = ap3.tile([128, 8, RR, HEAD], dt.bfloat16, tag="attnb")
                nc.vector.tensor_mul(attnb[:], logits[:], mask_bc)

                # ---- weighted v -> per-dy chunks; MLP0 accumulates per chunk
                if ti == 0:
                    mlp_ps = mp0.tile([128, 512], dt.float32, tag="mlp0ps",
                                      name="mlp0ps")
                    xchunks = [xkp.tile([128, DY_BLOCKS, 256], dt.bfloat16,
                                        tag=f"xc{dy}", name=f"xc{dy}")
                               for dy in range(RR)]

                def mlp0_chunk(dy):
                    chunk = xchunks[dy]
                    for m in range(2):
                        for b_ in range(DY_BLOCKS):
                            kb = dy * DY_BLOCKS + b_
                            nc.tensor.matmul(
                                mlp_ps[:, m * 256: m * 256 + gw * 128],
                                m0w_sb[:, kb, m * 128:(m + 1) * 128],
                                chunk[:, b_, 0: gw * 128], start=(kb == 0),
                                stop=(kb == KBLK - 1), skip_group_check=True)

                xcol = ti * 128
                for dy in range(RR):
                    wv = wvp.tile([128, DYW_P], dt.bfloat16, tag="wv")
                    nc.gpsimd.memset(wv[:, DYW:], 0.0)
                    weng = nc.gpsimd if dy >= 3 else nc.vector
                    # v channels are stored d-major (c' = d*8 + h) so that all
                    # three operands are innermost-contiguous -> DVE 2x mode
                    weng.tensor_mul(
                        wv[:, 0:DYW].rearrange("p (r d h) -> p r d h", d=HD, h=HEAD),
                        vslice(dy).rearrange("p r (d h) -> p r d h", h=HEAD),
                        attnb[:, dy, :, :].unsqueeze(2).broadcast_to(
                            (128, RR, HD, HEAD)))
                    for grp2, nb in ((0, 8), (1, 3)):
                        tps = tp_.tile([128, nb * 128], dt.bfloat16,
                                       tag=f"tps{grp2}")
                        for bi_ in range(nb):
                            blk = grp2 * 8 + bi_
                            nc.tensor.transpose(tps[:, bi_ * 128:(bi_ + 1) * 128],
                                                wv[:, blk * 128:(blk + 1) * 128],
                                                id_sb[:])
                        nc.scalar.copy(
                            xchunks[dy][:, grp2 * 8: grp2 * 8 + nb, xcol: xcol + 128],
                            tps[:].rearrange("p (a b) -> p a b", a=nb))
                    # emit chunk dy-1's MLP0 matmuls one dy late so the PE is
                    # not stalled on the PSUM->SBUF copy of the current chunk
                    if ti == gw - 1 and dy >= 1:
                        mlp0_chunk(dy - 1)

                # ---------------- MLP layers 0-3 + head (per tile group) -----
                if ti == gw - 1:
                    mlp0_chunk(RR - 1)
                    h0 = hp.tile([128, 2, 256], dt.bfloat16, tag="h")
                    for m in range(2):
                        nc.scalar.activation(h0[:, m, 0: gw * 128],
                                             mlp_ps[:, m * 256: m * 256 + gw * 128],
                                             AF.Relu, bias=bm_sb[:, m:m + 1])
                    cur = h0
                    for l in (1, 2, 3):
                        nxt = hp.tile([128, 2, 256], dt.bfloat16, tag="h")
                        for m in range(2):
                            ps = mp_.tile([128, 256], dt.float32, tag="mlpps")
                            for kc in range(2):
                                nc.tensor.matmul(
                                    ps[:, 0: gw * 128],
                                    m13_sb[:, (l - 1) * 2 + kc,
                                           m * 128:(m + 1) * 128],
                                    cur[:, kc, 0: gw * 128],
                                    start=(kc == 0), stop=(kc == 1))
                            nc.scalar.activation(nxt[:, m, 0: gw * 128],
                                                 ps[:, 0: gw * 128], AF.Relu,
                                                 bias=bm_sb[:, 2 * l + m: 2 * l + m + 1])
                        cur = nxt
                    for tt in range(gw):
                        psof = mp_.tile([128, 256], dt.float32, tag="mlpps")
                        pso = psof[:, 0:3]
                        for kc in range(2):
                            nc.tensor.matmul(pso,
                                             cur[:, kc, tt * 128: tt * 128 + 128],
                                             m4_sb[:, kc, :],
                                             start=(kc == 0), stop=(kc == 1))
                        o1 = ap_.tile([128, 3], dt.float32, tag="o1")
                        nc.vector.tensor_add(o1[:], pso, b4_sb[:])
                        nc.vector.tensor_add(out_sb[:, grp[tt], :], o1[:],
                                             base_tiles[tt][:])

        nc.sync.dma_start(
            out.rearrange("(t p) c -> p t c", p=128), out_sb[:])


# ============================ host preparation ==============================

def _host_prep(inputs):
    inp = np.asarray(inputs["inp"], f32)
    sc = np.asarray(inputs["sample_coord"], f32)
    cell = np.asarray(inputs["cell"], f32)

    enc_w = np.asarray(inputs["enc_w"], f32)
    ch_w = np.asarray(inputs["ch_w"], f32)

    w_enc = enc_w.transpose(1, 2, 3, 0).reshape(27, ENC).astype(bf16)
    w_chp = np.zeros((128, 3, DIM), bf16)
    w_ch2 = np.zeros((ENC, 3, DIM), bf16)
    for ky in range(3):
        w_chp[0:64, ky, :] = ch_w[:, :, ky, 0].T.astype(bf16)
        w_chp[64:128, ky, :] = ch_w[:, :, ky, 1].T.astype(bf16)
        w_ch2[:, ky, :] = ch_w[:, :, ky, 2].T.astype(bf16)

    # qkv weights M-packed: 576 output cols = [q 192 | k 192 | v 192]
    w_qkv0 = np.zeros((128, 9, 576), bf16)
    w_qkv1p = np.zeros((128, 3, 576), bf16)
    w_qkv1k2 = np.zeros((64, 3, 576), bf16)
    qkvb = np.zeros((128, 5), f32)
    # v output channels permuted d-major: device channel c' = d*8+h holds
    # reference channel h*24+d (lets the wv multiply run innermost-contiguous)
    cmap_v = (np.arange(192) % 8) * 24 + np.arange(192) // 8
    for ci, nm in enumerate(("q", "k", "v")):
        wt = np.asarray(inputs[f"{nm}_w"], f32)
        bt = np.asarray(inputs[f"{nm}_b"], f32)
        if nm == "v":
            wt = wt[cmap_v]
            bt = bt[cmap_v]
        cs_ = ci * 192
        for off in range(9):
            ky, kx = off // 3, off % 3
            w_qkv0[:, off, cs_: cs_ + 192] = wt[:, 0:128, ky, kx].T.astype(bf16)
        for ky in range(3):
            w_qkv1p[0:64, ky, cs_: cs_ + 192] = wt[:, 128:192, ky, 0].T.astype(bf16)
            w_qkv1p[64:128, ky, cs_: cs_ + 192] = wt[:, 128:192, ky, 1].T.astype(bf16)
            w_qkv1k2[:, ky, cs_: cs_ + 192] = wt[:, 128:192, ky, 2].T.astype(bf16)
        bfull = bt
        for g in range(5):
            msz = 128 if g < 4 else 64
            seg = np.arange(g * 128, g * 128 + msz)
            sel = (seg >= cs_) & (seg < cs_ + 192)
            if sel.any():
                qkvb[np.nonzero(sel)[0], g] = bfull[seg[sel] - cs_]

    # m0w rows permuted into 7 chunks of 1344 -> 1408 (zero padded); within a
    # chunk the device feature order is (dx, d, h): j = dx*192 + d*8 + h maps
    # to reference row (dy*7+dx)*192 + h*24 + d
    m0w_full = np.asarray(inputs["m0w"], f32)
    jj = np.arange(DYW)
    jdx, jc = jj // 192, jj % 192
    jd, jh = jc // 8, jc % 8
    ref_j = jdx * 192 + jh * 24 + jd
    perm = np.zeros((KBLK * 128, HID), f32)
    for i in range(RR):
        perm[i * DYW_P: i * DYW_P + DYW] = m0w_full[i * DYW + ref_j]
    m0w_dev = np.ascontiguousarray(
        perm.reshape(KBLK, 128, HID).transpose(1, 0, 2)).astype(bf16)

    m13w = np.zeros((128, 6, HID), bf16)
    for l in (1, 2, 3):
        wl = np.asarray(inputs[f"m{l}w"], f32)
        m13w[:, (l - 1) * 2 + 0, :] = wl[0:128].astype(bf16)
        m13w[:, (l - 1) * 2 + 1, :] = wl[128:256].astype(bf16)
    m4w_full = np.asarray(inputs["m4w"], f32)
    m4w = np.stack([m4w_full[0:128], m4w_full[128:256]], 1).astype(bf16)

    b4 = np.broadcast_to(np.asarray(inputs["m4b"], f32)[None, :], (128, 3)).copy()
    enc_bd = np.asarray(inputs["enc_b"], f32).reshape(ENC, 1)
    ch_bd = np.zeros((128, 2), f32)
    ch_bd[:, 0] = np.asarray(inputs["ch_b"], f32)[0:128]
    ch_bd[0:64, 1] = np.asarray(inputs["ch_b"], f32)[128:192]
    ident = np.eye(128, dtype=bf16)

    m0b = np.asarray(inputs["m0b"], f32)
    m0w_tail = m0w_full[RA * DIM: RA * DIM + 2]
    bias_rest = np.zeros((128, 8), f32)
    for l in (1, 2, 3):
        bl = np.asarray(inputs[f"m{l}b"], f32)
        bias_rest[:, 2 * l + 0] = bl[0:128]
        bias_rest[:, 2 * l + 1] = bl[128:256]

    # ---- ownership: core (bi, qc) owns queries with iy//16 == qc ----
    sqh = f32(1.0 / np.sqrt(HD))
    cy_all, cx_all = sc[..., 0], sc[..., 1]
    py_all = (cy_all + f32(1.0)) * f32(H * 0.5) - f32(0.5)
    px_all = (cx_all + f32(1.0)) * f32(W * 0.5) - f32(0.5)
    iy_all = np.clip(np.floor(py_all + f32(0.5)), 0, H - 1).astype(np.int64)
    ix_all = np.clip(np.floor(px_all + f32(0.5)), 0, W - 1).astype(np.int64)

    owners = []
    for core in range(N_CORES):
        bi, qc = core // 4, core % 4
        own = np.nonzero(iy_all[bi] // 16 == qc)[0]
        owners.append(own)
    NT = max((len(o) + 127) // 128 for o in owners)
    SLOTS = NT * 128

    batch_data = []
    for bi in range(B):
        x = inp[bi]
        xT = x.reshape(3, H * W).T
        hi = xT.astype(bf16).astype(f32)
        lo = (xT - hi).astype(bf16)
        hilo_full = np.concatenate([hi.astype(bf16), lo], 1)  # [4096, 6]

        rel_cell = cell[bi] * np.array([H, W], f32)
        b0 = m0b + rel_cell @ m0w_tail
        bm = bias_rest.copy()
        bm[:, 0] = b0[0:128]
        bm[:, 1] = b0[128:256]
        batch_data.append((x, hilo_full, bm))

    d = np.arange(-R, R + 1)
    percore = []
    for core in range(N_CORES):
        bi, qc = core // 4, core % 4
        R0 = 16 * qc
        own = owners[core]
        n = len(own)
        x, hilo_full, bm = batch_data[bi]

        # padded image: padded row p <-> image row R0 + p - 35
        xbig = np.zeros((CH_IN, 128, W + 2), f32)
        plo, phi = max(0, 35 - R0), min(128, 35 - R0 + H)
        xbig[:, plo:phi, 1:1 + W] = x[:, plo - 35 + R0: phi - 35 + R0, :]

        # im2col for enc strips 2..7 (rows 16..63)
        col = np.zeros((27, 6 * 512), bf16)
        for c in range(CH_IN):
            for ky in range(3):
                for kx in range(3):
                    col[c * 9 + ky * 3 + kx] = \
                        xbig[c, 15 + ky: 63 + ky, kx: kx + W] \
                        .reshape(-1).astype(bf16)
        tr = np.arange(16, 64) - 35 + R0  # image rows of enc output strips
        col.reshape(27, 48, W)[:, (tr < 0) | (tr >= H), :] = 0

        # ch rows 31..56 mask (image rows R0-4 .. R0+21)
        trf = np.arange(26) + R0 - 4
        maskfp = np.broadcast_to(
            ((trf >= 0) & (trf < H)).astype(bf16)[None, :], (128, 26)).copy()

        # hilo for local pixel rows 0..23 (image rows R0-3 .. R0+20)
        hl = np.zeros((NPX, 6), bf16)
        rlo, rhi = max(0, R0 - 3), min(H, R0 + 21)
        hl[(rlo - R0 + 3) * W: (rhi - R0 + 3) * W] = \
            hilo_full[rlo * W: rhi * W]
        hilo = np.ascontiguousarray(hl.reshape(12, 128, 6).transpose(1, 0, 2))

        # ---- per-query gather indices / weights ----
        iy, ix = iy_all[bi][own], ix_all[bi][own]
        py, px = py_all[bi][own], px_all[bi][own]

        dy_, dx_ = [a.reshape(-1) for a in np.meshgrid(d, d, indexing="ij")]
        yy = iy[:, None] + dy_[None, :]
        xx = ix[:, None] + dx_[None, :]
        valid = ((yy >= 0) & (yy < H) & (xx >= 0) & (xx < W)).astype(f32)

        # kv entry index for group g: entry (iy - R0 + 4g)*64 + ix - 3,
        # tensor row = HEADKV + entry = 192 + (iy-R0+4g)*64 + ix
        kvstart = np.zeros((n, 2), np.int64)
        for g in range(2):
            kvstart[:, g] = 192 + (iy - R0 + 4 * g) * 64 + ix

        y0 = np.floor(py)
        x0 = np.floor(px)
        wy, wx = py - y0, px - x0
        y0 = y0.astype(np.int64)
        x0 = x0.astype(np.int64)
        sy0 = np.clip(y0, 0, H - 2)
        sx0 = np.clip(x0, 0, W - 2)
        wq_eff = np.zeros((n, 2, 2), f32)
        wb_eff = np.zeros((n, 2, 2), f32)
        qq = np.arange(n)
        for ddy, syw in ((0, 1 - wy), (1, wy)):
            for ddx, sxw in ((0, 1 - wx), (1, wx)):
                w = (syw * sxw).astype(f32)
                yc, xc = y0 + ddy, x0 + ddx
                ly = np.clip(yc, 0, H - 1) - sy0
                lx = np.clip(xc, 0, W - 1) - sx0
                wb_eff[qq, ly, lx] += w
                vm = ((yc >= 0) & (yc < H) & (xc >= 0) & (xc < W))
                wq_eff[qq, ly, lx] += w * vm
        # qg4 layout is [x(sx), y(sy)] -> reorder weights to (lx, ly)
        wq4 = wq_eff.transpose(0, 2, 1).reshape(n, 4)
        wb4 = wb_eff.transpose(0, 2, 1).reshape(n, 4)
        qstart = HEADQ + (sy0 - R0 + 3) * 64 + sx0

        # pad to SLOTS
        def padto(a, fill):
            outp = np.full((SLOTS,) + a.shape[1:], fill, a.dtype)
            outp[:n] = a
            return outp

        kvstart_p = padto(kvstart, 192)
        qstart_p = padto(qstart, HEADQ + 128)
        valid_p = padto(valid, 0.0)
        wq4_p = padto(wq4.astype(f32), 0.0)
        wb4_p = padto(wb4, 0.0)

        kvidx = np.zeros((128, NT, 16), np.int16)
        qidx = np.zeros((128, NT, 8), np.int16)
        masktt = np.zeros((128, NT, 56), f32)
        qwt = np.zeros((128, NT, 4), bf16)
        qwbt = np.zeros((128, NT, 4), f32)
        for t in range(NT):
            ts = slice(t * 128, (t + 1) * 128)
            masktt[:, t, 0:RA] = valid_p[ts]  # dy-major [8 dy (pad), 7 dx]
            qwt[:, t, :] = (wq4_p[ts] * sqh).astype(bf16)
            qwbt[:, t, :] = wb4_p[ts]
            flat = kvstart_p[ts].T.reshape(-1)  # j = g*128 + q
            kvidx[:, t, :] = np.tile(flat.reshape(-1, 16).T, (8, 1)).astype(np.int16)
            fq = qstart_p[ts]
            qidx[:, t, :] = np.tile(fq.reshape(-1, 16).T, (8, 1)).astype(np.int16)

        percore.append({
            "inp_col": col, "inp_hilo": hilo,
            "w_enc": w_enc, "w_chp": w_chp, "w_ch2": w_ch2,
            "w_qkv0": w_qkv0, "w_qkv1p": w_qkv1p, "w_qkv1k2": w_qkv1k2,
            "qkvb": qkvb, "maskfp": maskfp, "enc_b": enc_bd, "ch_b": ch_bd,
            "m0w": m0w_dev, "m13w": m13w, "m4w": m4w, "bmlp": bm, "b4": b4,
            "ident": ident, "kvidx": kvidx, "qidx": qidx, "maskt": masktt,
            "qwt": qwt, "qwbt": qwbt,
        })
    return percore, NT, owners


# ============================== entry point =================================

def _get_program(NT):
    if NT not in _PROGRAMS:
        _PROGRAMS[NT] = build_program(NT)
    return _PROGRAMS[NT]


def kernel(**inputs):
    from concourse import bass_utils
    in_maps, NT, owners = _host_prep(inputs)
    nc = _get_program(NT)
    res = bass_utils.run_bass_kernel_spmd(nc, in_maps, core_ids=list(range(N_CORES)))
    full = np.empty((B, Q, 3), f32)
    for core in range(N_CORES):
        bi = core // 4
        own = owners[core]
        full[bi, own] = res.results[core]["out"][:len(own)]
    return full


if __name__ == "__main__":
    import time
    t0 = time.time()
    nc = _get_program(9)
    print("built+compiled in", time.time() - t0, "s")


# revision 22
# speedup vs baseline: 1.0091x; 1.0091x over previous
"""Trainium2 Bass kernel for the CLIT-style sparse local attention module.

Strategy (8 NeuronCores, SPMD, no collectives):
  - core c = (batch bi = c // 4, strip qc = c % 4) OWNS the queries whose
    nearest-pixel center row falls in image rows [16qc, 16qc+16). Each core
    computes the 5 convs locally for the 24-row band [16qc-3, 16qc+21)
    (own 16 rows + 3-row halo each side + 2 spare), so every window/bilinear
    read its queries need is produced locally -- the AllGather disappears.
  - qkv conv outputs are PE-transposed to pixel-major entries in local DRAM
    with a 4x row-shifted duplication: entry (r, x) holds rows r..r+3 of
    column x back to back. A 7x7 window then needs only TWO dma_gather
    descriptors per query (4+3 window rows each), and the bilinear q read
    needs ONE (2x2 patch via 2x row duplication).
  - v is stored as fp8e4 (k stays bf16): halves the v gather traffic; the
    wv multiply runs at DVE 1x anyway (broadcast attn operand), so fp8
    costs nothing there. rel-err budget is ~2e-2; bf16 baseline is ~8e-5.
  - Attention per 128-query tile: QK logits as DVE mul + binary-tree adds
    (tensor_reduce is 1x-mode and slow), softmax, attention-weighted v
    (split DVE/GpSimd), PE-transposed into the K-major layout for the
    9410x256 MLP0 (bf16, fp32 PSUM), then MLP1-3 + head per tile group.
  - Host precomputes ownership, gather indices, bilinear weights, masks;
    kernel() scatters per-core outputs back to the original query order.
"""

import sys

sys.path.insert(0, "/opt/trn_rl_repo")

import numpy as np
import ml_dtypes

# ---------------- problem constants (hardcoded per contract) ----------------
B, CH_IN, H, W = 2, 3, 64, 64
Q = 4096
DIM, HEAD, R = 192, 8, 3
RR = 2 * R + 1
RA = RR * RR          # 49
HD = DIM // HEAD      # 24
ENC = 64
HID = 256
N_CORES = 8

NROWS = 24            # local conv band rows (strip 16 + 3 halo + 2 spare)
NPX = NROWS * W       # 1536 pixel entries
SUB_E = 288           # bf16 slots per kv sub-entry: 192 k bf16 + 192 v fp8
KV_ENT = 4 * SUB_E    # 1152: entry holds rows r..r+3
HEADKV = 195          # 192 write-shift room + 3 px guard
KVROWS = HEADKV + NPX # 1731
Q_ENT = 2 * 256       # q entry holds rows r..r+1
HEADQ = 64
QROWS = HEADQ + NPX   # 1600

DYW = RR * DIM                    # 1344 columns per window-row chunk
DYW_P = 1408                      # padded to 11 x 128
DY_BLOCKS = DYW_P // 128          # 11
KBLK = RR * DY_BLOCKS             # 77 K-blocks for MLP layer 0

f32 = np.float32
bf16 = ml_dtypes.bfloat16

_PROGRAMS = {}  # cached compiled Bass programs keyed by NT


def _groups(nt):
    gs = []
    t = 0
    while t < nt:
        gs.append(tuple(range(t, min(t + 2, nt))))
        t += 2
    return gs


# ============================ device program ================================

def build_program(NT):
    import concourse.bass as bass
    import concourse.tile as tile
    from concourse import bacc, mybir

    dt = mybir.dt

    nc = bacc.Bacc("TRN2", target_bir_lowering=False, debug=False,
                   enable_asserts=False, num_devices=N_CORES)

    def din(name, shape, dtype):
        return nc.dram_tensor(name, list(shape), dtype, kind="ExternalInput").ap()

    inp_col = din("inp_col", [27, 6 * 512], dt.bfloat16)
    inp_hilo = din("inp_hilo", [128, 12, 6], dt.bfloat16)
    w_enc = din("w_enc", [27, ENC], dt.bfloat16)
    w_chp = din("w_chp", [128, 3, DIM], dt.bfloat16)
    w_ch2 = din("w_ch2", [ENC, 3, DIM], dt.bfloat16)
    w_qkv0 = din("w_qkv0", [128, 9, 576], dt.bfloat16)
    w_qkv1p = din("w_qkv1p", [128, 3, 576], dt.bfloat16)
    w_qkv1k2 = din("w_qkv1k2", [64, 3, 576], dt.bfloat16)
    qkvb = din("qkvb", [128, 5], dt.float32)
    maskfp = din("maskfp", [128, 26], dt.bfloat16)
    enc_b = din("enc_b", [ENC, 1], dt.float32)
    ch_b = din("ch_b", [128, 2], dt.float32)
    m0w = din("m0w", [128, KBLK, HID], dt.bfloat16)
    m13w = din("m13w", [128, 6, HID], dt.bfloat16)
    m4w = din("m4w", [128, 2, 3], dt.bfloat16)
    bmlp = din("bmlp", [128, 8], dt.float32)
    b4 = din("b4", [128, 3], dt.float32)
    ident = din("ident", [128, 128], dt.bfloat16)
    kvidx = din("kvidx", [128, NT, 16], dt.int16)
    qidx = din("qidx", [128, NT, 8], dt.int16)
    maskt = din("maskt", [128, NT, 56], dt.float32)
    qwt = din("qwt", [128, NT, 4], dt.bfloat16)
    qwbt = din("qwbt", [128, NT, 4], dt.float32)
    out = nc.dram_tensor("out", [NT * 128, 3], dt.float32,
                         kind="ExternalOutput").ap()

    with tile.TileContext(nc) as tc:
        with tc.tile_pool(name="dram", bufs=1, space="DRAM") as dp:
            kv4 = dp.tile([KVROWS, KV_ENT], dt.bfloat16)
            q2 = dp.tile([QROWS, Q_ENT], dt.bfloat16)

            _convs(nc, tc, mybir, locals())
            _attention(nc, tc, mybir, NT, locals())

    nc.compile()
    return nc


def _convs(nc, tc, mybir, env):
    dt = mybir.dt
    AF = mybir.ActivationFunctionType

    inp_col, w_enc = env["inp_col"], env["w_enc"]
    w_chp, w_ch2 = env["w_chp"], env["w_ch2"]
    w_qkv0, w_qkv1p, w_qkv1k2 = env["w_qkv0"], env["w_qkv1p"], env["w_qkv1k2"]
    qkvb = env["qkvb"]
    enc_b, ch_b, inp_hilo = env["enc_b"], env["ch_b"], env["inp_hilo"]
    kv4, q2 = env["kv4"], env["q2"]

    with (
        tc.tile_pool(name="cw", bufs=1) as cw,
        tc.tile_pool(name="cfeat", bufs=1) as cf,
        tc.tile_pool(name="cpsum", bufs=2, space="PSUM") as cp,
        tc.tile_pool(name="qpsum", bufs=2, space="PSUM") as cpq,
        tc.tile_pool(name="ctpsum", bufs=2, space="PSUM") as cpt,
        tc.tile_pool(name="cstage", bufs=3) as cs,
    ):
        col_sb = cw.tile([27, 6 * 512], dt.bfloat16)
        nc.sync.dma_start(col_sb[:], inp_col)
        wenc_sb = cw.tile([27, ENC], dt.bfloat16)
        nc.sync.dma_start(wenc_sb[:], w_enc)
        wchp_sb = cw.tile([128, 3, DIM], dt.bfloat16)
        nc.sync.dma_start(wchp_sb[:], w_chp)
        wch2_sb = cw.tile([ENC, 3, DIM], dt.bfloat16)
        nc.sync.dma_start(wch2_sb[:], w_ch2)
        wq0 = cw.tile([128, 9, 576], dt.bfloat16)
        nc.sync.dma_start(wq0[:], w_qkv0)
        wq1p = cw.tile([128, 3, 576], dt.bfloat16)
        nc.sync.dma_start(wq1p[:], w_qkv1p)
        wq1k2 = cw.tile([64, 3, 576], dt.bfloat16)
        nc.sync.dma_start(wq1k2[:], w_qkv1k2)
        qkvb_sb = cw.tile([128, 5], dt.float32)
        nc.sync.dma_start(qkvb_sb[:], qkvb)
        encb_sb = cw.tile([ENC, 1], dt.float32)
        nc.sync.dma_start(encb_sb[:], enc_b)
        chb_sb = cw.tile([128, 2], dt.float32)
        nc.sync.dma_start(chb_sb[:], ch_b)
        hilo_sb = cw.tile([128, 12, 6], dt.bfloat16)
        nc.sync.dma_start(hilo_sb[:], inp_hilo)
        id_c = cw.tile([128, 128], dt.bfloat16)
        nc.sync.dma_start(id_c[:], env["ident"])
        maskfp_sb = cw.tile([128, 26], dt.bfloat16)
        nc.sync.dma_start(maskfp_sb[:], env["maskfp"])

        # zero the kv head region (write-shift room + guard entries): rows
        # 0..HEADKV of kv4; shifted writes partially overwrite it afterwards.
        zt = cw.tile([128, 1755], dt.bfloat16)
        nc.vector.memset(zt[:], 0.0)
        kvf = kv4[:, :].flatten()
        nc.sync.dma_start(kvf[0: HEADKV * KV_ENT]
                          .rearrange("(p a) -> p a", p=128), zt[:])

        encp = cf.tile([ENC, 66, 66], dt.bfloat16)
        nc.vector.memset(encp[:], 0.0)
        fp0 = cf.tile([128, 66, 66], dt.bfloat16)
        nc.vector.memset(fp0[:], 0.0)
        fp1 = cf.tile([64, 66, 66], dt.bfloat16)
        nc.vector.memset(fp1[:], 0.0)
        encb2 = cf.tile([128, 66, 66], dt.bfloat16)
        fp1b = cf.tile([128, 66, 66], dt.bfloat16)

        # ---- enc conv (strips 2-7) ----
        for t in range(2, 8):
            ps = cp.tile([128, 512], dt.float32, tag="cps")
            nc.tensor.matmul(ps[:ENC, :], wenc_sb[:],
                             col_sb[:, (t - 2) * 512:(t - 1) * 512],
                             start=True, stop=True)
            dst = encp[:, 1 + t * 8: 1 + t * 8 + 8, 1:65]
            nc.scalar.activation(dst, ps[:ENC, :].rearrange("p (a b) -> p a b", a=8),
                                 AF.Identity, bias=encb_sb[:, 0:1])

        # encb2: enc features with a one-column-shifted copy in partitions 64:
        nc.vector.tensor_copy(encb2[0:64, 24:66, :], encp[:, 24:66, :])
        nc.vector.tensor_copy(encb2[64:128, 24:66, 0:65], encp[:, 24:66, 1:66])

        # ---- ch conv (strips 3-7; kx 0/1 paired into K=128, kx=2 single) ----
        for t in range(3, 8):
            for m, msz in ((0, 128), (1, 64)):
                ps = cp.tile([128, 512], dt.float32, tag="cps")
                for ky in range(3):
                    rhs = encb2[:, t * 8 + ky: t * 8 + ky + 8, 0:64]
                    nc.tensor.matmul(ps[:msz, :],
                                     wchp_sb[:, ky, m * 128: m * 128 + msz],
                                     rhs, start=(ky == 0), stop=False)
                for ky in range(3):
                    rhs = encp[:, t * 8 + ky: t * 8 + ky + 8, 2:66]
                    nc.tensor.matmul(ps[:msz, :],
                                     wch2_sb[:, ky, m * 128: m * 128 + msz],
                                     rhs, start=False, stop=(ky == 2))
                dstp = (fp0 if m == 0 else fp1)
                dst = dstp[:msz, 1 + t * 8: 1 + t * 8 + 8, 1:65]
                nc.scalar.activation(dst,
                                     ps[:msz, :].rearrange("p (a b) -> p a b", a=8),
                                     AF.Identity, bias=chb_sb[:msz, m: m + 1])

        # zero ch features of out-of-image rows (qkv convs read fp rows 32..57)
        mbc = maskfp_sb[:].unsqueeze(2).broadcast_to((128, 26, 66))
        nc.vector.tensor_mul(fp0[:, 32:58, :], fp0[:, 32:58, :], mbc)
        nc.vector.tensor_mul(fp1[:, 32:58, :], fp1[:, 32:58, :],
                             maskfp_sb[0:64, :].unsqueeze(2)
                             .broadcast_to((64, 26, 66)))

        # fp1b: channel-chunk-1 features with one-column-shifted copy
        nc.vector.tensor_copy(fp1b[0:64, 32:58, :], fp1[:, 32:58, :])
        nc.vector.tensor_copy(fp1b[64:128, 32:58, 0:65], fp1[:, 32:58, 1:66])

        # ---- q/k/v convs (strips 4-6; M-packed 576 = 4x128+64) + transpose ----
        # 576 cols = [q 0:192 | k 192:384 | v 384:576]
        # per group: list of (dst_kind, src_off, dst_off, n)
        #   kind 0 = qstage bf16, 1 = kvstage k bf16, 2 = kvstage v fp8 view
        gdst = [
            [(0, 0, 0, 128)],
            [(0, 0, 128, 64), (1, 64, 0, 64)],
            [(1, 0, 64, 128)],
            [(2, 0, 0, 128)],
            [(2, 0, 128, 64)],
        ]
        for t in (4, 5, 6):
            kvstage = cs.tile([128, 4, SUB_E], dt.bfloat16, tag="kvstage")
            qstage = cs.tile([128, 4, 256], dt.bfloat16, tag="qstage")
            vview = kvstage[:, :, 192:SUB_E].bitcast(dt.float8e4)
            nc.vector.memset(qstage[:, :, 198:], 0.0)
            nc.vector.tensor_copy(qstage[:, :, 192:198],
                                  hilo_sb[:, (t - 4) * 4:(t - 3) * 4, :])
            for g in range(5):
                msz = 128 if g < 4 else 64
                ps = cpq.tile([128, 512], dt.float32, tag="qkvps")
                for off in range(9):
                    ky, kx = off // 3, off % 3
                    rhs0 = fp0[:, t * 8 + ky: t * 8 + ky + 8, kx: kx + 64]
                    nc.tensor.matmul(ps[:msz, :],
                                     wq0[:, off, g * 128: g * 128 + msz],
                                     rhs0, start=(off == 0), stop=False)
                for ky in range(3):
                    rhs1 = fp1b[:, t * 8 + ky: t * 8 + ky + 8, 0:64]
                    nc.tensor.matmul(ps[:msz, :],
                                     wq1p[:, ky, g * 128: g * 128 + msz],
                                     rhs1, start=False, stop=False)
                for ky in range(3):
                    rhs1 = fp1[:, t * 8 + ky: t * 8 + ky + 8, 2:66]
                    nc.tensor.matmul(ps[:msz, :],
                                     wq1k2[:, ky, g * 128: g * 128 + msz],
                                     rhs1, start=False, stop=(ky == 2))
                csb = cs.tile([128, 512], dt.bfloat16, tag="convsb")
                nc.scalar.activation(csb[:msz, :], ps[:msz, :], AF.Identity,
                                     bias=qkvb_sb[:msz, g: g + 1])
                tps = cpt.tile([128, 512], dt.bfloat16, tag="ctps")
                for blk in range(4):
                    nc.tensor.transpose(
                        tps[:, blk * 128: blk * 128 + msz],
                        csb[:msz, blk * 128:(blk + 1) * 128],
                        id_c[:msz, :msz])
                tview = tps[:].rearrange("p (a b) -> p a b", a=4)
                for kind, so, do, n in gdst[g]:
                    if kind == 0:
                        nc.scalar.copy(qstage[:, :, do: do + n],
                                       tview[:, :, so: so + n])
                    elif kind == 1:
                        nc.scalar.copy(kvstage[:, :, do: do + n],
                                       tview[:, :, so: so + n])
                    else:
                        nc.scalar.copy(vview[:, :, do: do + n],
                                       tview[:, :, so: so + n])
            # 4x / 2x row-shifted duplicated writes: entry (r, x) sub s holds
            # local row r+s. Strip t covers local rows LT..LT+7.
            LT = (t - 4) * 8
            for s in range(4):
                r0 = HEADKV + (LT - s) * 64
                nc.sync.dma_start(
                    kv4[r0: r0 + 512, s * SUB_E: (s + 1) * SUB_E]
                    .rearrange("(b p) e -> p b e", p=128), kvstage[:])
            for s in range(2):
                r0 = HEADQ + (LT - s) * 64
                nc.sync.dma_start(
                    q2[r0: r0 + 512, s * 256: (s + 1) * 256]
                    .rearrange("(b p) e -> p b e", p=128), qstage[:])


def _attention(nc, tc, mybir, NT, env):
    import concourse.bass as bass
    dt = mybir.dt
    AX = mybir.AxisListType
    AF = mybir.ActivationFunctionType

    kv4, q2 = env["kv4"], env["q2"]
    m0w, m13w, m4w = env["m0w"], env["m13w"], env["m4w"]
    bmlp, b4, ident = env["bmlp"], env["b4"], env["ident"]
    kvidx, qidx = env["kvidx"], env["qidx"]
    maskt, qwt, qwbt = env["maskt"], env["qwt"], env["qwbt"]
    out = env["out"]

    with (
        tc.tile_pool(name="aw", bufs=1) as aw,
        tc.tile_pool(name="gath", bufs=2) as gp,
        tc.tile_pool(name="attn", bufs=3) as ap_,
        tc.tile_pool(name="attn3", bufs=3) as ap3,
        tc.tile_pool(name="wvp", bufs=6) as wvp,
        tc.tile_pool(name="xkp", bufs=1) as xkp,
        tc.tile_pool(name="hp", bufs=2) as hp,
        tc.tile_pool(name="outp", bufs=1) as op_,
        tc.tile_pool(name="tpsum", bufs=2, space="PSUM") as tp_,
        tc.tile_pool(name="mpsum", bufs=2, space="PSUM") as mp_,
        tc.tile_pool(name="m0psum", bufs=2, space="PSUM") as mp0,
    ):
        m0w_sb = aw.tile([128, KBLK, HID], dt.bfloat16)
        nc.sync.dma_start(m0w_sb[:], m0w)
        m13_sb = aw.tile([128, 6, HID], dt.bfloat16)
        nc.sync.dma_start(m13_sb[:], m13w)
        m4_sb = aw.tile([128, 2, 3], dt.bfloat16)
        nc.sync.dma_start(m4_sb[:], m4w)
        bm_sb = aw.tile([128, 8], dt.float32)
        nc.sync.dma_start(bm_sb[:], bmlp)
        b4_sb = aw.tile([128, 3], dt.float32)
        nc.sync.dma_start(b4_sb[:], b4)
        id_sb = aw.tile([128, 128], dt.bfloat16)
        nc.sync.dma_start(id_sb[:], ident)
        kvi_sb = aw.tile([128, NT, 16], dt.int16)
        nc.sync.dma_start(kvi_sb[:], kvidx)
        qi_sb = aw.tile([128, NT, 8], dt.int16)
        nc.sync.dma_start(qi_sb[:], qidx)
        mk_sb = aw.tile([128, NT, 56], dt.float32)
        nc.sync.dma_start(mk_sb[:], maskt)
        qw_sb = aw.tile([128, NT, 4], dt.bfloat16)
        nc.sync.dma_start(qw_sb[:], qwt)
        qwb_sb = aw.tile([128, NT, 4], dt.float32)
        nc.sync.dma_start(qwb_sb[:], qwbt)

        out_sb = op_.tile([128, NT, 3], dt.float32)

        kv_ap = kv4[:, :]
        kv_ap = bass.AP(kv_ap.tensor, kv_ap.offset,
                        [[KV_ENT, KVROWS - 6], [1, RR * KV_ENT]])
        q_ap = q2[:, :]
        q_ap = bass.AP(q_ap.tensor, q_ap.offset,
                       [[Q_ENT, QROWS - 2], [1, 2 * Q_ENT]])

        def issue_gathers(t):
            kvg = gp.tile([128, 2, RR * KV_ENT], dt.bfloat16, tag="kvg")
            nc.gpsimd.dma_gather(kvg[:], kv_ap, kvi_sb[:, t, :],
                                 num_idxs=256, num_idxs_reg=256,
                                 elem_size=RR * KV_ENT, elem_step=KV_ENT,
                                 single_packet=False)
            qg = gp.tile([128, 1, 2 * Q_ENT], dt.bfloat16, tag="qg")
            nc.gpsimd.dma_gather(qg[:], q_ap, qi_sb[:, t, :],
                                 num_idxs=128, num_idxs_reg=128,
                                 elem_size=2 * Q_ENT, elem_step=Q_ENT,
                                 single_packet=False)
            return qg, kvg

        groups = _groups(NT)
        base_tiles = [None, None]
        mlp_ps = None
        xchunks = None

        pending = issue_gathers(0)
        for gi, grp in enumerate(groups):
            gw = len(grp)  # tiles in this MLP group (2 or trailing 1)
            for ti, t in enumerate(grp):
                qg, kvg = pending
                if t + 1 < NT:
                    # desc-gen first in the gpsimd queue so the next tile's
                    # gather DMA overlaps this tile's compute fully
                    pending = issue_gathers(t + 1)
                kvv = kvg[:].rearrange("p g (x s e) -> p g x s e", x=RR, s=4)
                qg4 = qg[:].rearrange("p o (x y e) -> p o x y e", x=2, y=2)

                def kslice(dy):
                    g, s = (0, dy) if dy < 4 else (1, dy - 4)
                    return kvv[:, g, :, s, 0:DIM]

                def vslice(dy):
                    g, s = (0, dy) if dy < 4 else (1, dy - 4)
                    return kvv[:, g, :, s, DIM:SUB_E].bitcast(dt.float8e4)

                # ---------------- q vector (bilinear blend) + base ----------
                qprod = ap_.tile([128, 2, 2, DIM], dt.bfloat16, tag="qprod")
                nc.vector.tensor_mul(
                    qprod[:], qg4[:, 0, :, :, 0:DIM],
                    qw_sb[:, t, :].rearrange("p (a b) -> p a b", a=2)
                    .unsqueeze(3).broadcast_to((128, 2, 2, DIM)))
                qt1 = ap_.tile([128, 2, DIM], dt.bfloat16, tag="qt1")
                nc.vector.tensor_add(qt1[:], qprod[:, 0], qprod[:, 1])
                qb = ap_.tile([128, DIM], dt.bfloat16, tag="qb")
                nc.vector.tensor_add(qb[:], qt1[:, 0], qt1[:, 1])

                badd = ap_.tile([128, 2, 2, 3], dt.float32, tag="badd")
                nc.vector.tensor_add(badd[:], qg4[:, 0, :, :, 192:195],
                                     qg4[:, 0, :, :, 195:198])
                bprod = ap_.tile([128, 2, 2, 3], dt.float32, tag="bprod")
                nc.vector.tensor_mul(
                    bprod[:], badd[:],
                    qwb_sb[:, t, :].rearrange("p (a b) -> p a b", a=2)
                    .unsqueeze(3).broadcast_to((128, 2, 2, 3)))
                bt1 = ap_.tile([128, 2, 3], dt.float32, tag="bt1")
                nc.vector.tensor_add(bt1[:], bprod[:, 0], bprod[:, 1])
                base_t = ap_.tile([128, 3], dt.float32, tag="base")
                nc.vector.tensor_add(base_t[:], bt1[:, 0], bt1[:, 1])
                base_tiles[ti] = base_t

                # -------- QK logits: 2 bulk muls + per-group binary trees ----
                # logits layout [128, g 2, x 7, s 4, h 8]; (g1, s3) = junk;
                # junk is masked to 0, exp's to 1, and the 7 spurious ones are
                # subtracted from the softmax denominator.
                logits = ap3.tile([128, 2, RR, 4, HEAD], dt.float32, tag="logits")
                for g in range(2):
                    e = ap3.tile([128, RR, 4, DIM], dt.bfloat16, tag="emul",
                                 bufs=1)
                    nc.vector.tensor_mul(
                        e[:], kvv[:, g, :, :, 0:DIM],
                        qb[:].unsqueeze(1).unsqueeze(2)
                        .broadcast_to((128, RR, 4, DIM)))
                    ev = e[:].rearrange("p x s (h d) -> p (x s) h d", h=HEAD)
                    l1 = ap3.tile([128, 28, HEAD, 12], dt.bfloat16, tag="l1",
                                  bufs=1)
                    nc.vector.tensor_add(l1[:], ev[:, :, :, 0:12], ev[:, :, :, 12:24])
                    l2 = ap3.tile([128, 28, HEAD, 6], dt.bfloat16, tag="l2",
                                  bufs=1)
                    nc.vector.tensor_add(l2[:], l1[:, :, :, 0:6], l1[:, :, :, 6:12])
                    l3 = ap3.tile([128, 28, HEAD, 3], dt.bfloat16, tag="l3",
                                  bufs=1)
                    nc.vector.tensor_add(l3[:], l2[:, :, :, 0:3], l2[:, :, :, 3:6])
                    l4 = ap3.tile([128, 28, HEAD, 1], dt.bfloat16, tag="l4",
                                  bufs=1)
                    nc.vector.tensor_add(l4[:], l3[:, :, :, 0:1], l3[:, :, :, 1:2])
                    nc.vector.tensor_add(
                        logits[:, g].rearrange("p x s h -> p (x s) h"),
                        l4[:].rearrange("p u h d -> p u (h d)"),
                        l3[:, :, :, 2:3].rearrange("p u h d -> p u (h d)"))

                lgv = logits[:].rearrange("p g x s h -> p (g x s) h")
                mask_bc = mk_sb[:, t, :].unsqueeze(2).broadcast_to((128, 56, HEAD))
                nc.vector.tensor_mul(lgv, lgv, mask_bc)
                nc.scalar.activation(lgv, lgv, AF.Exp)
                ssum = ap_.tile([128, HEAD], dt.float32, tag="ssum")
                nc.vector.reduce_sum(
                    ssum[:], logits[:].rearrange("p g x s h -> p h (g x s)"),
                    axis=AX.X)
                nc.vector.tensor_scalar_add(ssum[:], ssum[:], -7.0)
                rec = ap_.tile([128, HEAD], dt.float32, tag="rec")
                nc.vector.reciprocal(rec[:], ssum[:])
                nc.vector.tensor_mul(
                    lgv, lgv, rec[:].unsqueeze(1).broadcast_to((128, 56, HEAD)))
                attnb = ap3.tile([128, 2, RR, 4, HEAD], dt.bfloat16, tag="attnb")
                nc.vector.tensor_mul(
                    attnb[:].rearrange("p g x s h -> p (g x s) h"), lgv, mask_bc)

                # ---- weighted v -> per-dy chunks; MLP0 accumulates per chunk
                if ti == 0:
                    mlp_ps = mp0.tile([128, 512], dt.float32, tag="mlp0ps",
                                      name="mlp0ps")
                    xchunks = [xkp.tile([128, DY_BLOCKS, 256], dt.bfloat16,
                                        tag=f"xc{dy}", name=f"xc{dy}")
                               for dy in range(RR)]

                def mlp0_chunk(dy):
                    chunk = xchunks[dy]
                    for m in range(2):
                        for b_ in range(DY_BLOCKS):
                            kb = dy * DY_BLOCKS + b_
                            nc.tensor.matmul(
                                mlp_ps[:, m * 256: m * 256 + gw * 128],
                                m0w_sb[:, kb, m * 128:(m + 1) * 128],
                                chunk[:, b_, 0: gw * 128], start=(kb == 0),
                                stop=(kb == KBLK - 1), skip_group_check=True)

                xcol = ti * 128
                for dy in range(RR):
                    wv = wvp.tile([128, DYW_P], dt.bfloat16, tag="wv")
                    nc.gpsimd.memset(wv[:, DYW:], 0.0)
                    weng = nc.gpsimd if dy >= 3 else nc.vector
                    # v channels are stored d-major (c' = d*8 + h) so that all
                    # three operands are innermost-contiguous -> DVE 2x mode
                    g_, s_ = (0, dy) if dy < 4 else (1, dy - 4)
                    weng.tensor_mul(
                        wv[:, 0:DYW].rearrange("p (r d h) -> p r d h", d=HD, h=HEAD),
                        vslice(dy).rearrange("p r (d h) -> p r d h", h=HEAD),
                        attnb[:, g_, :, s_, :].unsqueeze(2).broadcast_to(
                            (128, RR, HD, HEAD)))
                    for grp2, nb in ((0, 8), (1, 3)):
                        tps = tp_.tile([128, nb * 128], dt.bfloat16,
                                       tag=f"tps{grp2}")
                        for bi_ in range(nb):
                            blk = grp2 * 8 + bi_
                            nc.tensor.transpose(tps[:, bi_ * 128:(bi_ + 1) * 128],
                                                wv[:, blk * 128:(blk + 1) * 128],
                                                id_sb[:])
                        nc.scalar.copy(
                            xchunks[dy][:, grp2 * 8: grp2 * 8 + nb, xcol: xcol + 128],
                            tps[:].rearrange("p (a b) -> p a b", a=nb))
                    # emit chunk dy-1's MLP0 matmuls one dy late so the PE is
                    # not stalled on the PSUM->SBUF copy of the current chunk
                    if ti == gw - 1 and dy >= 1:
                        mlp0_chunk(dy - 1)

                # ---------------- MLP layers 0-3 + head (per tile group) -----
                if ti == gw - 1:
                    mlp0_chunk(RR - 1)
                    h0 = hp.tile([128, 2, 256], dt.bfloat16, tag="h")
                    for m in range(2):
                        nc.scalar.activation(h0[:, m, 0: gw * 128],
                                             mlp_ps[:, m * 256: m * 256 + gw * 128],
                                             AF.Relu, bias=bm_sb[:, m:m + 1])
                    cur = h0
                    for l in (1, 2, 3):
                        nxt = hp.tile([128, 2, 256], dt.bfloat16, tag="h")
                        for m in range(2):
                            ps = mp_.tile([128, 256], dt.float32, tag="mlpps")
                            for kc in range(2):
                                nc.tensor.matmul(
                                    ps[:, 0: gw * 128],
                                    m13_sb[:, (l - 1) * 2 + kc,
                                           m * 128:(m + 1) * 128],
                                    cur[:, kc, 0: gw * 128],
                                    start=(kc == 0), stop=(kc == 1))
                            nc.scalar.activation(nxt[:, m, 0: gw * 128],
                                                 ps[:, 0: gw * 128], AF.Relu,
                                                 bias=bm_sb[:, 2 * l + m: 2 * l + m + 1])
                        cur = nxt
                    for tt in range(gw):
                        psof = mp_.tile([128, 256], dt.float32, tag="mlpps")
                        pso = psof[:, 0:3]
                        for kc in range(2):
                            nc.tensor.matmul(pso,
                                             cur[:, kc, tt * 128: tt * 128 + 128],
                                             m4_sb[:, kc, :],
                                             start=(kc == 0), stop=(kc == 1))
                        o1 = ap_.tile([128, 3], dt.float32, tag="o1")
                        nc.vector.tensor_add(o1[:], pso, b4_sb[:])
                        nc.vector.tensor_add(out_sb[:, grp[tt], :], o1[:],
                                             base_tiles[tt][:])

        nc.sync.dma_start(
            out.rearrange("(t p) c -> p t c", p=128), out_sb[:])


# ============================ host preparation ==============================

def _host_prep(inputs):
    inp = np.asarray(inputs["inp"], f32)
    sc = np.asarray(inputs["sample_coord"], f32)
    cell = np.asarray(inputs["cell"], f32)

    enc_w = np.asarray(inputs["enc_w"], f32)
    ch_w = np.asarray(inputs["ch_w"], f32)

    w_enc = enc_w.transpose(1, 2, 3, 0).reshape(27, ENC).astype(bf16)
    w_chp = np.zeros((128, 3, DIM), bf16)
    w_ch2 = np.zeros((ENC, 3, DIM), bf16)
    for ky in range(3):
        w_chp[0:64, ky, :] = ch_w[:, :, ky, 0].T.astype(bf16)
        w_chp[64:128, ky, :] = ch_w[:, :, ky, 1].T.astype(bf16)
        w_ch2[:, ky, :] = ch_w[:, :, ky, 2].T.astype(bf16)

    # qkv weights M-packed: 576 output cols = [q 192 | k 192 | v 192]
    w_qkv0 = np.zeros((128, 9, 576), bf16)
    w_qkv1p = np.zeros((128, 3, 576), bf16)
    w_qkv1k2 = np.zeros((64, 3, 576), bf16)
    qkvb = np.zeros((128, 5), f32)
    # v output channels permuted d-major: device channel c' = d*8+h holds
    # reference channel h*24+d (lets the wv multiply run innermost-contiguous)
    cmap_v = (np.arange(192) % 8) * 24 + np.arange(192) // 8
    for ci, nm in enumerate(("q", "k", "v")):
        wt = np.asarray(inputs[f"{nm}_w"], f32)
        bt = np.asarray(inputs[f"{nm}_b"], f32)
        if nm == "v":
            wt = wt[cmap_v]
            bt = bt[cmap_v]
        cs_ = ci * 192
        for off in range(9):
            ky, kx = off // 3, off % 3
            w_qkv0[:, off, cs_: cs_ + 192] = wt[:, 0:128, ky, kx].T.astype(bf16)
        for ky in range(3):
            w_qkv1p[0:64, ky, cs_: cs_ + 192] = wt[:, 128:192, ky, 0].T.astype(bf16)
            w_qkv1p[64:128, ky, cs_: cs_ + 192] = wt[:, 128:192, ky, 1].T.astype(bf16)
            w_qkv1k2[:, ky, cs_: cs_ + 192] = wt[:, 128:192, ky, 2].T.astype(bf16)
        bfull = bt
        for g in range(5):
            msz = 128 if g < 4 else 64
            seg = np.arange(g * 128, g * 128 + msz)
            sel = (seg >= cs_) & (seg < cs_ + 192)
            if sel.any():
                qkvb[np.nonzero(sel)[0], g] = bfull[seg[sel] - cs_]

    # m0w rows permuted into 7 chunks of 1344 -> 1408 (zero padded); within a
    # chunk the device feature order is (dx, d, h): j = dx*192 + d*8 + h maps
    # to reference row (dy*7+dx)*192 + h*24 + d
    m0w_full = np.asarray(inputs["m0w"], f32)
    jj = np.arange(DYW)
    jdx, jc = jj // 192, jj % 192
    jd, jh = jc // 8, jc % 8
    ref_j = jdx * 192 + jh * 24 + jd
    perm = np.zeros((KBLK * 128, HID), f32)
    for i in range(RR):
        perm[i * DYW_P: i * DYW_P + DYW] = m0w_full[i * DYW + ref_j]
    m0w_dev = np.ascontiguousarray(
        perm.reshape(KBLK, 128, HID).transpose(1, 0, 2)).astype(bf16)

    m13w = np.zeros((128, 6, HID), bf16)
    for l in (1, 2, 3):
        wl = np.asarray(inputs[f"m{l}w"], f32)
        m13w[:, (l - 1) * 2 + 0, :] = wl[0:128].astype(bf16)
        m13w[:, (l - 1) * 2 + 1, :] = wl[128:256].astype(bf16)
    m4w_full = np.asarray(inputs["m4w"], f32)
    m4w = np.stack([m4w_full[0:128], m4w_full[128:256]], 1).astype(bf16)

    b4 = np.broadcast_to(np.asarray(inputs["m4b"], f32)[None, :], (128, 3)).copy()
    enc_bd = np.asarray(inputs["enc_b"], f32).reshape(ENC, 1)
    ch_bd = np.zeros((128, 2), f32)
    ch_bd[:, 0] = np.asarray(inputs["ch_b"], f32)[0:128]
    ch_bd[0:64, 1] = np.asarray(inputs["ch_b"], f32)[128:192]
    ident = np.eye(128, dtype=bf16)

    m0b = np.asarray(inputs["m0b"], f32)
    m0w_tail = m0w_full[RA * DIM: RA * DIM + 2]
    bias_rest = np.zeros((128, 8), f32)
    for l in (1, 2, 3):
        bl = np.asarray(inputs[f"m{l}b"], f32)
        bias_rest[:, 2 * l + 0] = bl[0:128]
        bias_rest[:, 2 * l + 1] = bl[128:256]

    # ---- ownership: core (bi, qc) owns queries with iy//16 == qc ----
    sqh = f32(1.0 / np.sqrt(HD))
    cy_all, cx_all = sc[..., 0], sc[..., 1]
    py_all = (cy_all + f32(1.0)) * f32(H * 0.5) - f32(0.5)
    px_all = (cx_all + f32(1.0)) * f32(W * 0.5) - f32(0.5)
    iy_all = np.clip(np.floor(py_all + f32(0.5)), 0, H - 1).astype(np.int64)
    ix_all = np.clip(np.floor(px_all + f32(0.5)), 0, W - 1).astype(np.int64)

    owners = []
    for core in range(N_CORES):
        bi, qc = core // 4, core % 4
        own = np.nonzero(iy_all[bi] // 16 == qc)[0]
        owners.append(own)
    NT = max((len(o) + 127) // 128 for o in owners)
    SLOTS = NT * 128

    batch_data = []
    for bi in range(B):
        x = inp[bi]
        xT = x.reshape(3, H * W).T
        hi = xT.astype(bf16).astype(f32)
        lo = (xT - hi).astype(bf16)
        hilo_full = np.concatenate([hi.astype(bf16), lo], 1)  # [4096, 6]

        rel_cell = cell[bi] * np.array([H, W], f32)
        b0 = m0b + rel_cell @ m0w_tail
        bm = bias_rest.copy()
        bm[:, 0] = b0[0:128]
        bm[:, 1] = b0[128:256]
        batch_data.append((x, hilo_full, bm))

    d = np.arange(-R, R + 1)
    percore = []
    for core in range(N_CORES):
        bi, qc = core // 4, core % 4
        R0 = 16 * qc
        own = owners[core]
        n = len(own)
        x, hilo_full, bm = batch_data[bi]

        # padded image: padded row p <-> image row R0 + p - 35
        xbig = np.zeros((CH_IN, 128, W + 2), f32)
        plo, phi = max(0, 35 - R0), min(128, 35 - R0 + H)
        xbig[:, plo:phi, 1:1 + W] = x[:, plo - 35 + R0: phi - 35 + R0, :]

        # im2col for enc strips 2..7 (rows 16..63)
        col = np.zeros((27, 6 * 512), bf16)
        for c in range(CH_IN):
            for ky in range(3):
                for kx in range(3):
                    col[c * 9 + ky * 3 + kx] = \
                        xbig[c, 15 + ky: 63 + ky, kx: kx + W] \
                        .reshape(-1).astype(bf16)
        tr = np.arange(16, 64) - 35 + R0  # image rows of enc output strips
        col.reshape(27, 48, W)[:, (tr < 0) | (tr >= H), :] = 0

        # ch rows 31..56 mask (image rows R0-4 .. R0+21)
        trf = np.arange(26) + R0 - 4
        maskfp = np.broadcast_to(
            ((trf >= 0) & (trf < H)).astype(bf16)[None, :], (128, 26)).copy()

        # hilo for local pixel rows 0..23 (image rows R0-3 .. R0+20)
        hl = np.zeros((NPX, 6), bf16)
        rlo, rhi = max(0, R0 - 3), min(H, R0 + 21)
        hl[(rlo - R0 + 3) * W: (rhi - R0 + 3) * W] = \
            hilo_full[rlo * W: rhi * W]
        hilo = np.ascontiguousarray(hl.reshape(12, 128, 6).transpose(1, 0, 2))

        # ---- per-query gather indices / weights ----
        iy, ix = iy_all[bi][own], ix_all[bi][own]
        py, px = py_all[bi][own], px_all[bi][own]

        dy_, dx_ = [a.reshape(-1) for a in np.meshgrid(d, d, indexing="ij")]
        yy = iy[:, None] + dy_[None, :]
        xx = ix[:, None] + dx_[None, :]
        valid = ((yy >= 0) & (yy < H) & (xx >= 0) & (xx < W)).astype(f32)

        # kv entry index for group g: entry (iy - R0 + 4g)*64 + ix - 3,
        # tensor row = HEADKV + entry = 192 + (iy-R0+4g)*64 + ix
        kvstart = np.zeros((n, 2), np.int64)
        for g in range(2):
            kvstart[:, g] = 192 + (iy - R0 + 4 * g) * 64 + ix

        y0 = np.floor(py)
        x0 = np.floor(px)
        wy, wx = py - y0, px - x0
        y0 = y0.astype(np.int64)
        x0 = x0.astype(np.int64)
        sy0 = np.clip(y0, 0, H - 2)
        sx0 = np.clip(x0, 0, W - 2)
        wq_eff = np.zeros((n, 2, 2), f32)
        wb_eff = np.zeros((n, 2, 2), f32)
        qq = np.arange(n)
        for ddy, syw in ((0, 1 - wy), (1, wy)):
            for ddx, sxw in ((0, 1 - wx), (1, wx)):
                w = (syw * sxw).astype(f32)
                yc, xc = y0 + ddy, x0 + ddx
                ly = np.clip(yc, 0, H - 1) - sy0
                lx = np.clip(xc, 0, W - 1) - sx0
                wb_eff[qq, ly, lx] += w
                vm = ((yc >= 0) & (yc < H) & (xc >= 0) & (xc < W))
                wq_eff[qq, ly, lx] += w * vm
        # qg4 layout is [x(sx), y(sy)] -> reorder weights to (lx, ly)
        wq4 = wq_eff.transpose(0, 2, 1).reshape(n, 4)
        wb4 = wb_eff.transpose(0, 2, 1).reshape(n, 4)
        qstart = HEADQ + (sy0 - R0 + 3) * 64 + sx0

        # pad to SLOTS
        def padto(a, fill):
            outp = np.full((SLOTS,) + a.shape[1:], fill, a.dtype)
            outp[:n] = a
            return outp

        kvstart_p = padto(kvstart, 192)
        qstart_p = padto(qstart, HEADQ + 128)
        valid_p = padto(valid, 0.0)
        wq4_p = padto(wq4.astype(f32), 0.0)
        wb4_p = padto(wb4, 0.0)

        kvidx = np.zeros((128, NT, 16), np.int16)
        qidx = np.zeros((128, NT, 8), np.int16)
        masktt = np.zeros((128, NT, 56), f32)
        qwt = np.zeros((128, NT, 4), bf16)
        qwbt = np.zeros((128, NT, 4), f32)
        # mask in device (g, x, s) order: u = g*28 + x*4 + s, dy = g*4+s, dx = x
        uu = np.arange(56)
        gu, xu, su = uu // 28, (uu % 28) // 4, uu % 4
        dyu = gu * 4 + su
        usel = dyu < 7
        for t in range(NT):
            ts = slice(t * 128, (t + 1) * 128)
            masktt[:, t, uu[usel]] = valid_p[ts][:, dyu[usel] * 7 + xu[usel]]
            qwt[:, t, :] = (wq4_p[ts] * sqh).astype(bf16)
            qwbt[:, t, :] = wb4_p[ts]
            flat = kvstart_p[ts].T.reshape(-1)  # j = g*128 + q
            kvidx[:, t, :] = np.tile(flat.reshape(-1, 16).T, (8, 1)).astype(np.int16)
            fq = qstart_p[ts]
            qidx[:, t, :] = np.tile(fq.reshape(-1, 16).T, (8, 1)).astype(np.int16)

        percore.append({
            "inp_col": col, "inp_hilo": hilo,
            "w_enc": w_enc, "w_chp": w_chp, "w_ch2": w_ch2,
            "w_qkv0": w_qkv0, "w_qkv1p": w_qkv1p, "w_qkv1k2": w_qkv1k2,
            "qkvb": qkvb, "maskfp": maskfp, "enc_b": enc_bd, "ch_b": ch_bd,
            "m0w": m0w_dev, "m13w": m13w, "m4w": m4w, "bmlp": bm, "b4": b4,
            "ident": ident, "kvidx": kvidx, "qidx": qidx, "maskt": masktt,
            "qwt": qwt, "qwbt": qwbt,
        })
    return percore, NT, owners


# ============================== entry point =================================

def _get_program(NT):
    if NT not in _PROGRAMS:
        _PROGRAMS[NT] = build_program(NT)
    return _PROGRAMS[NT]


def kernel(**inputs):
    from concourse import bass_utils
    in_maps, NT, owners = _host_prep(inputs)
    nc = _get_program(NT)
    res = bass_utils.run_bass_kernel_spmd(nc, in_maps, core_ids=list(range(N_CORES)))
    full = np.empty((B, Q, 3), f32)
    for core in range(N_CORES):
        bi = core // 4
        own = owners[core]
        full[bi, own] = res.results[core]["out"][:len(own)]
    return full


if __name__ == "__main__":
    import time
    t0 = time.time()
    nc = _get_program(9)
    print("built+compiled in", time.time() - t0, "s")


# revision 23
# speedup vs baseline: 1.0821x; 1.0723x over previous
"""Trainium2 Bass kernel for the CLIT-style sparse local attention module.

Strategy (8 NeuronCores, SPMD, no collectives):
  - core c = (batch bi = c // 4, strip qc = c % 4) OWNS the queries whose
    nearest-pixel center row falls in image rows [16qc, 16qc+16). Each core
    computes the 5 convs locally for the 24-row band [16qc-3, 16qc+21)
    (own 16 rows + 3-row halo each side + 2 spare), so every window/bilinear
    read its queries need is produced locally -- the AllGather disappears.
  - qkv conv outputs are PE-transposed to pixel-major entries in local DRAM
    with a 4x row-shifted duplication: entry (r, x) holds rows r..r+3 of
    column x back to back. A 7x7 window then needs only TWO dma_gather
    descriptors per query (4+3 window rows each), and the bilinear q read
    needs ONE (2x2 patch via 2x row duplication).
  - v is stored as fp8e4 (k stays bf16): halves the v gather traffic; the
    wv multiply runs at DVE 1x anyway (broadcast attn operand), so fp8
    costs nothing there. rel-err budget is ~2e-2; bf16 baseline is ~8e-5.
  - Attention per 128-query tile: QK logits as DVE mul + binary-tree adds
    (tensor_reduce is 1x-mode and slow), softmax, attention-weighted v
    (split DVE/GpSimd), PE-transposed into the K-major layout for the
    9410x256 MLP0 (bf16, fp32 PSUM), then MLP1-3 + head per tile group.
  - Host precomputes ownership, gather indices, bilinear weights, masks;
    kernel() scatters per-core outputs back to the original query order.
"""

import sys

sys.path.insert(0, "/opt/trn_rl_repo")

import numpy as np
import ml_dtypes

# ---------------- problem constants (hardcoded per contract) ----------------
B, CH_IN, H, W = 2, 3, 64, 64
Q = 4096
DIM, HEAD, R = 192, 8, 3
RR = 2 * R + 1
RA = RR * RR          # 49
HD = DIM // HEAD      # 24
ENC = 64
HID = 256
N_CORES = 8

NROWS = 24            # local conv band rows (strip 16 + 3 halo + 2 spare)
NPX = NROWS * W       # 1536 pixel entries
SUB_E = 288           # bf16 slots per kv sub-entry: 192 k bf16 + 192 v fp8
KV_ENT = 4 * SUB_E    # 1152: entry holds rows r..r+3
HEADKV = 195          # 192 write-shift room + 3 px guard
KVROWS = HEADKV + NPX # 1731
Q_ENT = 2 * 256       # q entry holds rows r..r+1
HEADQ = 64
QROWS = HEADQ + NPX   # 1600

DYW = RR * DIM                    # 1344 columns per window-row chunk
DYW_P = 1408                      # padded to 11 x 128
DY_BLOCKS = DYW_P // 128          # 11
KBLK = RR * DY_BLOCKS             # 77 K-blocks for MLP layer 0

f32 = np.float32
bf16 = ml_dtypes.bfloat16

_PROGRAMS = {}  # cached compiled Bass programs keyed by NT


def _groups(nt):
    gs = []
    t = 0
    while t < nt:
        gs.append(tuple(range(t, min(t + 2, nt))))
        t += 2
    return gs


# ============================ device program ================================

def build_program(NT):
    import concourse.bass as bass
    import concourse.tile as tile
    from concourse import bacc, mybir

    dt = mybir.dt

    nc = bacc.Bacc("TRN2", target_bir_lowering=False, debug=False,
                   enable_asserts=False, num_devices=N_CORES)

    def din(name, shape, dtype):
        return nc.dram_tensor(name, list(shape), dtype, kind="ExternalInput").ap()

    inp_col = din("inp_col", [27, 6 * 512], dt.bfloat16)
    inp_hilo = din("inp_hilo", [128, 12, 6], dt.bfloat16)
    w_enc = din("w_enc", [27, ENC], dt.bfloat16)
    w_chp = din("w_chp", [128, 3, DIM], dt.bfloat16)
    w_ch2 = din("w_ch2", [ENC, 3, DIM], dt.bfloat16)
    w_qkv0 = din("w_qkv0", [128, 9, 576], dt.bfloat16)
    w_qkv1p = din("w_qkv1p", [128, 3, 576], dt.bfloat16)
    w_qkv1k2 = din("w_qkv1k2", [64, 3, 576], dt.bfloat16)
    qkvb = din("qkvb", [128, 5], dt.float32)
    maskfp = din("maskfp", [128, 26], dt.bfloat16)
    enc_b = din("enc_b", [ENC, 1], dt.float32)
    ch_b = din("ch_b", [128, 2], dt.float32)
    m0w = din("m0w", [128, KBLK, HID], dt.bfloat16)
    m13w = din("m13w", [128, 6, HID], dt.bfloat16)
    m4w = din("m4w", [128, 2, 3], dt.bfloat16)
    bmlp = din("bmlp", [128, 8], dt.float32)
    b4 = din("b4", [128, 3], dt.float32)
    ident = din("ident", [128, 128], dt.bfloat16)
    kvidx = din("kvidx", [128, NT, 16], dt.int16)
    qidx = din("qidx", [128, NT, 8], dt.int16)
    maskt = din("maskt", [128, NT, 56], dt.float32)
    qwt = din("qwt", [128, NT, 4], dt.bfloat16)
    qwbt = din("qwbt", [128, NT, 4], dt.float32)
    out = nc.dram_tensor("out", [NT * 128, 3], dt.float32,
                         kind="ExternalOutput").ap()

    with tile.TileContext(nc) as tc:
        with tc.tile_pool(name="dram", bufs=1, space="DRAM") as dp:
            kv4 = dp.tile([KVROWS, KV_ENT], dt.bfloat16)
            q2 = dp.tile([QROWS, Q_ENT], dt.bfloat16)

            _convs(nc, tc, mybir, locals())
            _attention(nc, tc, mybir, NT, locals())

    nc.compile()
    return nc


def _convs(nc, tc, mybir, env):
    dt = mybir.dt
    AF = mybir.ActivationFunctionType

    inp_col, w_enc = env["inp_col"], env["w_enc"]
    w_chp, w_ch2 = env["w_chp"], env["w_ch2"]
    w_qkv0, w_qkv1p, w_qkv1k2 = env["w_qkv0"], env["w_qkv1p"], env["w_qkv1k2"]
    qkvb = env["qkvb"]
    enc_b, ch_b, inp_hilo = env["enc_b"], env["ch_b"], env["inp_hilo"]
    kv4, q2 = env["kv4"], env["q2"]

    with (
        tc.tile_pool(name="cw", bufs=1) as cw,
        tc.tile_pool(name="cfeat", bufs=1) as cf,
        tc.tile_pool(name="cpsum", bufs=2, space="PSUM") as cp,
        tc.tile_pool(name="qpsum", bufs=2, space="PSUM") as cpq,
        tc.tile_pool(name="ctpsum", bufs=2, space="PSUM") as cpt,
        tc.tile_pool(name="cstage", bufs=3) as cs,
    ):
        col_sb = cw.tile([27, 6 * 512], dt.bfloat16)
        nc.sync.dma_start(col_sb[:], inp_col)
        wenc_sb = cw.tile([27, ENC], dt.bfloat16)
        nc.sync.dma_start(wenc_sb[:], w_enc)
        wchp_sb = cw.tile([128, 3, DIM], dt.bfloat16)
        nc.sync.dma_start(wchp_sb[:], w_chp)
        wch2_sb = cw.tile([ENC, 3, DIM], dt.bfloat16)
        nc.sync.dma_start(wch2_sb[:], w_ch2)
        wq0 = cw.tile([128, 9, 576], dt.bfloat16)
        nc.sync.dma_start(wq0[:], w_qkv0)
        wq1p = cw.tile([128, 3, 576], dt.bfloat16)
        nc.sync.dma_start(wq1p[:], w_qkv1p)
        wq1k2 = cw.tile([64, 3, 576], dt.bfloat16)
        nc.sync.dma_start(wq1k2[:], w_qkv1k2)
        qkvb_sb = cw.tile([128, 5], dt.float32)
        nc.sync.dma_start(qkvb_sb[:], qkvb)
        encb_sb = cw.tile([ENC, 1], dt.float32)
        nc.sync.dma_start(encb_sb[:], enc_b)
        chb_sb = cw.tile([128, 2], dt.float32)
        nc.sync.dma_start(chb_sb[:], ch_b)
        hilo_sb = cw.tile([128, 12, 6], dt.bfloat16)
        nc.sync.dma_start(hilo_sb[:], inp_hilo)
        id_c = cw.tile([128, 128], dt.bfloat16)
        nc.sync.dma_start(id_c[:], env["ident"])
        maskfp_sb = cw.tile([128, 26], dt.bfloat16)
        nc.sync.dma_start(maskfp_sb[:], env["maskfp"])

        # zero the kv head region (write-shift room + guard entries): rows
        # 0..HEADKV of kv4; shifted writes partially overwrite it afterwards.
        zt = cw.tile([128, 1755], dt.bfloat16)
        nc.vector.memset(zt[:], 0.0)
        kvf = kv4[:, :].flatten()
        nc.sync.dma_start(kvf[0: HEADKV * KV_ENT]
                          .rearrange("(p a) -> p a", p=128), zt[:])

        encp = cf.tile([ENC, 66, 66], dt.bfloat16)
        nc.vector.memset(encp[:], 0.0)
        fp0 = cf.tile([128, 66, 66], dt.bfloat16)
        nc.vector.memset(fp0[:], 0.0)
        fp1 = cf.tile([64, 66, 66], dt.bfloat16)
        nc.vector.memset(fp1[:], 0.0)
        encb2 = cf.tile([128, 66, 66], dt.bfloat16)
        fp1b = cf.tile([128, 66, 66], dt.bfloat16)

        # ---- enc conv (strips 2-7) ----
        for t in range(2, 8):
            ps = cp.tile([128, 512], dt.float32, tag="cps")
            nc.tensor.matmul(ps[:ENC, :], wenc_sb[:],
                             col_sb[:, (t - 2) * 512:(t - 1) * 512],
                             start=True, stop=True)
            dst = encp[:, 1 + t * 8: 1 + t * 8 + 8, 1:65]
            nc.scalar.activation(dst, ps[:ENC, :].rearrange("p (a b) -> p a b", a=8),
                                 AF.Identity, bias=encb_sb[:, 0:1])

        # encb2: enc features with a one-column-shifted copy in partitions 64:
        nc.vector.tensor_copy(encb2[0:64, 24:66, :], encp[:, 24:66, :])
        nc.vector.tensor_copy(encb2[64:128, 24:66, 0:65], encp[:, 24:66, 1:66])

        # ---- ch conv (strips 3-7; kx 0/1 paired into K=128, kx=2 single) ----
        for t in range(3, 8):
            for m, msz in ((0, 128), (1, 64)):
                ps = cp.tile([128, 512], dt.float32, tag="cps")
                for ky in range(3):
                    rhs = encb2[:, t * 8 + ky: t * 8 + ky + 8, 0:64]
                    nc.tensor.matmul(ps[:msz, :],
                                     wchp_sb[:, ky, m * 128: m * 128 + msz],
                                     rhs, start=(ky == 0), stop=False)
                for ky in range(3):
                    rhs = encp[:, t * 8 + ky: t * 8 + ky + 8, 2:66]
                    nc.tensor.matmul(ps[:msz, :],
                                     wch2_sb[:, ky, m * 128: m * 128 + msz],
                                     rhs, start=False, stop=(ky == 2))
                dstp = (fp0 if m == 0 else fp1)
                dst = dstp[:msz, 1 + t * 8: 1 + t * 8 + 8, 1:65]
                nc.scalar.activation(dst,
                                     ps[:msz, :].rearrange("p (a b) -> p a b", a=8),
                                     AF.Identity, bias=chb_sb[:msz, m: m + 1])

        # zero ch features of out-of-image rows (qkv convs read fp rows 32..57)
        mbc = maskfp_sb[:].unsqueeze(2).broadcast_to((128, 26, 66))
        nc.vector.tensor_mul(fp0[:, 32:58, :], fp0[:, 32:58, :], mbc)
        nc.vector.tensor_mul(fp1[:, 32:58, :], fp1[:, 32:58, :],
                             maskfp_sb[0:64, :].unsqueeze(2)
                             .broadcast_to((64, 26, 66)))

        # fp1b: channel-chunk-1 features with one-column-shifted copy
        nc.vector.tensor_copy(fp1b[0:64, 32:58, :], fp1[:, 32:58, :])
        nc.vector.tensor_copy(fp1b[64:128, 32:58, 0:65], fp1[:, 32:58, 1:66])

        # ---- q/k/v convs (strips 4-6; M-packed 576 = 4x128+64) + transpose ----
        # 576 cols = [q 0:192 | k 192:384 | v 384:576]
        # per group: list of (dst_kind, src_off, dst_off, n)
        #   kind 0 = qstage bf16, 1 = kvstage k bf16, 2 = kvstage v fp8 view
        gdst = [
            [(0, 0, 0, 128)],
            [(0, 0, 128, 64), (1, 64, 0, 64)],
            [(1, 0, 64, 128)],
            [(2, 0, 0, 128)],
            [(2, 0, 128, 64)],
        ]
        for t in (4, 5, 6):
            kvstage = cs.tile([128, 4, SUB_E], dt.bfloat16, tag="kvstage")
            qstage = cs.tile([128, 4, 256], dt.bfloat16, tag="qstage")
            vview = kvstage[:, :, 192:SUB_E].bitcast(dt.float8e4)
            nc.vector.memset(qstage[:, :, 198:], 0.0)
            nc.vector.tensor_copy(qstage[:, :, 192:198],
                                  hilo_sb[:, (t - 4) * 4:(t - 3) * 4, :])
            for g in range(5):
                msz = 128 if g < 4 else 64
                ps = cpq.tile([128, 512], dt.float32, tag="qkvps")
                for off in range(9):
                    ky, kx = off // 3, off % 3
                    rhs0 = fp0[:, t * 8 + ky: t * 8 + ky + 8, kx: kx + 64]
                    nc.tensor.matmul(ps[:msz, :],
                                     wq0[:, off, g * 128: g * 128 + msz],
                                     rhs0, start=(off == 0), stop=False)
                for ky in range(3):
                    rhs1 = fp1b[:, t * 8 + ky: t * 8 + ky + 8, 0:64]
                    nc.tensor.matmul(ps[:msz, :],
                                     wq1p[:, ky, g * 128: g * 128 + msz],
                                     rhs1, start=False, stop=False)
                for ky in range(3):
                    rhs1 = fp1[:, t * 8 + ky: t * 8 + ky + 8, 2:66]
                    nc.tensor.matmul(ps[:msz, :],
                                     wq1k2[:, ky, g * 128: g * 128 + msz],
                                     rhs1, start=False, stop=(ky == 2))
                csb = cs.tile([128, 512], dt.bfloat16, tag="convsb")
                nc.scalar.activation(csb[:msz, :], ps[:msz, :], AF.Identity,
                                     bias=qkvb_sb[:msz, g: g + 1])
                tps = cpt.tile([128, 512], dt.bfloat16, tag="ctps")
                for blk in range(4):
                    nc.tensor.transpose(
                        tps[:, blk * 128: blk * 128 + msz],
                        csb[:msz, blk * 128:(blk + 1) * 128],
                        id_c[:msz, :msz])
                tview = tps[:].rearrange("p (a b) -> p a b", a=4)
                for kind, so, do, n in gdst[g]:
                    if kind == 0:
                        nc.scalar.copy(qstage[:, :, do: do + n],
                                       tview[:, :, so: so + n])
                    elif kind == 1:
                        nc.scalar.copy(kvstage[:, :, do: do + n],
                                       tview[:, :, so: so + n])
                    else:
                        nc.scalar.copy(vview[:, :, do: do + n],
                                       tview[:, :, so: so + n])
            # 4x / 2x row-shifted duplicated writes: entry (r, x) sub s holds
            # local row r+s. Strip t covers local rows LT..LT+7.
            LT = (t - 4) * 8
            for s in range(4):
                r0 = HEADKV + (LT - s) * 64
                nc.sync.dma_start(
                    kv4[r0: r0 + 512, s * SUB_E: (s + 1) * SUB_E]
                    .rearrange("(b p) e -> p b e", p=128), kvstage[:])
            for s in range(2):
                r0 = HEADQ + (LT - s) * 64
                nc.sync.dma_start(
                    q2[r0: r0 + 512, s * 256: (s + 1) * 256]
                    .rearrange("(b p) e -> p b e", p=128), qstage[:])


def _attention(nc, tc, mybir, NT, env):
    import concourse.bass as bass
    dt = mybir.dt
    AX = mybir.AxisListType
    AF = mybir.ActivationFunctionType

    kv4, q2 = env["kv4"], env["q2"]
    m0w, m13w, m4w = env["m0w"], env["m13w"], env["m4w"]
    bmlp, b4, ident = env["bmlp"], env["b4"], env["ident"]
    kvidx, qidx = env["kvidx"], env["qidx"]
    maskt, qwt, qwbt = env["maskt"], env["qwt"], env["qwbt"]
    out = env["out"]

    with (
        tc.tile_pool(name="aw", bufs=1) as aw,
        tc.tile_pool(name="gath", bufs=2) as gp,
        tc.tile_pool(name="attn", bufs=3) as ap_,
        tc.tile_pool(name="attn3", bufs=3) as ap3,
        tc.tile_pool(name="wvp", bufs=6) as wvp,
        tc.tile_pool(name="xkp", bufs=1) as xkp,
        tc.tile_pool(name="hp", bufs=2) as hp,
        tc.tile_pool(name="outp", bufs=1) as op_,
        tc.tile_pool(name="tpsum", bufs=2, space="PSUM") as tp_,
        tc.tile_pool(name="mpsum", bufs=2, space="PSUM") as mp_,
        tc.tile_pool(name="m0psum", bufs=2, space="PSUM") as mp0,
    ):
        m0w_sb = aw.tile([128, KBLK, HID], dt.bfloat16)
        nc.sync.dma_start(m0w_sb[:], m0w)
        m13_sb = aw.tile([128, 6, HID], dt.bfloat16)
        nc.sync.dma_start(m13_sb[:], m13w)
        m4_sb = aw.tile([128, 2, 3], dt.bfloat16)
        nc.sync.dma_start(m4_sb[:], m4w)
        bm_sb = aw.tile([128, 8], dt.float32)
        nc.sync.dma_start(bm_sb[:], bmlp)
        b4_sb = aw.tile([128, 3], dt.float32)
        nc.sync.dma_start(b4_sb[:], b4)
        id_sb = aw.tile([128, 128], dt.bfloat16)
        nc.sync.dma_start(id_sb[:], ident)
        kvi_sb = aw.tile([128, NT, 16], dt.int16)
        nc.sync.dma_start(kvi_sb[:], kvidx)
        qi_sb = aw.tile([128, NT, 8], dt.int16)
        nc.sync.dma_start(qi_sb[:], qidx)
        mk_sb = aw.tile([128, NT, 56], dt.float32)
        nc.sync.dma_start(mk_sb[:], maskt)
        qw_sb = aw.tile([128, NT, 4], dt.bfloat16)
        nc.sync.dma_start(qw_sb[:], qwt)
        qwb_sb = aw.tile([128, NT, 4], dt.float32)
        nc.sync.dma_start(qwb_sb[:], qwbt)

        out_sb = op_.tile([128, NT, 3], dt.float32)

        kv_ap = kv4[:, :]
        kv_ap = bass.AP(kv_ap.tensor, kv_ap.offset,
                        [[KV_ENT, KVROWS - 6], [1, RR * KV_ENT]])
        q_ap = q2[:, :]
        q_ap = bass.AP(q_ap.tensor, q_ap.offset,
                       [[Q_ENT, QROWS - 2], [1, 2 * Q_ENT]])

        def issue_gathers(t):
            kvg = gp.tile([128, 2, RR * KV_ENT], dt.bfloat16, tag="kvg")
            nc.gpsimd.dma_gather(kvg[:], kv_ap, kvi_sb[:, t, :],
                                 num_idxs=256, num_idxs_reg=256,
                                 elem_size=RR * KV_ENT, elem_step=KV_ENT,
                                 single_packet=False)
            qg = gp.tile([128, 1, 2 * Q_ENT], dt.bfloat16, tag="qg")
            nc.gpsimd.dma_gather(qg[:], q_ap, qi_sb[:, t, :],
                                 num_idxs=128, num_idxs_reg=128,
                                 elem_size=2 * Q_ENT, elem_step=Q_ENT,
                                 single_packet=False)
            return qg, kvg

        groups = _groups(NT)
        base_tiles = [None, None]
        mlp_ps = None
        xchunks = None

        pending = issue_gathers(0)
        for gi, grp in enumerate(groups):
            gw = len(grp)  # tiles in this MLP group (2 or trailing 1)
            for ti, t in enumerate(grp):
                qg, kvg = pending
                if t + 1 < NT:
                    # desc-gen first in the gpsimd queue so the next tile's
                    # gather DMA overlaps this tile's compute fully
                    pending = issue_gathers(t + 1)
                kvv = kvg[:].rearrange("p g (x s e) -> p g x s e", x=RR, s=4)
                qg4 = qg[:].rearrange("p o (x y e) -> p o x y e", x=2, y=2)

                def kslice(dy):
                    g, s = (0, dy) if dy < 4 else (1, dy - 4)
                    return kvv[:, g, :, s, 0:DIM]

                def vslice(dy):
                    g, s = (0, dy) if dy < 4 else (1, dy - 4)
                    return kvv[:, g, :, s, DIM:SUB_E].bitcast(dt.float8e4)

                # ---------------- q vector (bilinear blend) + base ----------
                qprod = ap_.tile([128, 2, 2, DIM], dt.bfloat16, tag="qprod")
                nc.vector.tensor_mul(
                    qprod[:], qg4[:, 0, :, :, 0:DIM],
                    qw_sb[:, t, :].rearrange("p (a b) -> p a b", a=2)
                    .unsqueeze(3).broadcast_to((128, 2, 2, DIM)))
                qt1 = ap_.tile([128, 2, DIM], dt.bfloat16, tag="qt1")
                nc.vector.tensor_add(qt1[:], qprod[:, 0], qprod[:, 1])
                qb = ap_.tile([128, DIM], dt.bfloat16, tag="qb")
                nc.vector.tensor_add(qb[:], qt1[:, 0], qt1[:, 1])

                badd = ap_.tile([128, 2, 2, 3], dt.float32, tag="badd")
                nc.vector.tensor_add(badd[:], qg4[:, 0, :, :, 192:195],
                                     qg4[:, 0, :, :, 195:198])
                bprod = ap_.tile([128, 2, 2, 3], dt.float32, tag="bprod")
                nc.vector.tensor_mul(
                    bprod[:], badd[:],
                    qwb_sb[:, t, :].rearrange("p (a b) -> p a b", a=2)
                    .unsqueeze(3).broadcast_to((128, 2, 2, 3)))
                bt1 = ap_.tile([128, 2, 3], dt.float32, tag="bt1")
                nc.vector.tensor_add(bt1[:], bprod[:, 0], bprod[:, 1])
                base_t = ap_.tile([128, 3], dt.float32, tag="base")
                nc.vector.tensor_add(base_t[:], bt1[:, 0], bt1[:, 1])
                base_tiles[ti] = base_t

                # -------- QK logits: 2 bulk muls + per-group binary trees ----
                # logits layout [128, g 2, x 7, s 4, h 8]; (g1, s3) = junk;
                # junk is masked to 0, exp's to 1, and the 7 spurious ones are
                # subtracted from the softmax denominator.
                logits = ap3.tile([128, 2, RR, 4, HEAD], dt.float32, tag="logits")
                for g in range(2):
                    e = ap3.tile([128, RR, 4, DIM], dt.bfloat16, tag="emul",
                                 bufs=1)
                    nc.vector.tensor_mul(
                        e[:], kvv[:, g, :, :, 0:DIM],
                        qb[:].unsqueeze(1).unsqueeze(2)
                        .broadcast_to((128, RR, 4, DIM)))
                    ev = e[:].rearrange("p x s (h d) -> p (x s) h d", h=HEAD)
                    l1 = ap3.tile([128, 28, HEAD, 12], dt.bfloat16, tag="l1",
                                  bufs=1)
                    nc.vector.tensor_add(l1[:], ev[:, :, :, 0:12], ev[:, :, :, 12:24])
                    l2 = ap3.tile([128, 28, HEAD, 6], dt.bfloat16, tag="l2",
                                  bufs=1)
                    nc.vector.tensor_add(l2[:], l1[:, :, :, 0:6], l1[:, :, :, 6:12])
                    l3 = ap3.tile([128, 28, HEAD, 3], dt.bfloat16, tag="l3",
                                  bufs=1)
                    nc.vector.tensor_add(l3[:], l2[:, :, :, 0:3], l2[:, :, :, 3:6])
                    l4 = ap3.tile([128, 28, HEAD, 1], dt.bfloat16, tag="l4",
                                  bufs=1)
                    nc.vector.tensor_add(l4[:], l3[:, :, :, 0:1], l3[:, :, :, 1:2])
                    nc.vector.tensor_add(
                        logits[:, g].rearrange("p x s h -> p (x s) h"),
                        l4[:].rearrange("p u h d -> p u (h d)"),
                        l3[:, :, :, 2:3].rearrange("p u h d -> p u (h d)"))

                lgv = logits[:].rearrange("p g x s h -> p (g x s) h")
                mask_bc = mk_sb[:, t, :].unsqueeze(2).broadcast_to((128, 56, HEAD))
                nc.vector.tensor_mul(lgv, lgv, mask_bc)
                nc.scalar.activation(lgv, lgv, AF.Exp)
                ssum = ap_.tile([128, HEAD], dt.float32, tag="ssum")
                nc.vector.reduce_sum(
                    ssum[:], logits[:].rearrange("p g x s h -> p h (g x s)"),
                    axis=AX.X)
                nc.vector.tensor_scalar_add(ssum[:], ssum[:], -7.0)
                rec = ap_.tile([128, HEAD], dt.float32, tag="rec")
                nc.vector.reciprocal(rec[:], ssum[:])
                nc.vector.tensor_mul(
                    lgv, lgv, rec[:].unsqueeze(1).broadcast_to((128, 56, HEAD)))
                attnb = ap3.tile([128, 2, RR, 4, HEAD], dt.bfloat16, tag="attnb")
                nc.vector.tensor_mul(
                    attnb[:].rearrange("p g x s h -> p (g x s) h"), lgv, mask_bc)

                # ---- weighted v -> per-dy chunks; MLP0 accumulates per chunk
                if ti == 0:
                    mlp_ps = mp0.tile([128, 512], dt.float32, tag="mlp0ps",
                                      name="mlp0ps")
                    xchunks = [xkp.tile([128, DY_BLOCKS, 256], dt.bfloat16,
                                        tag=f"xc{dy}", name=f"xc{dy}")
                               for dy in range(RR)]

                def mlp0_chunk(dy):
                    chunk = xchunks[dy]
                    for m in range(2):
                        for b_ in range(DY_BLOCKS):
                            kb = dy * DY_BLOCKS + b_
                            nc.tensor.matmul(
                                mlp_ps[:, m * 256: m * 256 + gw * 128],
                                m0w_sb[:, kb, m * 128:(m + 1) * 128],
                                chunk[:, b_, 0: gw * 128], start=(kb == 0),
                                stop=(kb == KBLK - 1), skip_group_check=True)

                xcol = ti * 128
                for dy in range(RR):
                    wv = wvp.tile([128, DYW_P], dt.bfloat16, tag="wv")
                    nc.gpsimd.memset(wv[:, DYW:], 0.0)
                    weng = nc.gpsimd if dy >= 4 else nc.vector
                    # v channels are stored d-major (c' = d*8 + h) so that all
                    # three operands are innermost-contiguous -> DVE 2x mode
                    g_, s_ = (0, dy) if dy < 4 else (1, dy - 4)
                    weng.tensor_mul(
                        wv[:, 0:DYW].rearrange("p (r d h) -> p r d h", d=HD, h=HEAD),
                        vslice(dy).rearrange("p r (d h) -> p r d h", h=HEAD),
                        attnb[:, g_, :, s_, :].unsqueeze(2).broadcast_to(
                            (128, RR, HD, HEAD)))
                    for grp2, nb in ((0, 8), (1, 3)):
                        tps = tp_.tile([128, nb * 128], dt.bfloat16,
                                       tag=f"tps{grp2}")
                        for bi_ in range(nb):
                            blk = grp2 * 8 + bi_
                            nc.tensor.transpose(tps[:, bi_ * 128:(bi_ + 1) * 128],
                                                wv[:, blk * 128:(blk + 1) * 128],
                                                id_sb[:])
                        nc.scalar.copy(
                            xchunks[dy][:, grp2 * 8: grp2 * 8 + nb, xcol: xcol + 128],
                            tps[:].rearrange("p (a b) -> p a b", a=nb))
                    # emit chunk dy-1's MLP0 matmuls one dy late so the PE is
                    # not stalled on the PSUM->SBUF copy of the current chunk
                    if ti == gw - 1 and dy >= 1:
                        mlp0_chunk(dy - 1)

                # ---------------- MLP layers 0-3 + head (per tile group) -----
                if ti == gw - 1:
                    mlp0_chunk(RR - 1)
                    h0 = hp.tile([128, 2, 256], dt.bfloat16, tag="h")
                    for m in range(2):
                        nc.scalar.activation(h0[:, m, 0: gw * 128],
                                             mlp_ps[:, m * 256: m * 256 + gw * 128],
                                             AF.Relu, bias=bm_sb[:, m:m + 1])
                    cur = h0
                    for l in (1, 2, 3):
                        nxt = hp.tile([128, 2, 256], dt.bfloat16, tag="h")
                        for m in range(2):
                            ps = mp_.tile([128, 256], dt.float32, tag="mlpps")
                            for kc in range(2):
                                nc.tensor.matmul(
                                    ps[:, 0: gw * 128],
                                    m13_sb[:, (l - 1) * 2 + kc,
                                           m * 128:(m + 1) * 128],
                                    cur[:, kc, 0: gw * 128],
                                    start=(kc == 0), stop=(kc == 1))
                            nc.scalar.activation(nxt[:, m, 0: gw * 128],
                                                 ps[:, 0: gw * 128], AF.Relu,
                                                 bias=bm_sb[:, 2 * l + m: 2 * l + m + 1])
                        cur = nxt
                    for tt in range(gw):
                        psof = mp_.tile([128, 256], dt.float32, tag="mlpps")
                        pso = psof[:, 0:3]
                        for kc in range(2):
                            nc.tensor.matmul(pso,
                                             cur[:, kc, tt * 128: tt * 128 + 128],
                                             m4_sb[:, kc, :],
                                             start=(kc == 0), stop=(kc == 1))
                        o1 = ap_.tile([128, 3], dt.float32, tag="o1")
                        nc.vector.tensor_add(o1[:], pso, b4_sb[:])
                        nc.vector.tensor_add(out_sb[:, grp[tt], :], o1[:],
                                             base_tiles[tt][:])

        nc.sync.dma_start(
            out.rearrange("(t p) c -> p t c", p=128), out_sb[:])


# ============================ host preparation ==============================

def _host_prep(inputs):
    inp = np.asarray(inputs["inp"], f32)
    sc = np.asarray(inputs["sample_coord"], f32)
    cell = np.asarray(inputs["cell"], f32)

    enc_w = np.asarray(inputs["enc_w"], f32)
    ch_w = np.asarray(inputs["ch_w"], f32)

    w_enc = enc_w.transpose(1, 2, 3, 0).reshape(27, ENC).astype(bf16)
    w_chp = np.zeros((128, 3, DIM), bf16)
    w_ch2 = np.zeros((ENC, 3, DIM), bf16)
    for ky in range(3):
        w_chp[0:64, ky, :] = ch_w[:, :, ky, 0].T.astype(bf16)
        w_chp[64:128, ky, :] = ch_w[:, :, ky, 1].T.astype(bf16)
        w_ch2[:, ky, :] = ch_w[:, :, ky, 2].T.astype(bf16)

    # qkv weights M-packed: 576 output cols = [q 192 | k 192 | v 192]
    w_qkv0 = np.zeros((128, 9, 576), bf16)
    w_qkv1p = np.zeros((128, 3, 576), bf16)
    w_qkv1k2 = np.zeros((64, 3, 576), bf16)
    qkvb = np.zeros((128, 5), f32)
    # v output channels permuted d-major: device channel c' = d*8+h holds
    # reference channel h*24+d (lets the wv multiply run innermost-contiguous)
    cmap_v = (np.arange(192) % 8) * 24 + np.arange(192) // 8
    for ci, nm in enumerate(("q", "k", "v")):
        wt = np.asarray(inputs[f"{nm}_w"], f32)
        bt = np.asarray(inputs[f"{nm}_b"], f32)
        if nm == "v":
            wt = wt[cmap_v]
            bt = bt[cmap_v]
        cs_ = ci * 192
        for off in range(9):
            ky, kx = off // 3, off % 3
            w_qkv0[:, off, cs_: cs_ + 192] = wt[:, 0:128, ky, kx].T.astype(bf16)
        for ky in range(3):
            w_qkv1p[0:64, ky, cs_: cs_ + 192] = wt[:, 128:192, ky, 0].T.astype(bf16)
            w_qkv1p[64:128, ky, cs_: cs_ + 192] = wt[:, 128:192, ky, 1].T.astype(bf16)
            w_qkv1k2[:, ky, cs_: cs_ + 192] = wt[:, 128:192, ky, 2].T.astype(bf16)
        bfull = bt
        for g in range(5):
            msz = 128 if g < 4 else 64
            seg = np.arange(g * 128, g * 128 + msz)
            sel = (seg >= cs_) & (seg < cs_ + 192)
            if sel.any():
                qkvb[np.nonzero(sel)[0], g] = bfull[seg[sel] - cs_]

    # m0w rows permuted into 7 chunks of 1344 -> 1408 (zero padded); within a
    # chunk the device feature order is (dx, d, h): j = dx*192 + d*8 + h maps
    # to reference row (dy*7+dx)*192 + h*24 + d
    m0w_full = np.asarray(inputs["m0w"], f32)
    jj = np.arange(DYW)
    jdx, jc = jj // 192, jj % 192
    jd, jh = jc // 8, jc % 8
    ref_j = jdx * 192 + jh * 24 + jd
    perm = np.zeros((KBLK * 128, HID), f32)
    for i in range(RR):
        perm[i * DYW_P: i * DYW_P + DYW] = m0w_full[i * DYW + ref_j]
    m0w_dev = np.ascontiguousarray(
        perm.reshape(KBLK, 128, HID).transpose(1, 0, 2)).astype(bf16)

    m13w = np.zeros((128, 6, HID), bf16)
    for l in (1, 2, 3):
        wl = np.asarray(inputs[f"m{l}w"], f32)
        m13w[:, (l - 1) * 2 + 0, :] = wl[0:128].astype(bf16)
        m13w[:, (l - 1) * 2 + 1, :] = wl[128:256].astype(bf16)
    m4w_full = np.asarray(inputs["m4w"], f32)
    m4w = np.stack([m4w_full[0:128], m4w_full[128:256]], 1).astype(bf16)

    b4 = np.broadcast_to(np.asarray(inputs["m4b"], f32)[None, :], (128, 3)).copy()
    enc_bd = np.asarray(inputs["enc_b"], f32).reshape(ENC, 1)
    ch_bd = np.zeros((128, 2), f32)
    ch_bd[:, 0] = np.asarray(inputs["ch_b"], f32)[0:128]
    ch_bd[0:64, 1] = np.asarray(inputs["ch_b"], f32)[128:192]
    ident = np.eye(128, dtype=bf16)

    m0b = np.asarray(inputs["m0b"], f32)
    m0w_tail = m0w_full[RA * DIM: RA * DIM + 2]
    bias_rest = np.zeros((128, 8), f32)
    for l in (1, 2, 3):
        bl = np.asarray(inputs[f"m{l}b"], f32)
        bias_rest[:, 2 * l + 0] = bl[0:128]
        bias_rest[:, 2 * l + 1] = bl[128:256]

    # ---- ownership: core (bi, qc) owns queries with iy//16 == qc ----
    sqh = f32(1.0 / np.sqrt(HD))
    cy_all, cx_all = sc[..., 0], sc[..., 1]
    py_all = (cy_all + f32(1.0)) * f32(H * 0.5) - f32(0.5)
    px_all = (cx_all + f32(1.0)) * f32(W * 0.5) - f32(0.5)
    iy_all = np.clip(np.floor(py_all + f32(0.5)), 0, H - 1).astype(np.int64)
    ix_all = np.clip(np.floor(px_all + f32(0.5)), 0, W - 1).astype(np.int64)

    owners = []
    for core in range(N_CORES):
        bi, qc = core // 4, core % 4
        own = np.nonzero(iy_all[bi] // 16 == qc)[0]
        owners.append(own)
    NT = max((len(o) + 127) // 128 for o in owners)
    SLOTS = NT * 128

    batch_data = []
    for bi in range(B):
        x = inp[bi]
        xT = x.reshape(3, H * W).T
        hi = xT.astype(bf16).astype(f32)
        lo = (xT - hi).astype(bf16)
        hilo_full = np.concatenate([hi.astype(bf16), lo], 1)  # [4096, 6]

        rel_cell = cell[bi] * np.array([H, W], f32)
        b0 = m0b + rel_cell @ m0w_tail
        bm = bias_rest.copy()
        bm[:, 0] = b0[0:128]
        bm[:, 1] = b0[128:256]
        batch_data.append((x, hilo_full, bm))

    d = np.arange(-R, R + 1)
    percore = []
    for core in range(N_CORES):
        bi, qc = core // 4, core % 4
        R0 = 16 * qc
        own = owners[core]
        n = len(own)
        x, hilo_full, bm = batch_data[bi]

        # padded image: padded row p <-> image row R0 + p - 35
        xbig = np.zeros((CH_IN, 128, W + 2), f32)
        plo, phi = max(0, 35 - R0), min(128, 35 - R0 + H)
        xbig[:, plo:phi, 1:1 + W] = x[:, plo - 35 + R0: phi - 35 + R0, :]

        # im2col for enc strips 2..7 (rows 16..63)
        col = np.zeros((27, 6 * 512), bf16)
        for c in range(CH_IN):
            for ky in range(3):
                for kx in range(3):
                    col[c * 9 + ky * 3 + kx] = \
                        xbig[c, 15 + ky: 63 + ky, kx: kx + W] \
                        .reshape(-1).astype(bf16)
        tr = np.arange(16, 64) - 35 + R0  # image rows of enc output strips
        col.reshape(27, 48, W)[:, (tr < 0) | (tr >= H), :] = 0

        # ch rows 31..56 mask (image rows R0-4 .. R0+21)
        trf = np.arange(26) + R0 - 4
        maskfp = np.broadcast_to(
            ((trf >= 0) & (trf < H)).astype(bf16)[None, :], (128, 26)).copy()

        # hilo for local pixel rows 0..23 (image rows R0-3 .. R0+20)
        hl = np.zeros((NPX, 6), bf16)
        rlo, rhi = max(0, R0 - 3), min(H, R0 + 21)
        hl[(rlo - R0 + 3) * W: (rhi - R0 + 3) * W] = \
            hilo_full[rlo * W: rhi * W]
        hilo = np.ascontiguousarray(hl.reshape(12, 128, 6).transpose(1, 0, 2))

        # ---- per-query gather indices / weights ----
        iy, ix = iy_all[bi][own], ix_all[bi][own]
        py, px = py_all[bi][own], px_all[bi][own]

        dy_, dx_ = [a.reshape(-1) for a in np.meshgrid(d, d, indexing="ij")]
        yy = iy[:, None] + dy_[None, :]
        xx = ix[:, None] + dx_[None, :]
        valid = ((yy >= 0) & (yy < H) & (xx >= 0) & (xx < W)).astype(f32)

        # kv entry index for group g: entry (iy - R0 + 4g)*64 + ix - 3,
        # tensor row = HEADKV + entry = 192 + (iy-R0+4g)*64 + ix
        kvstart = np.zeros((n, 2), np.int64)
        for g in range(2):
            kvstart[:, g] = 192 + (iy - R0 + 4 * g) * 64 + ix

        y0 = np.floor(py)
        x0 = np.floor(px)
        wy, wx = py - y0, px - x0
        y0 = y0.astype(np.int64)
        x0 = x0.astype(np.int64)
        sy0 = np.clip(y0, 0, H - 2)
        sx0 = np.clip(x0, 0, W - 2)
        wq_eff = np.zeros((n, 2, 2), f32)
        wb_eff = np.zeros((n, 2, 2), f32)
        qq = np.arange(n)
        for ddy, syw in ((0, 1 - wy), (1, wy)):
            for ddx, sxw in ((0, 1 - wx), (1, wx)):
                w = (syw * sxw).astype(f32)
                yc, xc = y0 + ddy, x0 + ddx
                ly = np.clip(yc, 0, H - 1) - sy0
                lx = np.clip(xc, 0, W - 1) - sx0
                wb_eff[qq, ly, lx] += w
                vm = ((yc >= 0) & (yc < H) & (xc >= 0) & (xc < W))
                wq_eff[qq, ly, lx] += w * vm
        # qg4 layout is [x(sx), y(sy)] -> reorder weights to (lx, ly)
        wq4 = wq_eff.transpose(0, 2, 1).reshape(n, 4)
        wb4 = wb_eff.transpose(0, 2, 1).reshape(n, 4)
        qstart = HEADQ + (sy0 - R0 + 3) * 64 + sx0

        # pad to SLOTS
        def padto(a, fill):
            outp = np.full((SLOTS,) + a.shape[1:], fill, a.dtype)
            outp[:n] = a
            return outp

        kvstart_p = padto(kvstart, 192)
        qstart_p = padto(qstart, HEADQ + 128)
        valid_p = padto(valid, 0.0)
        wq4_p = padto(wq4.astype(f32), 0.0)
        wb4_p = padto(wb4, 0.0)

        kvidx = np.zeros((128, NT, 16), np.int16)
        qidx = np.zeros((128, NT, 8), np.int16)
        masktt = np.zeros((128, NT, 56), f32)
        qwt = np.zeros((128, NT, 4), bf16)
        qwbt = np.zeros((128, NT, 4), f32)
        # mask in device (g, x, s) order: u = g*28 + x*4 + s, dy = g*4+s, dx = x
        uu = np.arange(56)
        gu, xu, su = uu // 28, (uu % 28) // 4, uu % 4
        dyu = gu * 4 + su
        usel = dyu < 7
        for t in range(NT):
            ts = slice(t * 128, (t + 1) * 128)
            masktt[:, t, uu[usel]] = valid_p[ts][:, dyu[usel] * 7 + xu[usel]]
            qwt[:, t, :] = (wq4_p[ts] * sqh).astype(bf16)
            qwbt[:, t, :] = wb4_p[ts]
            flat = kvstart_p[ts].T.reshape(-1)  # j = g*128 + q
            kvidx[:, t, :] = np.tile(flat.reshape(-1, 16).T, (8, 1)).astype(np.int16)
            fq = qstart_p[ts]
            qidx[:, t, :] = np.tile(fq.reshape(-1, 16).T, (8, 1)).astype(np.int16)

        percore.append({
            "inp_col": col, "inp_hilo": hilo,
            "w_enc": w_enc, "w_chp": w_chp, "w_ch2": w_ch2,
            "w_qkv0": w_qkv0, "w_qkv1p": w_qkv1p, "w_qkv1k2": w_qkv1k2,
            "qkvb": qkvb, "maskfp": maskfp, "enc_b": enc_bd, "ch_b": ch_bd,
            "m0w": m0w_dev, "m13w": m13w, "m4w": m4w, "bmlp": bm, "b4": b4,
            "ident": ident, "kvidx": kvidx, "qidx": qidx, "maskt": masktt,
            "qwt": qwt, "qwbt": qwbt,
        })
    return percore, NT, owners


# ============================== entry point =================================

def _get_program(NT):
    if NT not in _PROGRAMS:
        _PROGRAMS[NT] = build_program(NT)
    return _PROGRAMS[NT]


def kernel(**inputs):
    from concourse import bass_utils
    in_maps, NT, owners = _host_prep(inputs)
    nc = _get_program(NT)
    res = bass_utils.run_bass_kernel_spmd(nc, in_maps, core_ids=list(range(N_CORES)))
    full = np.empty((B, Q, 3), f32)
    for core in range(N_CORES):
        bi = core // 4
        own = owners[core]
        full[bi, own] = res.results[core]["out"][:len(own)]
    return full


if __name__ == "__main__":
    import time
    t0 = time.time()
    nc = _get_program(9)
    print("built+compiled in", time.time() - t0, "s")


# revision 25
# speedup vs baseline: 1.0825x; 1.0003x over previous
"""Trainium2 Bass kernel for the CLIT-style sparse local attention module.

Strategy (8 NeuronCores, SPMD, no collectives):
  - core c = (batch bi = c // 4, strip qc = c % 4) OWNS the queries whose
    nearest-pixel center row falls in image rows [16qc, 16qc+16). Each core
    computes the 5 convs locally for the 24-row band [16qc-3, 16qc+21)
    (own 16 rows + 3-row halo each side + 2 spare), so every window/bilinear
    read its queries need is produced locally -- the AllGather disappears.
  - qkv conv outputs are PE-transposed to pixel-major entries in local DRAM
    with a 4x row-shifted duplication: entry (r, x) holds rows r..r+3 of
    column x back to back. A 7x7 window then needs only TWO dma_gather
    descriptors per query (4+3 window rows each), and the bilinear q read
    needs ONE (2x2 patch via 2x row duplication).
  - v is stored as fp8e4 (k stays bf16): halves the v gather traffic; the
    wv multiply runs at DVE 1x anyway (broadcast attn operand), so fp8
    costs nothing there. rel-err budget is ~2e-2; bf16 baseline is ~8e-5.
  - Attention per 128-query tile: QK logits as DVE mul + binary-tree adds
    (tensor_reduce is 1x-mode and slow), softmax, attention-weighted v
    (split DVE/GpSimd), PE-transposed into the K-major layout for the
    9410x256 MLP0 (bf16, fp32 PSUM), then MLP1-3 + head per tile group.
  - Host precomputes ownership, gather indices, bilinear weights, masks;
    kernel() scatters per-core outputs back to the original query order.
"""

import sys

sys.path.insert(0, "/opt/trn_rl_repo")

import numpy as np
import ml_dtypes

# ---------------- problem constants (hardcoded per contract) ----------------
B, CH_IN, H, W = 2, 3, 64, 64
Q = 4096
DIM, HEAD, R = 192, 8, 3
RR = 2 * R + 1
RA = RR * RR          # 49
HD = DIM // HEAD      # 24
ENC = 64
HID = 256
N_CORES = 8

NROWS = 24            # local conv band rows (strip 16 + 3 halo + 2 spare)
NPX = NROWS * W       # 1536 pixel entries
SUB_E = 288           # bf16 slots per kv sub-entry: 192 k bf16 + 192 v fp8
KV_ENT = 4 * SUB_E    # 1152: entry holds rows r..r+3
HEADKV = 195          # 192 write-shift room + 3 px guard
KVROWS = HEADKV + NPX # 1731
Q_ENT = 2 * 256       # q entry holds rows r..r+1
HEADQ = 64
QROWS = HEADQ + NPX   # 1600

DYW = RR * DIM                    # 1344 columns per window-row chunk
DYW_P = 1408                      # padded to 11 x 128
DY_BLOCKS = DYW_P // 128          # 11
KBLK = RR * DY_BLOCKS             # 77 K-blocks for MLP layer 0

f32 = np.float32
bf16 = ml_dtypes.bfloat16

_PROGRAMS = {}  # cached compiled Bass programs keyed by NT


def _groups(nt):
    gs = []
    t = 0
    while t < nt:
        gs.append(tuple(range(t, min(t + 2, nt))))
        t += 2
    return gs


# ============================ device program ================================

def build_program(NT):
    import concourse.bass as bass
    import concourse.tile as tile
    from concourse import bacc, mybir

    dt = mybir.dt

    nc = bacc.Bacc("TRN2", target_bir_lowering=False, debug=False,
                   enable_asserts=False, num_devices=N_CORES)

    def din(name, shape, dtype):
        return nc.dram_tensor(name, list(shape), dtype, kind="ExternalInput").ap()

    inp_col = din("inp_col", [27, 6 * 512], dt.bfloat16)
    inp_hilo = din("inp_hilo", [128, 12, 6], dt.bfloat16)
    w_enc = din("w_enc", [27, ENC], dt.bfloat16)
    w_chp = din("w_chp", [128, 3, DIM], dt.bfloat16)
    w_ch2 = din("w_ch2", [ENC, 3, DIM], dt.bfloat16)
    w_qkv0 = din("w_qkv0", [128, 9, 576], dt.bfloat16)
    w_qkv1p = din("w_qkv1p", [128, 3, 576], dt.bfloat16)
    w_qkv1k2 = din("w_qkv1k2", [64, 3, 576], dt.bfloat16)
    qkvb = din("qkvb", [128, 5], dt.float32)
    maskfp = din("maskfp", [128, 26], dt.bfloat16)
    enc_b = din("enc_b", [ENC, 1], dt.float32)
    ch_b = din("ch_b", [128, 2], dt.float32)
    m0w = din("m0w", [128, KBLK, HID], dt.bfloat16)
    m13w = din("m13w", [128, 6, HID], dt.bfloat16)
    m4w = din("m4w", [128, 2, 3], dt.bfloat16)
    bmlp = din("bmlp", [128, 8], dt.float32)
    b4 = din("b4", [128, 3], dt.float32)
    ident = din("ident", [128, 128], dt.bfloat16)
    kvidx = din("kvidx", [128, NT, 16], dt.int16)
    qidx = din("qidx", [128, NT, 8], dt.int16)
    maskt = din("maskt", [128, NT, 56], dt.float32)
    qwt = din("qwt", [128, NT, 4], dt.bfloat16)
    qwbt = din("qwbt", [128, NT, 4], dt.float32)
    out = nc.dram_tensor("out", [NT * 128, 3], dt.float32,
                         kind="ExternalOutput").ap()

    with tile.TileContext(nc) as tc:
        with tc.tile_pool(name="dram", bufs=1, space="DRAM") as dp:
            kv4 = dp.tile([KVROWS, KV_ENT], dt.bfloat16)
            q2 = dp.tile([QROWS, Q_ENT], dt.bfloat16)

            _convs(nc, tc, mybir, locals())
            _attention(nc, tc, mybir, NT, locals())

    nc.compile()
    return nc


def _convs(nc, tc, mybir, env):
    dt = mybir.dt
    AF = mybir.ActivationFunctionType

    inp_col, w_enc = env["inp_col"], env["w_enc"]
    w_chp, w_ch2 = env["w_chp"], env["w_ch2"]
    w_qkv0, w_qkv1p, w_qkv1k2 = env["w_qkv0"], env["w_qkv1p"], env["w_qkv1k2"]
    qkvb = env["qkvb"]
    enc_b, ch_b, inp_hilo = env["enc_b"], env["ch_b"], env["inp_hilo"]
    kv4, q2 = env["kv4"], env["q2"]

    with (
        tc.tile_pool(name="cw", bufs=1) as cw,
        tc.tile_pool(name="cfeat", bufs=1) as cf,
        tc.tile_pool(name="cpsum", bufs=2, space="PSUM") as cp,
        tc.tile_pool(name="qpsum", bufs=2, space="PSUM") as cpq,
        tc.tile_pool(name="ctpsum", bufs=2, space="PSUM") as cpt,
        tc.tile_pool(name="cstage", bufs=3) as cs,
    ):
        col_sb = cw.tile([27, 6 * 512], dt.bfloat16)
        nc.sync.dma_start(col_sb[:], inp_col)
        wenc_sb = cw.tile([27, ENC], dt.bfloat16)
        nc.sync.dma_start(wenc_sb[:], w_enc)
        wchp_sb = cw.tile([128, 3, DIM], dt.bfloat16)
        nc.sync.dma_start(wchp_sb[:], w_chp)
        wch2_sb = cw.tile([ENC, 3, DIM], dt.bfloat16)
        nc.sync.dma_start(wch2_sb[:], w_ch2)
        wq0 = cw.tile([128, 9, 576], dt.bfloat16)
        nc.sync.dma_start(wq0[:], w_qkv0)
        wq1p = cw.tile([128, 3, 576], dt.bfloat16)
        nc.sync.dma_start(wq1p[:], w_qkv1p)
        wq1k2 = cw.tile([64, 3, 576], dt.bfloat16)
        nc.sync.dma_start(wq1k2[:], w_qkv1k2)
        qkvb_sb = cw.tile([128, 5], dt.float32)
        nc.sync.dma_start(qkvb_sb[:], qkvb)
        encb_sb = cw.tile([ENC, 1], dt.float32)
        nc.sync.dma_start(encb_sb[:], enc_b)
        chb_sb = cw.tile([128, 2], dt.float32)
        nc.sync.dma_start(chb_sb[:], ch_b)
        hilo_sb = cw.tile([128, 12, 6], dt.bfloat16)
        nc.sync.dma_start(hilo_sb[:], inp_hilo)
        id_c = cw.tile([128, 128], dt.bfloat16)
        nc.sync.dma_start(id_c[:], env["ident"])
        maskfp_sb = cw.tile([128, 26], dt.bfloat16)
        nc.sync.dma_start(maskfp_sb[:], env["maskfp"])

        # zero the kv head region (write-shift room + guard entries): rows
        # 0..HEADKV of kv4; shifted writes partially overwrite it afterwards.
        zt = cw.tile([128, 1755], dt.bfloat16)
        nc.vector.memset(zt[:], 0.0)
        kvf = kv4[:, :].flatten()
        nc.sync.dma_start(kvf[0: HEADKV * KV_ENT]
                          .rearrange("(p a) -> p a", p=128), zt[:])

        encp = cf.tile([ENC, 66, 66], dt.bfloat16)
        nc.gpsimd.memset(encp[:], 0.0)
        fp0 = cf.tile([128, 66, 66], dt.bfloat16)
        nc.gpsimd.memset(fp0[:], 0.0)
        fp1 = cf.tile([64, 66, 66], dt.bfloat16)
        nc.gpsimd.memset(fp1[:], 0.0)
        encb2 = cf.tile([128, 66, 66], dt.bfloat16)
        fp1b = cf.tile([128, 66, 66], dt.bfloat16)

        # ---- enc conv (strips 2-7) ----
        for t in range(2, 8):
            ps = cp.tile([128, 512], dt.float32, tag="cps")
            nc.tensor.matmul(ps[:ENC, :], wenc_sb[:],
                             col_sb[:, (t - 2) * 512:(t - 1) * 512],
                             start=True, stop=True)
            dst = encp[:, 1 + t * 8: 1 + t * 8 + 8, 1:65]
            nc.scalar.activation(dst, ps[:ENC, :].rearrange("p (a b) -> p a b", a=8),
                                 AF.Identity, bias=encb_sb[:, 0:1])

        # encb2: enc features with a one-column-shifted copy in partitions 64:
        nc.vector.tensor_copy(encb2[0:64, 24:66, :], encp[:, 24:66, :])
        nc.vector.tensor_copy(encb2[64:128, 24:66, 0:65], encp[:, 24:66, 1:66])

        # ---- ch conv (strips 3-7; kx 0/1 paired into K=128, kx=2 single) ----
        for t in range(3, 8):
            for m, msz in ((0, 128), (1, 64)):
                ps = cp.tile([128, 512], dt.float32, tag="cps")
                for ky in range(3):
                    rhs = encb2[:, t * 8 + ky: t * 8 + ky + 8, 0:64]
                    nc.tensor.matmul(ps[:msz, :],
                                     wchp_sb[:, ky, m * 128: m * 128 + msz],
                                     rhs, start=(ky == 0), stop=False)
                for ky in range(3):
                    rhs = encp[:, t * 8 + ky: t * 8 + ky + 8, 2:66]
                    nc.tensor.matmul(ps[:msz, :],
                                     wch2_sb[:, ky, m * 128: m * 128 + msz],
                                     rhs, start=False, stop=(ky == 2))
                dstp = (fp0 if m == 0 else fp1)
                dst = dstp[:msz, 1 + t * 8: 1 + t * 8 + 8, 1:65]
                nc.scalar.activation(dst,
                                     ps[:msz, :].rearrange("p (a b) -> p a b", a=8),
                                     AF.Identity, bias=chb_sb[:msz, m: m + 1])

        # zero ch features of out-of-image rows (qkv convs read fp rows 32..57)
        mbc = maskfp_sb[:].unsqueeze(2).broadcast_to((128, 26, 66))
        nc.vector.tensor_mul(fp0[:, 32:58, :], fp0[:, 32:58, :], mbc)
        nc.vector.tensor_mul(fp1[:, 32:58, :], fp1[:, 32:58, :],
                             maskfp_sb[0:64, :].unsqueeze(2)
                             .broadcast_to((64, 26, 66)))

        # fp1b: channel-chunk-1 features with one-column-shifted copy
        nc.vector.tensor_copy(fp1b[0:64, 32:58, :], fp1[:, 32:58, :])
        nc.vector.tensor_copy(fp1b[64:128, 32:58, 0:65], fp1[:, 32:58, 1:66])

        # ---- q/k/v convs (strips 4-6; M-packed 576 = 4x128+64) + transpose ----
        # 576 cols = [q 0:192 | k 192:384 | v 384:576]
        # per group: list of (dst_kind, src_off, dst_off, n)
        #   kind 0 = qstage bf16, 1 = kvstage k bf16, 2 = kvstage v fp8 view
        gdst = [
            [(0, 0, 0, 128)],
            [(0, 0, 128, 64), (1, 64, 0, 64)],
            [(1, 0, 64, 128)],
            [(2, 0, 0, 128)],
            [(2, 0, 128, 64)],
        ]
        for t in (4, 5, 6):
            kvstage = cs.tile([128, 4, SUB_E], dt.bfloat16, tag="kvstage")
            qstage = cs.tile([128, 4, 256], dt.bfloat16, tag="qstage")
            vview = kvstage[:, :, 192:SUB_E].bitcast(dt.float8e4)
            nc.vector.memset(qstage[:, :, 198:], 0.0)
            nc.vector.tensor_copy(qstage[:, :, 192:198],
                                  hilo_sb[:, (t - 4) * 4:(t - 3) * 4, :])
            for g in range(5):
                msz = 128 if g < 4 else 64
                ps = cpq.tile([128, 512], dt.float32, tag="qkvps")
                for off in range(9):
                    ky, kx = off // 3, off % 3
                    rhs0 = fp0[:, t * 8 + ky: t * 8 + ky + 8, kx: kx + 64]
                    nc.tensor.matmul(ps[:msz, :],
                                     wq0[:, off, g * 128: g * 128 + msz],
                                     rhs0, start=(off == 0), stop=False)
                for ky in range(3):
                    rhs1 = fp1b[:, t * 8 + ky: t * 8 + ky + 8, 0:64]
                    nc.tensor.matmul(ps[:msz, :],
                                     wq1p[:, ky, g * 128: g * 128 + msz],
                                     rhs1, start=False, stop=False)
                for ky in range(3):
                    rhs1 = fp1[:, t * 8 + ky: t * 8 + ky + 8, 2:66]
                    nc.tensor.matmul(ps[:msz, :],
                                     wq1k2[:, ky, g * 128: g * 128 + msz],
                                     rhs1, start=False, stop=(ky == 2))
                csb = cs.tile([128, 512], dt.bfloat16, tag="convsb")
                nc.scalar.activation(csb[:msz, :], ps[:msz, :], AF.Identity,
                                     bias=qkvb_sb[:msz, g: g + 1])
                tps = cpt.tile([128, 512], dt.bfloat16, tag="ctps")
                for blk in range(4):
                    nc.tensor.transpose(
                        tps[:, blk * 128: blk * 128 + msz],
                        csb[:msz, blk * 128:(blk + 1) * 128],
                        id_c[:msz, :msz])
                tview = tps[:].rearrange("p (a b) -> p a b", a=4)
                for kind, so, do, n in gdst[g]:
                    if kind == 0:
                        nc.scalar.copy(qstage[:, :, do: do + n],
                                       tview[:, :, so: so + n])
                    elif kind == 1:
                        nc.scalar.copy(kvstage[:, :, do: do + n],
                                       tview[:, :, so: so + n])
                    else:
                        nc.scalar.copy(vview[:, :, do: do + n],
                                       tview[:, :, so: so + n])
            # 4x / 2x row-shifted duplicated writes: entry (r, x) sub s holds
            # local row r+s. Strip t covers local rows LT..LT+7.
            LT = (t - 4) * 8
            for s in range(4):
                r0 = HEADKV + (LT - s) * 64
                nc.sync.dma_start(
                    kv4[r0: r0 + 512, s * SUB_E: (s + 1) * SUB_E]
                    .rearrange("(b p) e -> p b e", p=128), kvstage[:])
            for s in range(2):
                r0 = HEADQ + (LT - s) * 64
                nc.sync.dma_start(
                    q2[r0: r0 + 512, s * 256: (s + 1) * 256]
                    .rearrange("(b p) e -> p b e", p=128), qstage[:])


def _attention(nc, tc, mybir, NT, env):
    import concourse.bass as bass
    dt = mybir.dt
    AX = mybir.AxisListType
    AF = mybir.ActivationFunctionType

    kv4, q2 = env["kv4"], env["q2"]
    m0w, m13w, m4w = env["m0w"], env["m13w"], env["m4w"]
    bmlp, b4, ident = env["bmlp"], env["b4"], env["ident"]
    kvidx, qidx = env["kvidx"], env["qidx"]
    maskt, qwt, qwbt = env["maskt"], env["qwt"], env["qwbt"]
    out = env["out"]

    with (
        tc.tile_pool(name="aw", bufs=1) as aw,
        tc.tile_pool(name="gath", bufs=2) as gp,
        tc.tile_pool(name="attn", bufs=3) as ap_,
        tc.tile_pool(name="attn3", bufs=3) as ap3,
        tc.tile_pool(name="wvp", bufs=6) as wvp,
        tc.tile_pool(name="xkp", bufs=1) as xkp,
        tc.tile_pool(name="hp", bufs=2) as hp,
        tc.tile_pool(name="outp", bufs=1) as op_,
        tc.tile_pool(name="tpsum", bufs=2, space="PSUM") as tp_,
        tc.tile_pool(name="mpsum", bufs=2, space="PSUM") as mp_,
        tc.tile_pool(name="m0psum", bufs=2, space="PSUM") as mp0,
    ):
        m0w_sb = aw.tile([128, KBLK, HID], dt.bfloat16)
        nc.sync.dma_start(m0w_sb[:], m0w)
        m13_sb = aw.tile([128, 6, HID], dt.bfloat16)
        nc.sync.dma_start(m13_sb[:], m13w)
        m4_sb = aw.tile([128, 2, 3], dt.bfloat16)
        nc.sync.dma_start(m4_sb[:], m4w)
        bm_sb = aw.tile([128, 8], dt.float32)
        nc.sync.dma_start(bm_sb[:], bmlp)
        b4_sb = aw.tile([128, 3], dt.float32)
        nc.sync.dma_start(b4_sb[:], b4)
        id_sb = aw.tile([128, 128], dt.bfloat16)
        nc.sync.dma_start(id_sb[:], ident)
        kvi_sb = aw.tile([128, NT, 16], dt.int16)
        nc.sync.dma_start(kvi_sb[:], kvidx)
        qi_sb = aw.tile([128, NT, 8], dt.int16)
        nc.sync.dma_start(qi_sb[:], qidx)
        mk_sb = aw.tile([128, NT, 56], dt.float32)
        nc.sync.dma_start(mk_sb[:], maskt)
        qw_sb = aw.tile([128, NT, 4], dt.bfloat16)
        nc.sync.dma_start(qw_sb[:], qwt)
        qwb_sb = aw.tile([128, NT, 4], dt.float32)
        nc.sync.dma_start(qwb_sb[:], qwbt)

        out_sb = op_.tile([128, NT, 3], dt.float32)

        kv_ap = kv4[:, :]
        kv_ap = bass.AP(kv_ap.tensor, kv_ap.offset,
                        [[KV_ENT, KVROWS - 6], [1, RR * KV_ENT]])
        q_ap = q2[:, :]
        q_ap = bass.AP(q_ap.tensor, q_ap.offset,
                       [[Q_ENT, QROWS - 2], [1, 2 * Q_ENT]])

        def issue_gathers(t):
            kvg = gp.tile([128, 2, RR * KV_ENT], dt.bfloat16, tag="kvg")
            nc.gpsimd.dma_gather(kvg[:], kv_ap, kvi_sb[:, t, :],
                                 num_idxs=256, num_idxs_reg=256,
                                 elem_size=RR * KV_ENT, elem_step=KV_ENT,
                                 single_packet=False)
            qg = gp.tile([128, 1, 2 * Q_ENT], dt.bfloat16, tag="qg")
            nc.gpsimd.dma_gather(qg[:], q_ap, qi_sb[:, t, :],
                                 num_idxs=128, num_idxs_reg=128,
                                 elem_size=2 * Q_ENT, elem_step=Q_ENT,
                                 single_packet=False)
            return qg, kvg

        groups = _groups(NT)
        base_tiles = [None, None]
        mlp_ps = None
        xchunks = None

        pending = issue_gathers(0)
        for gi, grp in enumerate(groups):
            gw = len(grp)  # tiles in this MLP group (2 or trailing 1)
            for ti, t in enumerate(grp):
                qg, kvg = pending
                if t + 1 < NT:
                    # desc-gen first in the gpsimd queue so the next tile's
                    # gather DMA overlaps this tile's compute fully
                    pending = issue_gathers(t + 1)
                kvv = kvg[:].rearrange("p g (x s e) -> p g x s e", x=RR, s=4)
                qg4 = qg[:].rearrange("p o (x y e) -> p o x y e", x=2, y=2)

                def kslice(dy):
                    g, s = (0, dy) if dy < 4 else (1, dy - 4)
                    return kvv[:, g, :, s, 0:DIM]

                def vslice(dy):
                    g, s = (0, dy) if dy < 4 else (1, dy - 4)
                    return kvv[:, g, :, s, DIM:SUB_E].bitcast(dt.float8e4)

                # ---------------- q vector (bilinear blend) + base ----------
                qprod = ap_.tile([128, 2, 2, DIM], dt.bfloat16, tag="qprod")
                nc.vector.tensor_mul(
                    qprod[:], qg4[:, 0, :, :, 0:DIM],
                    qw_sb[:, t, :].rearrange("p (a b) -> p a b", a=2)
                    .unsqueeze(3).broadcast_to((128, 2, 2, DIM)))
                qt1 = ap_.tile([128, 2, DIM], dt.bfloat16, tag="qt1")
                nc.vector.tensor_add(qt1[:], qprod[:, 0], qprod[:, 1])
                qb = ap_.tile([128, DIM], dt.bfloat16, tag="qb")
                nc.vector.tensor_add(qb[:], qt1[:, 0], qt1[:, 1])

                badd = ap_.tile([128, 2, 2, 3], dt.float32, tag="badd")
                nc.vector.tensor_add(badd[:], qg4[:, 0, :, :, 192:195],
                                     qg4[:, 0, :, :, 195:198])
                bprod = ap_.tile([128, 2, 2, 3], dt.float32, tag="bprod")
                nc.vector.tensor_mul(
                    bprod[:], badd[:],
                    qwb_sb[:, t, :].rearrange("p (a b) -> p a b", a=2)
                    .unsqueeze(3).broadcast_to((128, 2, 2, 3)))
                bt1 = ap_.tile([128, 2, 3], dt.float32, tag="bt1")
                nc.vector.tensor_add(bt1[:], bprod[:, 0], bprod[:, 1])
                base_t = ap_.tile([128, 3], dt.float32, tag="base")
                nc.vector.tensor_add(base_t[:], bt1[:, 0], bt1[:, 1])
                base_tiles[ti] = base_t

                # -------- QK logits: 2 bulk muls + per-group binary trees ----
                # logits layout [128, g 2, x 7, s 4, h 8]; (g1, s3) = junk;
                # junk is masked to 0, exp's to 1, and the 7 spurious ones are
                # subtracted from the softmax denominator.
                logits = ap3.tile([128, 2, RR, 4, HEAD], dt.float32, tag="logits")
                for g in range(2):
                    e = ap3.tile([128, RR, 4, DIM], dt.bfloat16, tag="emul",
                                 bufs=1)
                    nc.vector.tensor_mul(
                        e[:], kvv[:, g, :, :, 0:DIM],
                        qb[:].unsqueeze(1).unsqueeze(2)
                        .broadcast_to((128, RR, 4, DIM)))
                    ev = e[:].rearrange("p x s (h d) -> p (x s) h d", h=HEAD)
                    l1 = ap3.tile([128, 28, HEAD, 12], dt.bfloat16, tag="l1",
                                  bufs=1)
                    nc.vector.tensor_add(l1[:], ev[:, :, :, 0:12], ev[:, :, :, 12:24])
                    l2 = ap3.tile([128, 28, HEAD, 6], dt.bfloat16, tag="l2",
                                  bufs=1)
                    nc.vector.tensor_add(l2[:], l1[:, :, :, 0:6], l1[:, :, :, 6:12])
                    l3 = ap3.tile([128, 28, HEAD, 3], dt.bfloat16, tag="l3",
                                  bufs=1)
                    nc.vector.tensor_add(l3[:], l2[:, :, :, 0:3], l2[:, :, :, 3:6])
                    l4 = ap3.tile([128, 28, HEAD, 1], dt.bfloat16, tag="l4",
                                  bufs=1)
                    nc.vector.tensor_add(l4[:], l3[:, :, :, 0:1], l3[:, :, :, 1:2])
                    nc.vector.tensor_add(
                        logits[:, g].rearrange("p x s h -> p (x s) h"),
                        l4[:].rearrange("p u h d -> p u (h d)"),
                        l3[:, :, :, 2:3].rearrange("p u h d -> p u (h d)"))

                lgv = logits[:].rearrange("p g x s h -> p (g x s) h")
                mask_bc = mk_sb[:, t, :].unsqueeze(2).broadcast_to((128, 56, HEAD))
                nc.vector.tensor_mul(lgv, lgv, mask_bc)
                nc.scalar.activation(lgv, lgv, AF.Exp)
                ssum = ap_.tile([128, HEAD], dt.float32, tag="ssum")
                nc.vector.reduce_sum(
                    ssum[:], logits[:].rearrange("p g x s h -> p h (g x s)"),
                    axis=AX.X)
                nc.vector.tensor_scalar_add(ssum[:], ssum[:], -7.0)
                rec = ap_.tile([128, HEAD], dt.float32, tag="rec")
                nc.vector.reciprocal(rec[:], ssum[:])
                nc.vector.tensor_mul(
                    lgv, lgv, rec[:].unsqueeze(1).broadcast_to((128, 56, HEAD)))
                attnb = ap3.tile([128, 2, RR, 4, HEAD], dt.bfloat16, tag="attnb")
                nc.vector.tensor_mul(
                    attnb[:].rearrange("p g x s h -> p (g x s) h"), lgv, mask_bc)

                # ---- weighted v -> per-dy chunks; MLP0 accumulates per chunk
                if ti == 0:
                    mlp_ps = mp0.tile([128, 512], dt.float32, tag="mlp0ps",
                                      name="mlp0ps")
                    xchunks = [xkp.tile([128, DY_BLOCKS, 256], dt.bfloat16,
                                        tag=f"xc{dy}", name=f"xc{dy}")
                               for dy in range(RR)]

                def mlp0_chunk(dy):
                    chunk = xchunks[dy]
                    for m in range(2):
                        for b_ in range(DY_BLOCKS):
                            kb = dy * DY_BLOCKS + b_
                            nc.tensor.matmul(
                                mlp_ps[:, m * 256: m * 256 + gw * 128],
                                m0w_sb[:, kb, m * 128:(m + 1) * 128],
                                chunk[:, b_, 0: gw * 128], start=(kb == 0),
                                stop=(kb == KBLK - 1), skip_group_check=True)

                xcol = ti * 128
                for dy in range(RR):
                    wv = wvp.tile([128, DYW_P], dt.bfloat16, tag="wv")
                    nc.gpsimd.memset(wv[:, DYW:], 0.0)
                    weng = nc.gpsimd if dy >= 5 else nc.vector
                    # v channels are stored d-major (c' = d*8 + h) so that all
                    # three operands are innermost-contiguous -> DVE 2x mode
                    g_, s_ = (0, dy) if dy < 4 else (1, dy - 4)
                    weng.tensor_mul(
                        wv[:, 0:DYW].rearrange("p (r d h) -> p r d h", d=HD, h=HEAD),
                        vslice(dy).rearrange("p r (d h) -> p r d h", h=HEAD),
                        attnb[:, g_, :, s_, :].unsqueeze(2).broadcast_to(
                            (128, RR, HD, HEAD)))
                    for grp2, nb in ((0, 8), (1, 3)):
                        tps = tp_.tile([128, nb * 128], dt.bfloat16,
                                       tag=f"tps{grp2}")
                        for bi_ in range(nb):
                            blk = grp2 * 8 + bi_
                            nc.tensor.transpose(tps[:, bi_ * 128:(bi_ + 1) * 128],
                                                wv[:, blk * 128:(blk + 1) * 128],
                                                id_sb[:])
                        nc.scalar.copy(
                            xchunks[dy][:, grp2 * 8: grp2 * 8 + nb, xcol: xcol + 128],
                            tps[:].rearrange("p (a b) -> p a b", a=nb))
                    # emit chunk dy-1's MLP0 matmuls one dy late so the PE is
                    # not stalled on the PSUM->SBUF copy of the current chunk
                    if ti == gw - 1 and dy >= 1:
                        mlp0_chunk(dy - 1)

                # ---------------- MLP layers 0-3 + head (per tile group) -----
                if ti == gw - 1:
                    mlp0_chunk(RR - 1)
                    h0 = hp.tile([128, 2, 256], dt.bfloat16, tag="h")
                    for m in range(2):
                        nc.scalar.activation(h0[:, m, 0: gw * 128],
                                             mlp_ps[:, m * 256: m * 256 + gw * 128],
                                             AF.Relu, bias=bm_sb[:, m:m + 1])
                    cur = h0
                    for l in (1, 2, 3):
                        nxt = hp.tile([128, 2, 256], dt.bfloat16, tag="h")
                        for m in range(2):
                            ps = mp_.tile([128, 256], dt.float32, tag="mlpps")
                            for kc in range(2):
                                nc.tensor.matmul(
                                    ps[:, 0: gw * 128],
                                    m13_sb[:, (l - 1) * 2 + kc,
                                           m * 128:(m + 1) * 128],
                                    cur[:, kc, 0: gw * 128],
                                    start=(kc == 0), stop=(kc == 1))
                            nc.scalar.activation(nxt[:, m, 0: gw * 128],
                                                 ps[:, 0: gw * 128], AF.Relu,
                                                 bias=bm_sb[:, 2 * l + m: 2 * l + m + 1])
                        cur = nxt
                    for tt in range(gw):
                        psof = mp_.tile([128, 256], dt.float32, tag="mlpps")
                        pso = psof[:, 0:3]
                        for kc in range(2):
                            nc.tensor.matmul(pso,
                                             cur[:, kc, tt * 128: tt * 128 + 128],
                                             m4_sb[:, kc, :],
                                             start=(kc == 0), stop=(kc == 1))
                        o1 = ap_.tile([128, 3], dt.float32, tag="o1")
                        nc.vector.tensor_add(o1[:], pso, b4_sb[:])
                        nc.vector.tensor_add(out_sb[:, grp[tt], :], o1[:],
                                             base_tiles[tt][:])

        nc.sync.dma_start(
            out.rearrange("(t p) c -> p t c", p=128), out_sb[:])


# ============================ host preparation ==============================

def _host_prep(inputs):
    inp = np.asarray(inputs["inp"], f32)
    sc = np.asarray(inputs["sample_coord"], f32)
    cell = np.asarray(inputs["cell"], f32)

    enc_w = np.asarray(inputs["enc_w"], f32)
    ch_w = np.asarray(inputs["ch_w"], f32)

    w_enc = enc_w.transpose(1, 2, 3, 0).reshape(27, ENC).astype(bf16)
    w_chp = np.zeros((128, 3, DIM), bf16)
    w_ch2 = np.zeros((ENC, 3, DIM), bf16)
    for ky in range(3):
        w_chp[0:64, ky, :] = ch_w[:, :, ky, 0].T.astype(bf16)
        w_chp[64:128, ky, :] = ch_w[:, :, ky, 1].T.astype(bf16)
        w_ch2[:, ky, :] = ch_w[:, :, ky, 2].T.astype(bf16)

    # qkv weights M-packed: 576 output cols = [q 192 | k 192 | v 192]
    w_qkv0 = np.zeros((128, 9, 576), bf16)
    w_qkv1p = np.zeros((128, 3, 576), bf16)
    w_qkv1k2 = np.zeros((64, 3, 576), bf16)
    qkvb = np.zeros((128, 5), f32)
    # v output channels permuted d-major: device channel c' = d*8+h holds
    # reference channel h*24+d (lets the wv multiply run innermost-contiguous)
    cmap_v = (np.arange(192) % 8) * 24 + np.arange(192) // 8
    for ci, nm in enumerate(("q", "k", "v")):
        wt = np.asarray(inputs[f"{nm}_w"], f32)
        bt = np.asarray(inputs[f"{nm}_b"], f32)
        if nm == "v":
            wt = wt[cmap_v]
            bt = bt[cmap_v]
        cs_ = ci * 192
        for off in range(9):
            ky, kx = off // 3, off % 3
            w_qkv0[:, off, cs_: cs_ + 192] = wt[:, 0:128, ky, kx].T.astype(bf16)
        for ky in range(3):
            w_qkv1p[0:64, ky, cs_: cs_ + 192] = wt[:, 128:192, ky, 0].T.astype(bf16)
            w_qkv1p[64:128, ky, cs_: cs_ + 192] = wt[:, 128:192, ky, 1].T.astype(bf16)
            w_qkv1k2[:, ky, cs_: cs_ + 192] = wt[:, 128:192, ky, 2].T.astype(bf16)
        bfull = bt
        for g in range(5):
            msz = 128 if g < 4 else 64
            seg = np.arange(g * 128, g * 128 + msz)
            sel = (seg >= cs_) & (seg < cs_ + 192)
            if sel.any():
                qkvb[np.nonzero(sel)[0], g] = bfull[seg[sel] - cs_]

    # m0w rows permuted into 7 chunks of 1344 -> 1408 (zero padded); within a
    # chunk the device feature order is (dx, d, h): j = dx*192 + d*8 + h maps
    # to reference row (dy*7+dx)*192 + h*24 + d
    m0w_full = np.asarray(inputs["m0w"], f32)
    jj = np.arange(DYW)
    jdx, jc = jj // 192, jj % 192
    jd, jh = jc // 8, jc % 8
    ref_j = jdx * 192 + jh * 24 + jd
    perm = np.zeros((KBLK * 128, HID), f32)
    for i in range(RR):
        perm[i * DYW_P: i * DYW_P + DYW] = m0w_full[i * DYW + ref_j]
    m0w_dev = np.ascontiguousarray(
        perm.reshape(KBLK, 128, HID).transpose(1, 0, 2)).astype(bf16)

    m13w = np.zeros((128, 6, HID), bf16)
    for l in (1, 2, 3):
        wl = np.asarray(inputs[f"m{l}w"], f32)
        m13w[:, (l - 1) * 2 + 0, :] = wl[0:128].astype(bf16)
        m13w[:, (l - 1) * 2 + 1, :] = wl[128:256].astype(bf16)
    m4w_full = np.asarray(inputs["m4w"], f32)
    m4w = np.stack([m4w_full[0:128], m4w_full[128:256]], 1).astype(bf16)

    b4 = np.broadcast_to(np.asarray(inputs["m4b"], f32)[None, :], (128, 3)).copy()
    enc_bd = np.asarray(inputs["enc_b"], f32).reshape(ENC, 1)
    ch_bd = np.zeros((128, 2), f32)
    ch_bd[:, 0] = np.asarray(inputs["ch_b"], f32)[0:128]
    ch_bd[0:64, 1] = np.asarray(inputs["ch_b"], f32)[128:192]
    ident = np.eye(128, dtype=bf16)

    m0b = np.asarray(inputs["m0b"], f32)
    m0w_tail = m0w_full[RA * DIM: RA * DIM + 2]
    bias_rest = np.zeros((128, 8), f32)
    for l in (1, 2, 3):
        bl = np.asarray(inputs[f"m{l}b"], f32)
        bias_rest[:, 2 * l + 0] = bl[0:128]
        bias_rest[:, 2 * l + 1] = bl[128:256]

    # ---- ownership: core (bi, qc) owns queries with iy//16 == qc ----
    sqh = f32(1.0 / np.sqrt(HD))
    cy_all, cx_all = sc[..., 0], sc[..., 1]
    py_all = (cy_all + f32(1.0)) * f32(H * 0.5) - f32(0.5)
    px_all = (cx_all + f32(1.0)) * f32(W * 0.5) - f32(0.5)
    iy_all = np.clip(np.floor(py_all + f32(0.5)), 0, H - 1).astype(np.int64)
    ix_all = np.clip(np.floor(px_all + f32(0.5)), 0, W - 1).astype(np.int64)

    owners = []
    for core in range(N_CORES):
        bi, qc = core // 4, core % 4
        own = np.nonzero(iy_all[bi] // 16 == qc)[0]
        owners.append(own)
    NT = max((len(o) + 127) // 128 for o in owners)
    SLOTS = NT * 128

    batch_data = []
    for bi in range(B):
        x = inp[bi]
        xT = x.reshape(3, H * W).T
        hi = xT.astype(bf16).astype(f32)
        lo = (xT - hi).astype(bf16)
        hilo_full = np.concatenate([hi.astype(bf16), lo], 1)  # [4096, 6]

        rel_cell = cell[bi] * np.array([H, W], f32)
        b0 = m0b + rel_cell @ m0w_tail
        bm = bias_rest.copy()
        bm[:, 0] = b0[0:128]
        bm[:, 1] = b0[128:256]
        batch_data.append((x, hilo_full, bm))

    d = np.arange(-R, R + 1)
    percore = []
    for core in range(N_CORES):
        bi, qc = core // 4, core % 4
        R0 = 16 * qc
        own = owners[core]
        n = len(own)
        x, hilo_full, bm = batch_data[bi]

        # padded image: padded row p <-> image row R0 + p - 35
        xbig = np.zeros((CH_IN, 128, W + 2), f32)
        plo, phi = max(0, 35 - R0), min(128, 35 - R0 + H)
        xbig[:, plo:phi, 1:1 + W] = x[:, plo - 35 + R0: phi - 35 + R0, :]

        # im2col for enc strips 2..7 (rows 16..63)
        col = np.zeros((27, 6 * 512), bf16)
        for c in range(CH_IN):
            for ky in range(3):
                for kx in range(3):
                    col[c * 9 + ky * 3 + kx] = \
                        xbig[c, 15 + ky: 63 + ky, kx: kx + W] \
                        .reshape(-1).astype(bf16)
        tr = np.arange(16, 64) - 35 + R0  # image rows of enc output strips
        col.reshape(27, 48, W)[:, (tr < 0) | (tr >= H), :] = 0

        # ch rows 31..56 mask (image rows R0-4 .. R0+21)
        trf = np.arange(26) + R0 - 4
        maskfp = np.broadcast_to(
            ((trf >= 0) & (trf < H)).astype(bf16)[None, :], (128, 26)).copy()

        # hilo for local pixel rows 0..23 (image rows R0-3 .. R0+20)
        hl = np.zeros((NPX, 6), bf16)
        rlo, rhi = max(0, R0 - 3), min(H, R0 + 21)
        hl[(rlo - R0 + 3) * W: (rhi - R0 + 3) * W] = \
            hilo_full[rlo * W: rhi * W]
        hilo = np.ascontiguousarray(hl.reshape(12, 128, 6).transpose(1, 0, 2))

        # ---- per-query gather indices / weights ----
        iy, ix = iy_all[bi][own], ix_all[bi][own]
        py, px = py_all[bi][own], px_all[bi][own]

        dy_, dx_ = [a.reshape(-1) for a in np.meshgrid(d, d, indexing="ij")]
        yy = iy[:, None] + dy_[None, :]
        xx = ix[:, None] + dx_[None, :]
        valid = ((yy >= 0) & (yy < H) & (xx >= 0) & (xx < W)).astype(f32)

        # kv entry index for group g: entry (iy - R0 + 4g)*64 + ix - 3,
        # tensor row = HEADKV + entry = 192 + (iy-R0+4g)*64 + ix
        kvstart = np.zeros((n, 2), np.int64)
        for g in range(2):
            kvstart[:, g] = 192 + (iy - R0 + 4 * g) * 64 + ix

        y0 = np.floor(py)
        x0 = np.floor(px)
        wy, wx = py - y0, px - x0
        y0 = y0.astype(np.int64)
        x0 = x0.astype(np.int64)
        sy0 = np.clip(y0, 0, H - 2)
        sx0 = np.clip(x0, 0, W - 2)
        wq_eff = np.zeros((n, 2, 2), f32)
        wb_eff = np.zeros((n, 2, 2), f32)
        qq = np.arange(n)
        for ddy, syw in ((0, 1 - wy), (1, wy)):
            for ddx, sxw in ((0, 1 - wx), (1, wx)):
                w = (syw * sxw).astype(f32)
                yc, xc = y0 + ddy, x0 + ddx
                ly = np.clip(yc, 0, H - 1) - sy0
                lx = np.clip(xc, 0, W - 1) - sx0
                wb_eff[qq, ly, lx] += w
                vm = ((yc >= 0) & (yc < H) & (xc >= 0) & (xc < W))
                wq_eff[qq, ly, lx] += w * vm
        # qg4 layout is [x(sx), y(sy)] -> reorder weights to (lx, ly)
        wq4 = wq_eff.transpose(0, 2, 1).reshape(n, 4)
        wb4 = wb_eff.transpose(0, 2, 1).reshape(n, 4)
        qstart = HEADQ + (sy0 - R0 + 3) * 64 + sx0

        # pad to SLOTS
        def padto(a, fill):
            outp = np.full((SLOTS,) + a.shape[1:], fill, a.dtype)
            outp[:n] = a
            return outp

        kvstart_p = padto(kvstart, 192)
        qstart_p = padto(qstart, HEADQ + 128)
        valid_p = padto(valid, 0.0)
        wq4_p = padto(wq4.astype(f32), 0.0)
        wb4_p = padto(wb4, 0.0)

        kvidx = np.zeros((128, NT, 16), np.int16)
        qidx = np.zeros((128, NT, 8), np.int16)
        masktt = np.zeros((128, NT, 56), f32)
        qwt = np.zeros((128, NT, 4), bf16)
        qwbt = np.zeros((128, NT, 4), f32)
        # mask in device (g, x, s) order: u = g*28 + x*4 + s, dy = g*4+s, dx = x
        uu = np.arange(56)
        gu, xu, su = uu // 28, (uu % 28) // 4, uu % 4
        dyu = gu * 4 + su
        usel = dyu < 7
        for t in range(NT):
            ts = slice(t * 128, (t + 1) * 128)
            masktt[:, t, uu[usel]] = valid_p[ts][:, dyu[usel] * 7 + xu[usel]]
            qwt[:, t, :] = (wq4_p[ts] * sqh).astype(bf16)
            qwbt[:, t, :] = wb4_p[ts]
            flat = kvstart_p[ts].T.reshape(-1)  # j = g*128 + q
            kvidx[:, t, :] = np.tile(flat.reshape(-1, 16).T, (8, 1)).astype(np.int16)
            fq = qstart_p[ts]
            qidx[:, t, :] = np.tile(fq.reshape(-1, 16).T, (8, 1)).astype(np.int16)

        percore.append({
            "inp_col": col, "inp_hilo": hilo,
            "w_enc": w_enc, "w_chp": w_chp, "w_ch2": w_ch2,
            "w_qkv0": w_qkv0, "w_qkv1p": w_qkv1p, "w_qkv1k2": w_qkv1k2,
            "qkvb": qkvb, "maskfp": maskfp, "enc_b": enc_bd, "ch_b": ch_bd,
            "m0w": m0w_dev, "m13w": m13w, "m4w": m4w, "bmlp": bm, "b4": b4,
            "ident": ident, "kvidx": kvidx, "qidx": qidx, "maskt": masktt,
            "qwt": qwt, "qwbt": qwbt,
        })
    return percore, NT, owners


# ============================== entry point =================================

def _get_program(NT):
    if NT not in _PROGRAMS:
        _PROGRAMS[NT] = build_program(NT)
    return _PROGRAMS[NT]


def kernel(**inputs):
    from concourse import bass_utils
    in_maps, NT, owners = _host_prep(inputs)
    nc = _get_program(NT)
    res = bass_utils.run_bass_kernel_spmd(nc, in_maps, core_ids=list(range(N_CORES)))
    full = np.empty((B, Q, 3), f32)
    for core in range(N_CORES):
        bi = core // 4
        own = owners[core]
        full[bi, own] = res.results[core]["out"][:len(own)]
    return full


if __name__ == "__main__":
    import time
    t0 = time.time()
    nc = _get_program(9)
    print("built+compiled in", time.time() - t0, "s")


# revision 38
# speedup vs baseline: 1.1924x; 1.1016x over previous
"""Trainium2 Bass kernel for the CLIT-style sparse local attention module.

Strategy (8 NeuronCores, SPMD, no collectives):
  - core c = (batch bi = c // 4, strip qc = c % 4) OWNS the queries whose
    nearest-pixel center row falls in image rows [16qc, 16qc+16). Each core
    computes the 5 convs locally for the 24-row band [16qc-3, 16qc+21)
    (own 16 rows + 3-row halo each side + 2 spare), so every window/bilinear
    read its queries need is produced locally -- the AllGather disappears.
  - qkv conv outputs are PE-transposed to pixel-major entries in local DRAM
    with a 4x row-shifted duplication: entry (r, x) holds rows r..r+3 of
    column x back to back. A 7x7 window then needs only TWO dma_gather
    descriptors per query (4+3 window rows each), and the bilinear q read
    needs ONE (2x2 patch via 2x row duplication).
  - v is stored as fp8e4 (k stays bf16): halves the v gather traffic; the
    wv multiply runs at DVE 1x anyway (broadcast attn operand), so fp8
    costs nothing there. rel-err budget is ~2e-2; bf16 baseline is ~8e-5.
  - Attention per 128-query tile: QK logits as DVE mul + binary-tree adds
    (tensor_reduce is 1x-mode and slow), softmax, attention-weighted v
    (split DVE/GpSimd), PE-transposed into the K-major layout for the
    9410x256 MLP0 (bf16, fp32 PSUM), then MLP1-3 + head per tile group.
  - Host precomputes ownership, gather indices, bilinear weights, masks;
    kernel() scatters per-core outputs back to the original query order.
"""

import sys

sys.path.insert(0, "/opt/trn_rl_repo")

import numpy as np
import ml_dtypes

# ---------------- problem constants (hardcoded per contract) ----------------
B, CH_IN, H, W = 2, 3, 64, 64
Q = 4096
DIM, HEAD, R = 192, 8, 3
RR = 2 * R + 1
RA = RR * RR          # 49
HD = DIM // HEAD      # 24
ENC = 64
HID = 256
N_CORES = 8

NROWS = 24            # local conv band rows (strip 16 + 3 halo + 2 spare)
NPX = NROWS * W       # 1536 pixel entries
SUB_E = 288           # bf16 slots per kv sub-entry: 192 k bf16 + 192 v fp8
KV_ENT = 4 * SUB_E    # 1152: entry holds rows r..r+3
HEADKV = 195          # 192 write-shift room + 3 px guard
KVROWS = HEADKV + NPX # 1731
Q_ENT = 2 * 256       # q entry holds rows r..r+1
HEADQ = 64
QROWS = HEADQ + NPX   # 1600

DYW = RR * DIM                    # 1344 columns per window-row chunk
DYW_P = 1408                      # padded to 11 x 128
DY_BLOCKS = DYW_P // 128          # 11
KBLK = RR * DY_BLOCKS             # 77 K-blocks for MLP layer 0

f32 = np.float32
bf16 = ml_dtypes.bfloat16

_PROGRAMS = {}  # cached compiled Bass programs keyed by NT


def _groups(nt):
    gs = []
    t = 0
    while t < nt:
        gs.append(tuple(range(t, min(t + 2, nt))))
        t += 2
    return gs


# ============================ device program ================================

def build_program(NT):
    import concourse.bass as bass
    import concourse.tile as tile
    from concourse import bacc, mybir

    dt = mybir.dt

    nc = bacc.Bacc("TRN2", target_bir_lowering=False, debug=False,
                   enable_asserts=False, num_devices=N_CORES)

    def din(name, shape, dtype):
        return nc.dram_tensor(name, list(shape), dtype, kind="ExternalInput").ap()

    inp_col = din("inp_col", [27, 6 * 512], dt.bfloat16)
    inp_hilo = din("inp_hilo", [128, 12, 6], dt.bfloat16)
    w_enc = din("w_enc", [27, ENC], dt.bfloat16)
    w_chp = din("w_chp", [128, 3, DIM], dt.bfloat16)
    w_ch2 = din("w_ch2", [ENC, 3, DIM], dt.bfloat16)
    w_qkv0 = din("w_qkv0", [128, 9, 576], dt.bfloat16)
    w_qkv1p = din("w_qkv1p", [128, 3, 576], dt.bfloat16)
    w_qkv1k2 = din("w_qkv1k2", [64, 3, 576], dt.bfloat16)
    qkvb = din("qkvb", [128, 5], dt.float32)
    maskfp = din("maskfp", [128, 26], dt.bfloat16)
    enc_b = din("enc_b", [ENC, 1], dt.float32)
    ch_b = din("ch_b", [128, 2], dt.float32)
    m0w = din("m0w", [128, KBLK, HID], dt.bfloat16)
    m13w = din("m13w", [128, 6, HID], dt.bfloat16)
    m4w = din("m4w", [128, 2, 3], dt.bfloat16)
    bmlp = din("bmlp", [128, 8], dt.float32)
    b4 = din("b4", [128, 3], dt.float32)
    ident = din("ident", [128, 128], dt.bfloat16)
    kvidx = din("kvidx", [128, NT, 16], dt.int16)
    qidx = din("qidx", [128, NT, 8], dt.int16)
    maskt = din("maskt", [128, NT, 56], dt.float32)
    qwt = din("qwt", [128, NT, 4], dt.bfloat16)
    qwbt = din("qwbt", [128, NT, 4], dt.float32)
    out = nc.dram_tensor("out", [NT * 128, 3], dt.float32,
                         kind="ExternalOutput").ap()

    with tile.TileContext(nc) as tc:
        with tc.tile_pool(name="dram", bufs=1, space="DRAM") as dp:
            k4 = dp.tile([KVROWS, 4 * DIM], dt.bfloat16)
            v4 = dp.tile([KVROWS, 4 * DIM], dt.float8e4)
            q2 = dp.tile([QROWS, Q_ENT], dt.bfloat16)

            _convs(nc, tc, mybir, locals())
            _attention(nc, tc, mybir, NT, locals())

    nc.compile()
    return nc


def _convs(nc, tc, mybir, env):
    dt = mybir.dt
    AF = mybir.ActivationFunctionType

    inp_col, w_enc = env["inp_col"], env["w_enc"]
    w_chp, w_ch2 = env["w_chp"], env["w_ch2"]
    w_qkv0, w_qkv1p, w_qkv1k2 = env["w_qkv0"], env["w_qkv1p"], env["w_qkv1k2"]
    qkvb = env["qkvb"]
    enc_b, ch_b, inp_hilo = env["enc_b"], env["ch_b"], env["inp_hilo"]
    k4, v4, q2 = env["k4"], env["v4"], env["q2"]

    with (
        tc.tile_pool(name="cw", bufs=1) as cw,
        tc.tile_pool(name="cfeat", bufs=1) as cf,
        tc.tile_pool(name="cpsum", bufs=2, space="PSUM") as cp,
        tc.tile_pool(name="qpsum", bufs=2, space="PSUM") as cpq,
        tc.tile_pool(name="ctpsum", bufs=2, space="PSUM") as cpt,
        tc.tile_pool(name="cstage", bufs=3) as cs,
    ):
        col_sb = cw.tile([27, 6 * 512], dt.bfloat16)
        nc.sync.dma_start(col_sb[:], inp_col)
        wenc_sb = cw.tile([27, ENC], dt.bfloat16)
        nc.sync.dma_start(wenc_sb[:], w_enc)
        wchp_sb = cw.tile([128, 3, DIM], dt.bfloat16)
        nc.sync.dma_start(wchp_sb[:], w_chp)
        wch2_sb = cw.tile([ENC, 3, DIM], dt.bfloat16)
        nc.sync.dma_start(wch2_sb[:], w_ch2)
        wq0 = cw.tile([128, 9, 576], dt.bfloat16)
        nc.sync.dma_start(wq0[:], w_qkv0)
        wq1p = cw.tile([128, 3, 576], dt.bfloat16)
        nc.sync.dma_start(wq1p[:], w_qkv1p)
        wq1k2 = cw.tile([64, 3, 576], dt.bfloat16)
        nc.sync.dma_start(wq1k2[:], w_qkv1k2)
        qkvb_sb = cw.tile([128, 5], dt.float32)
        nc.sync.dma_start(qkvb_sb[:], qkvb)
        encb_sb = cw.tile([ENC, 1], dt.float32)
        nc.sync.dma_start(encb_sb[:], enc_b)
        chb_sb = cw.tile([128, 2], dt.float32)
        nc.sync.dma_start(chb_sb[:], ch_b)
        hilo_sb = cw.tile([128, 12, 6], dt.bfloat16)
        nc.sync.dma_start(hilo_sb[:], inp_hilo)
        id_c = cw.tile([128, 128], dt.bfloat16)
        nc.sync.dma_start(id_c[:], env["ident"])
        maskfp_sb = cw.tile([128, 26], dt.bfloat16)
        nc.sync.dma_start(maskfp_sb[:], env["maskfp"])

        # zero the k/v head regions (write-shift room + guard entries): rows
        # 0..HEADKV; shifted writes partially overwrite them afterwards.
        zt = cw.tile([128, 1170], dt.bfloat16)
        nc.vector.memset(zt[:], 0.0)
        nc.sync.dma_start(k4[:, :].flatten()[0: HEADKV * 4 * DIM]
                          .rearrange("(p a) -> p a", p=128), zt[:])
        ztv = cw.tile([128, 1170], dt.float8e4)
        nc.vector.memset(ztv[:], 0.0)
        nc.sync.dma_start(v4[:, :].flatten()[0: HEADKV * 4 * DIM]
                          .rearrange("(p a) -> p a", p=128), ztv[:])

        encp = cf.tile([ENC, 66, 66], dt.bfloat16)
        nc.gpsimd.memset(encp[:], 0.0)
        fp0 = cf.tile([128, 66, 66], dt.bfloat16)
        nc.gpsimd.memset(fp0[:], 0.0)
        fp1 = cf.tile([64, 66, 66], dt.bfloat16)
        nc.gpsimd.memset(fp1[:], 0.0)
        encb2 = cf.tile([128, 66, 66], dt.bfloat16)
        fp1b = cf.tile([128, 66, 66], dt.bfloat16)

        # ---- enc conv (strips 2-7) ----
        for t in range(2, 8):
            ps = cp.tile([128, 512], dt.float32, tag="cps")
            nc.tensor.matmul(ps[:ENC, :], wenc_sb[:],
                             col_sb[:, (t - 2) * 512:(t - 1) * 512],
                             start=True, stop=True)
            dst = encp[:, 1 + t * 8: 1 + t * 8 + 8, 1:65]
            nc.scalar.activation(dst, ps[:ENC, :].rearrange("p (a b) -> p a b", a=8),
                                 AF.Identity, bias=encb_sb[:, 0:1])

        # encb2: enc features with a one-column-shifted copy in partitions 64:
        nc.vector.tensor_copy(encb2[0:64, 24:66, :], encp[:, 24:66, :])
        nc.vector.tensor_copy(encb2[64:128, 24:66, 0:65], encp[:, 24:66, 1:66])

        # ---- ch conv (strips 3-7; kx 0/1 paired into K=128, kx=2 single) ----
        for t in range(3, 8):
            for m, msz in ((0, 128), (1, 64)):
                ps = cp.tile([128, 512], dt.float32, tag="cps")
                for ky in range(3):
                    rhs = encb2[:, t * 8 + ky: t * 8 + ky + 8, 0:64]
                    nc.tensor.matmul(ps[:msz, :],
                                     wchp_sb[:, ky, m * 128: m * 128 + msz],
                                     rhs, start=(ky == 0), stop=False)
                for ky in range(3):
                    rhs = encp[:, t * 8 + ky: t * 8 + ky + 8, 2:66]
                    nc.tensor.matmul(ps[:msz, :],
                                     wch2_sb[:, ky, m * 128: m * 128 + msz],
                                     rhs, start=False, stop=(ky == 2))
                dstp = (fp0 if m == 0 else fp1)
                dst = dstp[:msz, 1 + t * 8: 1 + t * 8 + 8, 1:65]
                nc.scalar.activation(dst,
                                     ps[:msz, :].rearrange("p (a b) -> p a b", a=8),
                                     AF.Identity, bias=chb_sb[:msz, m: m + 1])

        # zero ch features of out-of-image rows (qkv convs read fp rows 32..57)
        mbc = maskfp_sb[:].unsqueeze(2).broadcast_to((128, 26, 66))
        nc.vector.tensor_mul(fp0[:, 32:58, :], fp0[:, 32:58, :], mbc)
        nc.vector.tensor_mul(fp1[:, 32:58, :], fp1[:, 32:58, :],
                             maskfp_sb[0:64, :].unsqueeze(2)
                             .broadcast_to((64, 26, 66)))

        # fp1b: channel-chunk-1 features with one-column-shifted copy
        nc.vector.tensor_copy(fp1b[0:64, 32:58, :], fp1[:, 32:58, :])
        nc.vector.tensor_copy(fp1b[64:128, 32:58, 0:65], fp1[:, 32:58, 1:66])

        # ---- q/k/v convs (strips 4-6; M-packed 576 = 4x128+64) + transpose ----
        # 576 cols = [q 0:192 | k 192:384 | v 384:576]
        # per group: list of (dst_kind, src_off, dst_off, n)
        #   kind 0 = qstage bf16, 1 = kvstage k bf16, 2 = kvstage v fp8 view
        gdst = [
            [(0, 0, 0, 128)],
            [(0, 0, 128, 64), (1, 64, 0, 64)],
            [(1, 0, 64, 128)],
            [(2, 0, 0, 128)],
            [(2, 0, 128, 64)],
        ]
        for t in (4, 5, 6):
            kstage = cs.tile([128, 4, DIM], dt.bfloat16, tag="kstage")
            vstage = cs.tile([128, 4, DIM], dt.float8e4, tag="vstage")
            qstage = cs.tile([128, 4, 256], dt.bfloat16, tag="qstage")
            vview = vstage[:, :, :]
            nc.vector.memset(qstage[:, :, 198:], 0.0)
            nc.vector.tensor_copy(qstage[:, :, 192:198],
                                  hilo_sb[:, (t - 4) * 4:(t - 3) * 4, :])
            for g in range(5):
                msz = 128 if g < 4 else 64
                ps = cpq.tile([128, 512], dt.float32, tag="qkvps")
                for off in range(9):
                    ky, kx = off // 3, off % 3
                    rhs0 = fp0[:, t * 8 + ky: t * 8 + ky + 8, kx: kx + 64]
                    nc.tensor.matmul(ps[:msz, :],
                                     wq0[:, off, g * 128: g * 128 + msz],
                                     rhs0, start=(off == 0), stop=False)
                for ky in range(3):
                    rhs1 = fp1b[:, t * 8 + ky: t * 8 + ky + 8, 0:64]
                    nc.tensor.matmul(ps[:msz, :],
                                     wq1p[:, ky, g * 128: g * 128 + msz],
                                     rhs1, start=False, stop=False)
                for ky in range(3):
                    rhs1 = fp1[:, t * 8 + ky: t * 8 + ky + 8, 2:66]
                    nc.tensor.matmul(ps[:msz, :],
                                     wq1k2[:, ky, g * 128: g * 128 + msz],
                                     rhs1, start=False, stop=(ky == 2))
                csb = cs.tile([128, 512], dt.bfloat16, tag="convsb")
                nc.scalar.activation(csb[:msz, :], ps[:msz, :], AF.Identity,
                                     bias=qkvb_sb[:msz, g: g + 1])
                tps = cpt.tile([128, 512], dt.bfloat16, tag="ctps")
                for blk in range(4):
                    nc.tensor.transpose(
                        tps[:, blk * 128: blk * 128 + msz],
                        csb[:msz, blk * 128:(blk + 1) * 128],
                        id_c[:msz, :msz])
                tview = tps[:].rearrange("p (a b) -> p a b", a=4)
                for kind, so, do, n in gdst[g]:
                    if kind == 0:
                        nc.scalar.copy(qstage[:, :, do: do + n],
                                       tview[:, :, so: so + n])
                    elif kind == 1:
                        nc.scalar.copy(kstage[:, :, do: do + n],
                                       tview[:, :, so: so + n])
                    else:
                        nc.scalar.copy(vview[:, :, do: do + n],
                                       tview[:, :, so: so + n])
            # 4x / 2x row-shifted duplicated writes: entry (r, x) sub s holds
            # local row r+s. Strip t covers local rows LT..LT+7.
            LT = (t - 4) * 8
            for s in range(4):
                r0 = HEADKV + (LT - s) * 64
                nc.sync.dma_start(
                    k4[r0: r0 + 512, s * DIM: (s + 1) * DIM]
                    .rearrange("(b p) e -> p b e", p=128), kstage[:])
                nc.sync.dma_start(
                    v4[r0: r0 + 512, s * DIM: (s + 1) * DIM]
                    .rearrange("(b p) e -> p b e", p=128), vstage[:])
            for s in range(2):
                r0 = HEADQ + (LT - s) * 64
                nc.sync.dma_start(
                    q2[r0: r0 + 512, s * 256: (s + 1) * 256]
                    .rearrange("(b p) e -> p b e", p=128), qstage[:])


def _attention(nc, tc, mybir, NT, env):
    import concourse.bass as bass
    dt = mybir.dt
    AX = mybir.AxisListType
    AF = mybir.ActivationFunctionType

    k4, v4, q2 = env["k4"], env["v4"], env["q2"]
    m0w, m13w, m4w = env["m0w"], env["m13w"], env["m4w"]
    bmlp, b4, ident = env["bmlp"], env["b4"], env["ident"]
    kvidx, qidx = env["kvidx"], env["qidx"]
    maskt, qwt, qwbt = env["maskt"], env["qwt"], env["qwbt"]
    out = env["out"]

    with (
        tc.tile_pool(name="aw", bufs=1) as aw,
        tc.tile_pool(name="gath", bufs=2) as gp,
        tc.tile_pool(name="attn", bufs=3) as ap_,
        tc.tile_pool(name="attn3", bufs=2) as ap3,
        tc.tile_pool(name="wvp", bufs=4) as wvp,
        tc.tile_pool(name="xkp", bufs=1) as xkp,
        tc.tile_pool(name="hp", bufs=2) as hp,
        tc.tile_pool(name="outp", bufs=1) as op_,
        tc.tile_pool(name="tpsum", bufs=2, space="PSUM") as tp_,
        tc.tile_pool(name="mpsum", bufs=2, space="PSUM") as mp_,
        tc.tile_pool(name="m0psum", bufs=2, space="PSUM") as mp0,
    ):
        m0w_sb = aw.tile([128, KBLK, HID], dt.bfloat16)
        nc.sync.dma_start(m0w_sb[:], m0w)
        m13_sb = aw.tile([128, 6, HID], dt.bfloat16)
        nc.sync.dma_start(m13_sb[:], m13w)
        m4_sb = aw.tile([128, 2, 3], dt.bfloat16)
        nc.sync.dma_start(m4_sb[:], m4w)
        bm_sb = aw.tile([128, 8], dt.float32)
        nc.sync.dma_start(bm_sb[:], bmlp)
        b4_sb = aw.tile([128, 3], dt.float32)
        nc.sync.dma_start(b4_sb[:], b4)
        id_sb = aw.tile([128, 128], dt.bfloat16)
        nc.sync.dma_start(id_sb[:], ident)
        kvi_sb = aw.tile([128, NT, 16], dt.int16)
        nc.sync.dma_start(kvi_sb[:], kvidx)
        qi_sb = aw.tile([128, NT, 8], dt.int16)
        nc.sync.dma_start(qi_sb[:], qidx)
        mk_sb = aw.tile([128, NT, 56], dt.float32)
        nc.sync.dma_start(mk_sb[:], maskt)
        qw_sb = aw.tile([128, NT, 4], dt.bfloat16)
        nc.sync.dma_start(qw_sb[:], qwt)
        qwb_sb = aw.tile([128, NT, 4], dt.float32)
        nc.sync.dma_start(qwb_sb[:], qwbt)

        out_sb = op_.tile([128, NT, 3], dt.float32)

        KROW = 4 * DIM
        k_ap = k4[:, :]
        k_ap = bass.AP(k_ap.tensor, k_ap.offset,
                       [[KROW, KVROWS - 6], [1, RR * KROW]])
        v_ap = v4[:, :]
        v_ap = bass.AP(v_ap.tensor, v_ap.offset,
                       [[KROW, KVROWS - 6], [1, RR * KROW]])
        q_ap = q2[:, :]
        q_ap = bass.AP(q_ap.tensor, q_ap.offset,
                       [[Q_ENT, QROWS - 2], [1, 2 * Q_ENT]])

        def issue_gathers(t):
            kg = gp.tile([128, 2, RR * KROW], dt.bfloat16, tag="kg", bufs=2)
            nc.gpsimd.dma_gather(kg[:], k_ap, kvi_sb[:, t, :],
                                 num_idxs=256, num_idxs_reg=256,
                                 elem_size=RR * KROW, elem_step=KROW,
                                 single_packet=False)
            vg = gp.tile([128, 2, RR * KROW], dt.float8e4, tag="vg", bufs=3)
            nc.gpsimd.dma_gather(vg[:], v_ap, kvi_sb[:, t, :],
                                 num_idxs=256, num_idxs_reg=256,
                                 elem_size=RR * KROW, elem_step=KROW,
                                 single_packet=False)
            qg = gp.tile([128, 1, 2 * Q_ENT], dt.bfloat16, tag="qg")
            nc.gpsimd.dma_gather(qg[:], q_ap, qi_sb[:, t, :],
                                 num_idxs=128, num_idxs_reg=128,
                                 elem_size=2 * Q_ENT, elem_step=Q_ENT,
                                 single_packet=False)
            return qg, kg, vg

        groups = _groups(NT)
        gof = {}
        for gi, grp in enumerate(groups):
            for ti, t in enumerate(grp):
                gof[t] = (ti, len(grp), grp)

        base_tiles = {}
        logits_st = {}
        pend = {0: issue_gathers(0)}
        mlp_state = [None, None]  # mlp_ps, xchunks of current group

        def stage_a(t):
            qg, kg, vg = pend[t]
            kvv = kg[:].rearrange("p g (x s e) -> p g x s e", x=RR, s=4)
            qg4 = qg[:].rearrange("p o (x y e) -> p o x y e", x=2, y=2)
            if True:
                # ---------------- q vector (bilinear blend) + base ----------
                qprod = ap_.tile([128, 2, 2, DIM], dt.bfloat16, tag="qprod")
                nc.vector.tensor_mul(
                    qprod[:], qg4[:, 0, :, :, 0:DIM],
                    qw_sb[:, t, :].rearrange("p (a b) -> p a b", a=2)
                    .unsqueeze(3).broadcast_to((128, 2, 2, DIM)))
                qt1 = ap_.tile([128, 2, DIM], dt.bfloat16, tag="qt1")
                nc.vector.tensor_add(qt1[:], qprod[:, 0], qprod[:, 1])
                qb = ap_.tile([128, DIM], dt.bfloat16, tag="qb")
                nc.vector.tensor_add(qb[:], qt1[:, 0], qt1[:, 1])

                badd = ap_.tile([128, 2, 2, 3], dt.float32, tag="badd")
                nc.vector.tensor_add(badd[:], qg4[:, 0, :, :, 192:195],
                                     qg4[:, 0, :, :, 195:198])
                bprod = ap_.tile([128, 2, 2, 3], dt.float32, tag="bprod")
                nc.vector.tensor_mul(
                    bprod[:], badd[:],
                    qwb_sb[:, t, :].rearrange("p (a b) -> p a b", a=2)
                    .unsqueeze(3).broadcast_to((128, 2, 2, 3)))
                bt1 = ap_.tile([128, 2, 3], dt.float32, tag="bt1")
                nc.vector.tensor_add(bt1[:], bprod[:, 0], bprod[:, 1])
                base_t = ap_.tile([128, 3], dt.float32, tag="base")
                nc.vector.tensor_add(base_t[:], bt1[:, 0], bt1[:, 1])
                base_tiles[t] = base_t

                # -------- QK logits: 2 bulk muls + per-group binary trees ----
                # logits layout [128, g 2, x 7, s 4, h 8]; (g1, s3) = junk;
                # junk is masked to 0, exp's to 1, and the 7 spurious ones are
                # subtracted from the softmax denominator.
                logits = ap3.tile([128, 2, RR, 4, HEAD], dt.float32, tag="logits")
                for g in range(2):
                    e = ap3.tile([128, RR, 4, DIM], dt.bfloat16, tag="emul",
                                 bufs=1)
                    nc.vector.tensor_mul(
                        e[:], kvv[:, g, :, :, 0:DIM],
                        qb[:].unsqueeze(1).unsqueeze(2)
                        .broadcast_to((128, RR, 4, DIM)))
                    ev = e[:].rearrange("p x s (h d) -> p (x s) h d", h=HEAD)
                    l1 = ap3.tile([128, 28, HEAD, 12], dt.bfloat16, tag="l1",
                                  bufs=1)
                    nc.vector.tensor_add(l1[:], ev[:, :, :, 0:12], ev[:, :, :, 12:24])
                    l2 = ap3.tile([128, 28, HEAD, 6], dt.bfloat16, tag="l2",
                                  bufs=1)
                    nc.vector.tensor_add(l2[:], l1[:, :, :, 0:6], l1[:, :, :, 6:12])
                    l3 = ap3.tile([128, 28, HEAD, 3], dt.bfloat16, tag="l3",
                                  bufs=1)
                    nc.vector.tensor_add(l3[:], l2[:, :, :, 0:3], l2[:, :, :, 3:6])
                    l4 = ap3.tile([128, 28, HEAD, 1], dt.bfloat16, tag="l4",
                                  bufs=1)
                    nc.vector.tensor_add(l4[:], l3[:, :, :, 0:1], l3[:, :, :, 1:2])
                    nc.vector.tensor_add(
                        logits[:, g].rearrange("p x s h -> p (x s) h"),
                        l4[:].rearrange("p u h d -> p u (h d)"),
                        l3[:, :, :, 2:3].rearrange("p u h d -> p u (h d)"))

                lgv = logits[:].rearrange("p g x s h -> p (g x s) h")
                mask_bc = mk_sb[:, t, :].unsqueeze(2).broadcast_to((128, 56, HEAD))
                nc.vector.tensor_mul(lgv, lgv, mask_bc)
                nc.scalar.activation(lgv, lgv, AF.Exp)
                logits_st[t] = logits

        def stage_b(t):
            ti, gw, grp = gof[t]
            logits = logits_st.pop(t)
            lgv = logits[:].rearrange("p g x s h -> p (g x s) h")
            mask_bc = mk_sb[:, t, :].unsqueeze(2).broadcast_to((128, 56, HEAD))
            vg = pend.pop(t)[2]
            vvv = vg[:].rearrange("p g (x s e) -> p g x s e", x=RR, s=4)

            def vslice(dy):
                g, s = (0, dy) if dy < 4 else (1, dy - 4)
                return vvv[:, g, :, s, :]

            if True:
                ssum = ap_.tile([128, HEAD], dt.float32, tag="ssum")
                nc.vector.reduce_sum(
                    ssum[:], logits[:].rearrange("p g x s h -> p h (g x s)"),
                    axis=AX.X)
                nc.vector.tensor_scalar_add(ssum[:], ssum[:], -7.0)
                rec = ap_.tile([128, HEAD], dt.float32, tag="rec")
                nc.vector.reciprocal(rec[:], ssum[:])
                nc.vector.tensor_mul(
                    lgv, lgv, rec[:].unsqueeze(1).broadcast_to((128, 56, HEAD)))
                attnb = ap3.tile([128, 2, RR, 4, HEAD], dt.bfloat16, tag="attnb")
                nc.vector.tensor_mul(
                    attnb[:].rearrange("p g x s h -> p (g x s) h"), lgv, mask_bc)

                # ---- weighted v -> per-dy chunks; MLP0 accumulates per chunk
                if ti == 0:
                    mlp_state[0] = mp0.tile([128, 512], dt.float32, tag="mlp0ps",
                                            name="mlp0ps")
                    mlp_state[1] = [xkp.tile([128, DY_BLOCKS, 256], dt.bfloat16,
                                             tag=f"xc{dy}", name=f"xc{dy}")
                                    for dy in range(RR)]
                mlp_ps, xchunks = mlp_state

                def mlp0_chunk(dy):
                    chunk = xchunks[dy]
                    for m in range(2):
                        for b_ in range(DY_BLOCKS):
                            kb = dy * DY_BLOCKS + b_
                            nc.tensor.matmul(
                                mlp_ps[:, m * 256: m * 256 + gw * 128],
                                m0w_sb[:, kb, m * 128:(m + 1) * 128],
                                chunk[:, b_, 0: gw * 128], start=(kb == 0),
                                stop=(kb == KBLK - 1), skip_group_check=True)

                xcol = ti * 128
                for dy in range(RR):
                    wv = wvp.tile([128, DYW_P], dt.bfloat16, tag="wv")
                    nc.gpsimd.memset(wv[:, DYW:], 0.0)
                    weng = nc.gpsimd if dy >= 5 else nc.vector
                    # v channels are stored d-major (c' = d*8 + h) so that all
                    # three operands are innermost-contiguous -> DVE 2x mode
                    g_, s_ = (0, dy) if dy < 4 else (1, dy - 4)
                    weng.tensor_mul(
                        wv[:, 0:DYW].rearrange("p (r d h) -> p r d h", d=HD, h=HEAD),
                        vslice(dy).rearrange("p r (d h) -> p r d h", h=HEAD),
                        attnb[:, g_, :, s_, :].unsqueeze(2).broadcast_to(
                            (128, RR, HD, HEAD)))
                    for grp2, nb in ((0, 8), (1, 3)):
                        tps = tp_.tile([128, nb * 128], dt.bfloat16,
                                       tag=f"tps{grp2}")
                        for bi_ in range(nb):
                            blk = grp2 * 8 + bi_
                            nc.tensor.transpose(tps[:, bi_ * 128:(bi_ + 1) * 128],
                                                wv[:, blk * 128:(blk + 1) * 128],
                                                id_sb[:])
                        nc.scalar.copy(
                            xchunks[dy][:, grp2 * 8: grp2 * 8 + nb, xcol: xcol + 128],
                            tps[:].rearrange("p (a b) -> p a b", a=nb))
                    # emit chunk dy-1's MLP0 matmuls one dy late so the PE is
                    # not stalled on the PSUM->SBUF copy of the current chunk
                    if ti == gw - 1 and dy >= 1:
                        mlp0_chunk(dy - 1)

                # ---------------- MLP layers 0-3 + head (per tile group) -----
                if ti == gw - 1:
                    mlp0_chunk(RR - 1)
                    h0 = hp.tile([128, 2, 256], dt.bfloat16, tag="h")
                    for m in range(2):
                        nc.scalar.activation(h0[:, m, 0: gw * 128],
                                             mlp_ps[:, m * 256: m * 256 + gw * 128],
                                             AF.Relu, bias=bm_sb[:, m:m + 1])
                    cur = h0
                    for l in (1, 2, 3):
                        nxt = hp.tile([128, 2, 256], dt.bfloat16, tag="h")
                        for m in range(2):
                            ps = mp_.tile([128, 256], dt.float32, tag="mlpps")
                            for kc in range(2):
                                nc.tensor.matmul(
                                    ps[:, 0: gw * 128],
                                    m13_sb[:, (l - 1) * 2 + kc,
                                           m * 128:(m + 1) * 128],
                                    cur[:, kc, 0: gw * 128],
                                    start=(kc == 0), stop=(kc == 1))
                            nc.scalar.activation(nxt[:, m, 0: gw * 128],
                                                 ps[:, 0: gw * 128], AF.Relu,
                                                 bias=bm_sb[:, 2 * l + m: 2 * l + m + 1])
                        cur = nxt
                    for tt in range(gw):
                        psof = mp_.tile([128, 256], dt.float32, tag="mlpps")
                        pso = psof[:, 0:3]
                        for kc in range(2):
                            nc.tensor.matmul(pso,
                                             cur[:, kc, tt * 128: tt * 128 + 128],
                                             m4_sb[:, kc, :],
                                             start=(kc == 0), stop=(kc == 1))
                        o1 = ap_.tile([128, 3], dt.float32, tag="o1")
                        nc.vector.tensor_add(o1[:], pso, b4_sb[:])
                        nc.vector.tensor_add(out_sb[:, grp[tt], :], o1[:],
                                             base_tiles.pop(grp[tt])[:])

        # software pipeline: [gathers(t+1); stage A(t); stage B(t-1)] so the
        # DVE never blocks on the ACT exp or on gather completion
        for it in range(NT + 1):
            if it < NT:
                if it + 1 < NT:
                    pend[it + 1] = issue_gathers(it + 1)
                stage_a(it)
            if it >= 1:
                stage_b(it - 1)

        nc.sync.dma_start(
            out.rearrange("(t p) c -> p t c", p=128), out_sb[:])


# ============================ host preparation ==============================

def _host_prep(inputs):
    inp = np.asarray(inputs["inp"], f32)
    sc = np.asarray(inputs["sample_coord"], f32)
    cell = np.asarray(inputs["cell"], f32)

    enc_w = np.asarray(inputs["enc_w"], f32)
    ch_w = np.asarray(inputs["ch_w"], f32)

    w_enc = enc_w.transpose(1, 2, 3, 0).reshape(27, ENC).astype(bf16)
    w_chp = np.zeros((128, 3, DIM), bf16)
    w_ch2 = np.zeros((ENC, 3, DIM), bf16)
    for ky in range(3):
        w_chp[0:64, ky, :] = ch_w[:, :, ky, 0].T.astype(bf16)
        w_chp[64:128, ky, :] = ch_w[:, :, ky, 1].T.astype(bf16)
        w_ch2[:, ky, :] = ch_w[:, :, ky, 2].T.astype(bf16)

    # qkv weights M-packed: 576 output cols = [q 192 | k 192 | v 192]
    w_qkv0 = np.zeros((128, 9, 576), bf16)
    w_qkv1p = np.zeros((128, 3, 576), bf16)
    w_qkv1k2 = np.zeros((64, 3, 576), bf16)
    qkvb = np.zeros((128, 5), f32)
    # v output channels permuted d-major: device channel c' = d*8+h holds
    # reference channel h*24+d (lets the wv multiply run innermost-contiguous)
    cmap_v = (np.arange(192) % 8) * 24 + np.arange(192) // 8
    for ci, nm in enumerate(("q", "k", "v")):
        wt = np.asarray(inputs[f"{nm}_w"], f32)
        bt = np.asarray(inputs[f"{nm}_b"], f32)
        if nm == "v":
            wt = wt[cmap_v]
            bt = bt[cmap_v]
        cs_ = ci * 192
        for off in range(9):
            ky, kx = off // 3, off % 3
            w_qkv0[:, off, cs_: cs_ + 192] = wt[:, 0:128, ky, kx].T.astype(bf16)
        for ky in range(3):
            w_qkv1p[0:64, ky, cs_: cs_ + 192] = wt[:, 128:192, ky, 0].T.astype(bf16)
            w_qkv1p[64:128, ky, cs_: cs_ + 192] = wt[:, 128:192, ky, 1].T.astype(bf16)
            w_qkv1k2[:, ky, cs_: cs_ + 192] = wt[:, 128:192, ky, 2].T.astype(bf16)
        bfull = bt
        for g in range(5):
            msz = 128 if g < 4 else 64
            seg = np.arange(g * 128, g * 128 + msz)
            sel = (seg >= cs_) & (seg < cs_ + 192)
            if sel.any():
                qkvb[np.nonzero(sel)[0], g] = bfull[seg[sel] - cs_]

    # m0w rows permuted into 7 chunks of 1344 -> 1408 (zero padded); within a
    # chunk the device feature order is (dx, d, h): j = dx*192 + d*8 + h maps
    # to reference row (dy*7+dx)*192 + h*24 + d
    m0w_full = np.asarray(inputs["m0w"], f32)
    jj = np.arange(DYW)
    jdx, jc = jj // 192, jj % 192
    jd, jh = jc // 8, jc % 8
    ref_j = jdx * 192 + jh * 24 + jd
    perm = np.zeros((KBLK * 128, HID), f32)
    for i in range(RR):
        perm[i * DYW_P: i * DYW_P + DYW] = m0w_full[i * DYW + ref_j]
    m0w_dev = np.ascontiguousarray(
        perm.reshape(KBLK, 128, HID).transpose(1, 0, 2)).astype(bf16)

    m13w = np.zeros((128, 6, HID), bf16)
    for l in (1, 2, 3):
        wl = np.asarray(inputs[f"m{l}w"], f32)
        m13w[:, (l - 1) * 2 + 0, :] = wl[0:128].astype(bf16)
        m13w[:, (l - 1) * 2 + 1, :] = wl[128:256].astype(bf16)
    m4w_full = np.asarray(inputs["m4w"], f32)
    m4w = np.stack([m4w_full[0:128], m4w_full[128:256]], 1).astype(bf16)

    b4 = np.broadcast_to(np.asarray(inputs["m4b"], f32)[None, :], (128, 3)).copy()
    enc_bd = np.asarray(inputs["enc_b"], f32).reshape(ENC, 1)
    ch_bd = np.zeros((128, 2), f32)
    ch_bd[:, 0] = np.asarray(inputs["ch_b"], f32)[0:128]
    ch_bd[0:64, 1] = np.asarray(inputs["ch_b"], f32)[128:192]
    ident = np.eye(128, dtype=bf16)

    m0b = np.asarray(inputs["m0b"], f32)
    m0w_tail = m0w_full[RA * DIM: RA * DIM + 2]
    bias_rest = np.zeros((128, 8), f32)
    for l in (1, 2, 3):
        bl = np.asarray(inputs[f"m{l}b"], f32)
        bias_rest[:, 2 * l + 0] = bl[0:128]
        bias_rest[:, 2 * l + 1] = bl[128:256]

    # ---- ownership: core (bi, qc) owns queries with iy//16 == qc ----
    sqh = f32(1.0 / np.sqrt(HD))
    cy_all, cx_all = sc[..., 0], sc[..., 1]
    py_all = (cy_all + f32(1.0)) * f32(H * 0.5) - f32(0.5)
    px_all = (cx_all + f32(1.0)) * f32(W * 0.5) - f32(0.5)
    iy_all = np.clip(np.floor(py_all + f32(0.5)), 0, H - 1).astype(np.int64)
    ix_all = np.clip(np.floor(px_all + f32(0.5)), 0, W - 1).astype(np.int64)

    owners = []
    for core in range(N_CORES):
        bi, qc = core // 4, core % 4
        own = np.nonzero(iy_all[bi] // 16 == qc)[0]
        owners.append(own)
    NT = max((len(o) + 127) // 128 for o in owners)
    SLOTS = NT * 128

    batch_data = []
    for bi in range(B):
        x = inp[bi]
        xT = x.reshape(3, H * W).T
        hi = xT.astype(bf16).astype(f32)
        lo = (xT - hi).astype(bf16)
        hilo_full = np.concatenate([hi.astype(bf16), lo], 1)  # [4096, 6]

        rel_cell = cell[bi] * np.array([H, W], f32)
        b0 = m0b + rel_cell @ m0w_tail
        bm = bias_rest.copy()
        bm[:, 0] = b0[0:128]
        bm[:, 1] = b0[128:256]
        batch_data.append((x, hilo_full, bm))

    d = np.arange(-R, R + 1)
    percore = []
    for core in range(N_CORES):
        bi, qc = core // 4, core % 4
        R0 = 16 * qc
        own = owners[core]
        n = len(own)
        x, hilo_full, bm = batch_data[bi]

        # padded image: padded row p <-> image row R0 + p - 35
        xbig = np.zeros((CH_IN, 128, W + 2), f32)
        plo, phi = max(0, 35 - R0), min(128, 35 - R0 + H)
        xbig[:, plo:phi, 1:1 + W] = x[:, plo - 35 + R0: phi - 35 + R0, :]

        # im2col for enc strips 2..7 (rows 16..63)
        col = np.zeros((27, 6 * 512), bf16)
        for c in range(CH_IN):
            for ky in range(3):
                for kx in range(3):
                    col[c * 9 + ky * 3 + kx] = \
                        xbig[c, 15 + ky: 63 + ky, kx: kx + W] \
                        .reshape(-1).astype(bf16)
        tr = np.arange(16, 64) - 35 + R0  # image rows of enc output strips
        col.reshape(27, 48, W)[:, (tr < 0) | (tr >= H), :] = 0

        # ch rows 31..56 mask (image rows R0-4 .. R0+21)
        trf = np.arange(26) + R0 - 4
        maskfp = np.broadcast_to(
            ((trf >= 0) & (trf < H)).astype(bf16)[None, :], (128, 26)).copy()

        # hilo for local pixel rows 0..23 (image rows R0-3 .. R0+20)
        hl = np.zeros((NPX, 6), bf16)
        rlo, rhi = max(0, R0 - 3), min(H, R0 + 21)
        hl[(rlo - R0 + 3) * W: (rhi - R0 + 3) * W] = \
            hilo_full[rlo * W: rhi * W]
        hilo = np.ascontiguousarray(hl.reshape(12, 128, 6).transpose(1, 0, 2))

        # ---- per-query gather indices / weights ----
        iy, ix = iy_all[bi][own], ix_all[bi][own]
        py, px = py_all[bi][own], px_all[bi][own]

        dy_, dx_ = [a.reshape(-1) for a in np.meshgrid(d, d, indexing="ij")]
        yy = iy[:, None] + dy_[None, :]
        xx = ix[:, None] + dx_[None, :]
        valid = ((yy >= 0) & (yy < H) & (xx >= 0) & (xx < W)).astype(f32)

        # kv entry index for group g: entry (iy - R0 + 4g)*64 + ix - 3,
        # tensor row = HEADKV + entry = 192 + (iy-R0+4g)*64 + ix
        kvstart = np.zeros((n, 2), np.int64)
        for g in range(2):
            kvstart[:, g] = 192 + (iy - R0 + 4 * g) * 64 + ix

        y0 = np.floor(py)
        x0 = np.floor(px)
        wy, wx = py - y0, px - x0
        y0 = y0.astype(np.int64)
        x0 = x0.astype(np.int64)
        sy0 = np.clip(y0, 0, H - 2)
        sx0 = np.clip(x0, 0, W - 2)
        wq_eff = np.zeros((n, 2, 2), f32)
        wb_eff = np.zeros((n, 2, 2), f32)
        qq = np.arange(n)
        for ddy, syw in ((0, 1 - wy), (1, wy)):
            for ddx, sxw in ((0, 1 - wx), (1, wx)):
                w = (syw * sxw).astype(f32)
                yc, xc = y0 + ddy, x0 + ddx
                ly = np.clip(yc, 0, H - 1) - sy0
                lx = np.clip(xc, 0, W - 1) - sx0
                wb_eff[qq, ly, lx] += w
                vm = ((yc >= 0) & (yc < H) & (xc >= 0) & (xc < W))
                wq_eff[qq, ly, lx] += w * vm
        # qg4 layout is [x(sx), y(sy)] -> reorder weights to (lx, ly)
        wq4 = wq_eff.transpose(0, 2, 1).reshape(n, 4)
        wb4 = wb_eff.transpose(0, 2, 1).reshape(n, 4)
        qstart = HEADQ + (sy0 - R0 + 3) * 64 + sx0

        # pad to SLOTS
        def padto(a, fill):
            outp = np.full((SLOTS,) + a.shape[1:], fill, a.dtype)
            outp[:n] = a
            return outp

        kvstart_p = padto(kvstart, 192)
        qstart_p = padto(qstart, HEADQ + 128)
        valid_p = padto(valid, 0.0)
        wq4_p = padto(wq4.astype(f32), 0.0)
        wb4_p = padto(wb4, 0.0)

        kvidx = np.zeros((128, NT, 16), np.int16)
        qidx = np.zeros((128, NT, 8), np.int16)
        masktt = np.zeros((128, NT, 56), f32)
        qwt = np.zeros((128, NT, 4), bf16)
        qwbt = np.zeros((128, NT, 4), f32)
        # mask in device (g, x, s) order: u = g*28 + x*4 + s, dy = g*4+s, dx = x
        uu = np.arange(56)
        gu, xu, su = uu // 28, (uu % 28) // 4, uu % 4
        dyu = gu * 4 + su
        usel = dyu < 7
        for t in range(NT):
            ts = slice(t * 128, (t + 1) * 128)
            masktt[:, t, uu[usel]] = valid_p[ts][:, dyu[usel] * 7 + xu[usel]]
            qwt[:, t, :] = (wq4_p[ts] * sqh).astype(bf16)
            qwbt[:, t, :] = wb4_p[ts]
            flat = kvstart_p[ts].T.reshape(-1)  # j = g*128 + q
            kvidx[:, t, :] = np.tile(flat.reshape(-1, 16).T, (8, 1)).astype(np.int16)
            fq = qstart_p[ts]
            qidx[:, t, :] = np.tile(fq.reshape(-1, 16).T, (8, 1)).astype(np.int16)

        percore.append({
            "inp_col": col, "inp_hilo": hilo,
            "w_enc": w_enc, "w_chp": w_chp, "w_ch2": w_ch2,
            "w_qkv0": w_qkv0, "w_qkv1p": w_qkv1p, "w_qkv1k2": w_qkv1k2,
            "qkvb": qkvb, "maskfp": maskfp, "enc_b": enc_bd, "ch_b": ch_bd,
            "m0w": m0w_dev, "m13w": m13w, "m4w": m4w, "bmlp": bm, "b4": b4,
            "ident": ident, "kvidx": kvidx, "qidx": qidx, "maskt": masktt,
            "qwt": qwt, "qwbt": qwbt,
        })
    return percore, NT, owners


# ============================== entry point =================================

def _get_program(NT):
    if NT not in _PROGRAMS:
        _PROGRAMS[NT] = build_program(NT)
    return _PROGRAMS[NT]


def kernel(**inputs):
    from concourse import bass_utils
    in_maps, NT, owners = _host_prep(inputs)
    nc = _get_program(NT)
    res = bass_utils.run_bass_kernel_spmd(nc, in_maps, core_ids=list(range(N_CORES)))
    full = np.empty((B, Q, 3), f32)
    for core in range(N_CORES):
        bi = core // 4
        own = owners[core]
        full[bi, own] = res.results[core]["out"][:len(own)]
    return full


if __name__ == "__main__":
    import time
    t0 = time.time()
    nc = _get_program(9)
    print("built+compiled in", time.time() - t0, "s")


# revision 40
# speedup vs baseline: 1.2076x; 1.0128x over previous
"""Trainium2 Bass kernel for the CLIT-style sparse local attention module.

Strategy (8 NeuronCores, SPMD, no collectives):
  - core c = (batch bi = c // 4, strip qc = c % 4) OWNS the queries whose
    nearest-pixel center row falls in image rows [16qc, 16qc+16). Each core
    computes the 5 convs locally for the 24-row band [16qc-3, 16qc+21)
    (own 16 rows + 3-row halo each side + 2 spare), so every window/bilinear
    read its queries need is produced locally -- the AllGather disappears.
  - qkv conv outputs are PE-transposed to pixel-major entries in local DRAM
    with a 4x row-shifted duplication: entry (r, x) holds rows r..r+3 of
    column x back to back. A 7x7 window then needs only TWO dma_gather
    descriptors per query (4+3 window rows each), and the bilinear q read
    needs ONE (2x2 patch via 2x row duplication).
  - v is stored as fp8e4 (k stays bf16): halves the v gather traffic; the
    wv multiply runs at DVE 1x anyway (broadcast attn operand), so fp8
    costs nothing there. rel-err budget is ~2e-2; bf16 baseline is ~8e-5.
  - Attention per 128-query tile: QK logits as DVE mul + binary-tree adds
    (tensor_reduce is 1x-mode and slow), softmax, attention-weighted v
    (split DVE/GpSimd), PE-transposed into the K-major layout for the
    9410x256 MLP0 (bf16, fp32 PSUM), then MLP1-3 + head per tile group.
  - Host precomputes ownership, gather indices, bilinear weights, masks;
    kernel() scatters per-core outputs back to the original query order.
"""

import sys

sys.path.insert(0, "/opt/trn_rl_repo")

import numpy as np
import ml_dtypes

# ---------------- problem constants (hardcoded per contract) ----------------
B, CH_IN, H, W = 2, 3, 64, 64
Q = 4096
DIM, HEAD, R = 192, 8, 3
RR = 2 * R + 1
RA = RR * RR          # 49
HD = DIM // HEAD      # 24
ENC = 64
HID = 256
N_CORES = 8

NROWS = 24            # local conv band rows (strip 16 + 3 halo + 2 spare)
NPX = NROWS * W       # 1536 pixel entries
SUB_E = 288           # bf16 slots per kv sub-entry: 192 k bf16 + 192 v fp8
KV_ENT = 4 * SUB_E    # 1152: entry holds rows r..r+3
HEADKV = 195          # 192 write-shift room + 3 px guard
KVROWS = HEADKV + NPX # 1731
Q_ENT = 2 * 256       # q entry holds rows r..r+1
HEADQ = 64
QROWS = HEADQ + NPX   # 1600

DYW = RR * DIM                    # 1344 columns per window-row chunk
DYW_P = 1408                      # padded to 11 x 128
DY_BLOCKS = DYW_P // 128          # 11
KBLK = RR * DY_BLOCKS             # 77 K-blocks for MLP layer 0

f32 = np.float32
bf16 = ml_dtypes.bfloat16

_PROGRAMS = {}  # cached compiled Bass programs keyed by NT


def _groups(nt):
    gs = []
    t = 0
    while t < nt:
        gs.append(tuple(range(t, min(t + 2, nt))))
        t += 2
    return gs


# ============================ device program ================================

def build_program(NT):
    import concourse.bass as bass
    import concourse.tile as tile
    from concourse import bacc, mybir

    dt = mybir.dt

    nc = bacc.Bacc("TRN2", target_bir_lowering=False, debug=False,
                   enable_asserts=False, num_devices=N_CORES)

    def din(name, shape, dtype):
        return nc.dram_tensor(name, list(shape), dtype, kind="ExternalInput").ap()

    inp_col = din("inp_col", [27, 6 * 512], dt.bfloat16)
    inp_hilo = din("inp_hilo", [128, 12, 6], dt.bfloat16)
    w_enc = din("w_enc", [27, ENC], dt.bfloat16)
    w_chp = din("w_chp", [128, 3, DIM], dt.bfloat16)
    w_ch2 = din("w_ch2", [ENC, 3, DIM], dt.bfloat16)
    w_qkv0 = din("w_qkv0", [128, 9, 576], dt.bfloat16)
    w_qkv1p = din("w_qkv1p", [128, 3, 576], dt.bfloat16)
    w_qkv1k2 = din("w_qkv1k2", [64, 3, 576], dt.bfloat16)
    qkvb = din("qkvb", [128, 5], dt.float32)
    maskfp = din("maskfp", [128, 26], dt.bfloat16)
    enc_b = din("enc_b", [ENC, 1], dt.float32)
    ch_b = din("ch_b", [128, 2], dt.float32)
    m0w = din("m0w", [128, KBLK, HID], dt.bfloat16)
    m13w = din("m13w", [128, 6, HID], dt.bfloat16)
    m4w = din("m4w", [128, 2, 3], dt.bfloat16)
    bmlp = din("bmlp", [128, 8], dt.float32)
    b4 = din("b4", [128, 3], dt.float32)
    ident = din("ident", [128, 128], dt.bfloat16)
    kvidx = din("kvidx", [128, NT, 16], dt.int16)
    qidx = din("qidx", [128, NT, 8], dt.int16)
    maskt = din("maskt", [128, NT, 56], dt.float32)
    qwt = din("qwt", [128, NT, 4], dt.bfloat16)
    qwbt = din("qwbt", [128, NT, 4], dt.float32)
    out = nc.dram_tensor("out", [NT * 128, 3], dt.float32,
                         kind="ExternalOutput").ap()

    with tile.TileContext(nc) as tc:
        with tc.tile_pool(name="dram", bufs=1, space="DRAM") as dp:
            k4 = dp.tile([KVROWS, 4 * DIM], dt.bfloat16)
            v4 = dp.tile([KVROWS, 4 * DIM], dt.float8e4)
            q2 = dp.tile([QROWS, Q_ENT], dt.bfloat16)

            _convs(nc, tc, mybir, locals())
            _attention(nc, tc, mybir, NT, locals())

    nc.compile()
    return nc


def _convs(nc, tc, mybir, env):
    dt = mybir.dt
    AF = mybir.ActivationFunctionType

    inp_col, w_enc = env["inp_col"], env["w_enc"]
    w_chp, w_ch2 = env["w_chp"], env["w_ch2"]
    w_qkv0, w_qkv1p, w_qkv1k2 = env["w_qkv0"], env["w_qkv1p"], env["w_qkv1k2"]
    qkvb = env["qkvb"]
    enc_b, ch_b, inp_hilo = env["enc_b"], env["ch_b"], env["inp_hilo"]
    k4, v4, q2 = env["k4"], env["v4"], env["q2"]

    with (
        tc.tile_pool(name="cw", bufs=1) as cw,
        tc.tile_pool(name="cfeat", bufs=1) as cf,
        tc.tile_pool(name="cpsum", bufs=2, space="PSUM") as cp,
        tc.tile_pool(name="qpsum", bufs=2, space="PSUM") as cpq,
        tc.tile_pool(name="ctpsum", bufs=2, space="PSUM") as cpt,
        tc.tile_pool(name="cstage", bufs=3) as cs,
    ):
        col_sb = cw.tile([27, 6 * 512], dt.bfloat16)
        nc.sync.dma_start(col_sb[:], inp_col)
        wenc_sb = cw.tile([27, ENC], dt.bfloat16)
        nc.sync.dma_start(wenc_sb[:], w_enc)
        wchp_sb = cw.tile([128, 3, DIM], dt.bfloat16)
        nc.sync.dma_start(wchp_sb[:], w_chp)
        wch2_sb = cw.tile([ENC, 3, DIM], dt.bfloat16)
        nc.sync.dma_start(wch2_sb[:], w_ch2)
        wq0 = cw.tile([128, 9, 576], dt.bfloat16)
        nc.sync.dma_start(wq0[:], w_qkv0)
        wq1p = cw.tile([128, 3, 576], dt.bfloat16)
        nc.sync.dma_start(wq1p[:], w_qkv1p)
        wq1k2 = cw.tile([64, 3, 576], dt.bfloat16)
        nc.sync.dma_start(wq1k2[:], w_qkv1k2)
        qkvb_sb = cw.tile([128, 5], dt.float32)
        nc.sync.dma_start(qkvb_sb[:], qkvb)
        encb_sb = cw.tile([ENC, 1], dt.float32)
        nc.sync.dma_start(encb_sb[:], enc_b)
        chb_sb = cw.tile([128, 2], dt.float32)
        nc.sync.dma_start(chb_sb[:], ch_b)
        hilo_sb = cw.tile([128, 12, 6], dt.bfloat16)
        nc.sync.dma_start(hilo_sb[:], inp_hilo)
        id_c = cw.tile([128, 128], dt.bfloat16)
        nc.sync.dma_start(id_c[:], env["ident"])
        maskfp_sb = cw.tile([128, 26], dt.bfloat16)
        nc.sync.dma_start(maskfp_sb[:], env["maskfp"])

        # zero the k/v head regions (write-shift room + guard entries): rows
        # 0..HEADKV; shifted writes partially overwrite them afterwards.
        zt = cw.tile([128, 1170], dt.bfloat16)
        nc.vector.memset(zt[:], 0.0)
        nc.sync.dma_start(k4[:, :].flatten()[0: HEADKV * 4 * DIM]
                          .rearrange("(p a) -> p a", p=128), zt[:])
        ztv = cw.tile([128, 1170], dt.float8e4)
        nc.vector.memset(ztv[:], 0.0)
        nc.sync.dma_start(v4[:, :].flatten()[0: HEADKV * 4 * DIM]
                          .rearrange("(p a) -> p a", p=128), ztv[:])

        encp = cf.tile([ENC, 66, 66], dt.bfloat16)
        nc.gpsimd.memset(encp[:], 0.0)
        fp0 = cf.tile([128, 66, 66], dt.bfloat16)
        nc.gpsimd.memset(fp0[:], 0.0)
        fp1 = cf.tile([64, 66, 66], dt.bfloat16)
        nc.gpsimd.memset(fp1[:], 0.0)
        encb2 = cf.tile([128, 66, 66], dt.bfloat16)
        fp1b = cf.tile([128, 66, 66], dt.bfloat16)

        # ---- enc conv (strips 3-7) ----
        for t in range(3, 8):
            ps = cp.tile([128, 512], dt.float32, tag="cps")
            nc.tensor.matmul(ps[:ENC, :], wenc_sb[:],
                             col_sb[:, (t - 2) * 512:(t - 1) * 512],
                             start=True, stop=True)
            dst = encp[:, 1 + t * 8: 1 + t * 8 + 8, 1:65]
            nc.scalar.activation(dst, ps[:ENC, :].rearrange("p (a b) -> p a b", a=8),
                                 AF.Identity, bias=encb_sb[:, 0:1])

        # encb2: enc features with a one-column-shifted copy in partitions 64:
        nc.vector.tensor_copy(encb2[0:64, 24:66, :], encp[:, 24:66, :])
        nc.vector.tensor_copy(encb2[64:128, 24:66, 0:65], encp[:, 24:66, 1:66])

        # ---- ch conv (full strips 4-6; strips 3/7 contribute one row each) ----
        for t in (4, 5, 6):
            for m, msz in ((0, 128), (1, 64)):
                ps = cp.tile([128, 512], dt.float32, tag="cps")
                for ky in range(3):
                    rhs = encb2[:, t * 8 + ky: t * 8 + ky + 8, 0:64]
                    nc.tensor.matmul(ps[:msz, :],
                                     wchp_sb[:, ky, m * 128: m * 128 + msz],
                                     rhs, start=(ky == 0), stop=False)
                for ky in range(3):
                    rhs = encp[:, t * 8 + ky: t * 8 + ky + 8, 2:66]
                    nc.tensor.matmul(ps[:msz, :],
                                     wch2_sb[:, ky, m * 128: m * 128 + msz],
                                     rhs, start=False, stop=(ky == 2))
                dstp = (fp0 if m == 0 else fp1)
                dst = dstp[:msz, 1 + t * 8: 1 + t * 8 + 8, 1:65]
                nc.scalar.activation(dst,
                                     ps[:msz, :].rearrange("p (a b) -> p a b", a=8),
                                     AF.Identity, bias=chb_sb[:msz, m: m + 1])
        for pidx in (32, 57):  # single needed fp rows from strips 3 and 7
            for m, msz in ((0, 128), (1, 64)):
                ps = cp.tile([128, 512], dt.float32, tag="cps")
                for ky in range(3):
                    rhs = encb2[:, pidx - 1 + ky: pidx + ky, 0:64]
                    nc.tensor.matmul(ps[:msz, 0:64],
                                     wchp_sb[:, ky, m * 128: m * 128 + msz],
                                     rhs, start=(ky == 0), stop=False)
                for ky in range(3):
                    rhs = encp[:, pidx - 1 + ky: pidx + ky, 2:66]
                    nc.tensor.matmul(ps[:msz, 0:64],
                                     wch2_sb[:, ky, m * 128: m * 128 + msz],
                                     rhs, start=False, stop=(ky == 2))
                dstp = (fp0 if m == 0 else fp1)
                dst = dstp[:msz, pidx: pidx + 1, 1:65]
                nc.scalar.activation(dst,
                                     ps[:msz, 0:64].rearrange("p (a b) -> p a b", a=1),
                                     AF.Identity, bias=chb_sb[:msz, m: m + 1])

        # zero ch features of out-of-image rows (qkv convs read fp rows 32..57)
        mbc = maskfp_sb[:].unsqueeze(2).broadcast_to((128, 26, 66))
        nc.vector.tensor_mul(fp0[:, 32:58, :], fp0[:, 32:58, :], mbc)
        nc.vector.tensor_mul(fp1[:, 32:58, :], fp1[:, 32:58, :],
                             maskfp_sb[0:64, :].unsqueeze(2)
                             .broadcast_to((64, 26, 66)))

        # fp1b: channel-chunk-1 features with one-column-shifted copy
        nc.vector.tensor_copy(fp1b[0:64, 32:58, :], fp1[:, 32:58, :])
        nc.vector.tensor_copy(fp1b[64:128, 32:58, 0:65], fp1[:, 32:58, 1:66])

        # ---- q/k/v convs (strips 4-6; M-packed 576 = 4x128+64) + transpose ----
        # 576 cols = [q 0:192 | k 192:384 | v 384:576]
        # per group: list of (dst_kind, src_off, dst_off, n)
        #   kind 0 = qstage bf16, 1 = kvstage k bf16, 2 = kvstage v fp8 view
        gdst = [
            [(0, 0, 0, 128)],
            [(0, 0, 128, 64), (1, 64, 0, 64)],
            [(1, 0, 64, 128)],
            [(2, 0, 0, 128)],
            [(2, 0, 128, 64)],
        ]
        for t in (4, 5, 6):
            kstage = cs.tile([128, 4, DIM], dt.bfloat16, tag="kstage")
            vstage = cs.tile([128, 4, DIM], dt.float8e4, tag="vstage")
            qstage = cs.tile([128, 4, 256], dt.bfloat16, tag="qstage")
            vview = vstage[:, :, :]
            nc.vector.memset(qstage[:, :, 198:], 0.0)
            nc.vector.tensor_copy(qstage[:, :, 192:198],
                                  hilo_sb[:, (t - 4) * 4:(t - 3) * 4, :])
            for g in range(5):
                msz = 128 if g < 4 else 64
                ps = cpq.tile([128, 512], dt.float32, tag="qkvps")
                for off in range(9):
                    ky, kx = off // 3, off % 3
                    rhs0 = fp0[:, t * 8 + ky: t * 8 + ky + 8, kx: kx + 64]
                    nc.tensor.matmul(ps[:msz, :],
                                     wq0[:, off, g * 128: g * 128 + msz],
                                     rhs0, start=(off == 0), stop=False)
                for ky in range(3):
                    rhs1 = fp1b[:, t * 8 + ky: t * 8 + ky + 8, 0:64]
                    nc.tensor.matmul(ps[:msz, :],
                                     wq1p[:, ky, g * 128: g * 128 + msz],
                                     rhs1, start=False, stop=False)
                for ky in range(3):
                    rhs1 = fp1[:, t * 8 + ky: t * 8 + ky + 8, 2:66]
                    nc.tensor.matmul(ps[:msz, :],
                                     wq1k2[:, ky, g * 128: g * 128 + msz],
                                     rhs1, start=False, stop=(ky == 2))
                csb = cs.tile([128, 512], dt.bfloat16, tag="convsb")
                nc.scalar.activation(csb[:msz, :], ps[:msz, :], AF.Identity,
                                     bias=qkvb_sb[:msz, g: g + 1])
                tps = cpt.tile([128, 512], dt.bfloat16, tag="ctps")
                for blk in range(4):
                    nc.tensor.transpose(
                        tps[:, blk * 128: blk * 128 + msz],
                        csb[:msz, blk * 128:(blk + 1) * 128],
                        id_c[:msz, :msz])
                tview = tps[:].rearrange("p (a b) -> p a b", a=4)
                for kind, so, do, n in gdst[g]:
                    if kind == 0:
                        nc.scalar.copy(qstage[:, :, do: do + n],
                                       tview[:, :, so: so + n])
                    elif kind == 1:
                        nc.scalar.copy(kstage[:, :, do: do + n],
                                       tview[:, :, so: so + n])
                    else:
                        nc.scalar.copy(vview[:, :, do: do + n],
                                       tview[:, :, so: so + n])
            # 4x / 2x row-shifted duplicated writes: entry (r, x) sub s holds
            # local row r+s. Strip t covers local rows LT..LT+7.
            LT = (t - 4) * 8
            for s in range(4):
                r0 = HEADKV + (LT - s) * 64
                nc.sync.dma_start(
                    k4[r0: r0 + 512, s * DIM: (s + 1) * DIM]
                    .rearrange("(b p) e -> p b e", p=128), kstage[:])
                nc.sync.dma_start(
                    v4[r0: r0 + 512, s * DIM: (s + 1) * DIM]
                    .rearrange("(b p) e -> p b e", p=128), vstage[:])
            for s in range(2):
                r0 = HEADQ + (LT - s) * 64
                nc.sync.dma_start(
                    q2[r0: r0 + 512, s * 256: (s + 1) * 256]
                    .rearrange("(b p) e -> p b e", p=128), qstage[:])


def _attention(nc, tc, mybir, NT, env):
    import concourse.bass as bass
    dt = mybir.dt
    AX = mybir.AxisListType
    AF = mybir.ActivationFunctionType

    k4, v4, q2 = env["k4"], env["v4"], env["q2"]
    m0w, m13w, m4w = env["m0w"], env["m13w"], env["m4w"]
    bmlp, b4, ident = env["bmlp"], env["b4"], env["ident"]
    kvidx, qidx = env["kvidx"], env["qidx"]
    maskt, qwt, qwbt = env["maskt"], env["qwt"], env["qwbt"]
    out = env["out"]

    with (
        tc.tile_pool(name="aw", bufs=1) as aw,
        tc.tile_pool(name="gath", bufs=2) as gp,
        tc.tile_pool(name="attn", bufs=3) as ap_,
        tc.tile_pool(name="attn3", bufs=2) as ap3,
        tc.tile_pool(name="wvp", bufs=4) as wvp,
        tc.tile_pool(name="xkp", bufs=1) as xkp,
        tc.tile_pool(name="hp", bufs=2) as hp,
        tc.tile_pool(name="outp", bufs=1) as op_,
        tc.tile_pool(name="tpsum", bufs=2, space="PSUM") as tp_,
        tc.tile_pool(name="mpsum", bufs=2, space="PSUM") as mp_,
        tc.tile_pool(name="m0psum", bufs=2, space="PSUM") as mp0,
    ):
        m0w_sb = aw.tile([128, KBLK, HID], dt.bfloat16)
        nc.sync.dma_start(m0w_sb[:], m0w)
        m13_sb = aw.tile([128, 6, HID], dt.bfloat16)
        nc.sync.dma_start(m13_sb[:], m13w)
        m4_sb = aw.tile([128, 2, 3], dt.bfloat16)
        nc.sync.dma_start(m4_sb[:], m4w)
        bm_sb = aw.tile([128, 8], dt.float32)
        nc.sync.dma_start(bm_sb[:], bmlp)
        b4_sb = aw.tile([128, 3], dt.float32)
        nc.sync.dma_start(b4_sb[:], b4)
        id_sb = aw.tile([128, 128], dt.bfloat16)
        nc.sync.dma_start(id_sb[:], ident)
        kvi_sb = aw.tile([128, NT, 16], dt.int16)
        nc.sync.dma_start(kvi_sb[:], kvidx)
        qi_sb = aw.tile([128, NT, 8], dt.int16)
        nc.sync.dma_start(qi_sb[:], qidx)
        mk_sb = aw.tile([128, NT, 56], dt.float32)
        nc.sync.dma_start(mk_sb[:], maskt)
        qw_sb = aw.tile([128, NT, 4], dt.bfloat16)
        nc.sync.dma_start(qw_sb[:], qwt)
        qwb_sb = aw.tile([128, NT, 4], dt.float32)
        nc.sync.dma_start(qwb_sb[:], qwbt)

        out_sb = op_.tile([128, NT, 3], dt.float32)

        KROW = 4 * DIM
        k_ap = k4[:, :]
        k_ap = bass.AP(k_ap.tensor, k_ap.offset,
                       [[KROW, KVROWS - 6], [1, RR * KROW]])
        v_ap = v4[:, :]
        v_ap = bass.AP(v_ap.tensor, v_ap.offset,
                       [[KROW, KVROWS - 6], [1, RR * KROW]])
        q_ap = q2[:, :]
        q_ap = bass.AP(q_ap.tensor, q_ap.offset,
                       [[Q_ENT, QROWS - 2], [1, 2 * Q_ENT]])

        def issue_gathers(t):
            kg = gp.tile([128, 2, RR * KROW], dt.bfloat16, tag="kg", bufs=2)
            nc.gpsimd.dma_gather(kg[:], k_ap, kvi_sb[:, t, :],
                                 num_idxs=256, num_idxs_reg=256,
                                 elem_size=RR * KROW, elem_step=KROW,
                                 single_packet=False)
            vg = gp.tile([128, 2, RR * KROW], dt.float8e4, tag="vg", bufs=3)
            nc.gpsimd.dma_gather(vg[:], v_ap, kvi_sb[:, t, :],
                                 num_idxs=256, num_idxs_reg=256,
                                 elem_size=RR * KROW, elem_step=KROW,
                                 single_packet=False)
            qg = gp.tile([128, 1, 2 * Q_ENT], dt.bfloat16, tag="qg")
            nc.gpsimd.dma_gather(qg[:], q_ap, qi_sb[:, t, :],
                                 num_idxs=128, num_idxs_reg=128,
                                 elem_size=2 * Q_ENT, elem_step=Q_ENT,
                                 single_packet=False)
            return qg, kg, vg

        groups = _groups(NT)
        gof = {}
        for gi, grp in enumerate(groups):
            for ti, t in enumerate(grp):
                gof[t] = (ti, len(grp), grp)

        base_tiles = {}
        logits_st = {}
        pend = {0: issue_gathers(0)}
        mlp_state = [None, None]  # mlp_ps, xchunks of current group

        def stage_a(t):
            qg, kg, vg = pend[t]
            kvv = kg[:].rearrange("p g (x s e) -> p g x s e", x=RR, s=4)
            qg4 = qg[:].rearrange("p o (x y e) -> p o x y e", x=2, y=2)
            if True:
                # ---------------- q vector (bilinear blend) + base ----------
                qprod = ap_.tile([128, 2, 2, DIM], dt.bfloat16, tag="qprod")
                nc.vector.tensor_mul(
                    qprod[:], qg4[:, 0, :, :, 0:DIM],
                    qw_sb[:, t, :].rearrange("p (a b) -> p a b", a=2)
                    .unsqueeze(3).broadcast_to((128, 2, 2, DIM)))
                qt1 = ap_.tile([128, 2, DIM], dt.bfloat16, tag="qt1")
                nc.vector.tensor_add(qt1[:], qprod[:, 0], qprod[:, 1])
                qb = ap_.tile([128, DIM], dt.bfloat16, tag="qb")
                nc.vector.tensor_add(qb[:], qt1[:, 0], qt1[:, 1])

                badd = ap_.tile([128, 2, 2, 3], dt.float32, tag="badd")
                nc.vector.tensor_add(badd[:], qg4[:, 0, :, :, 192:195],
                                     qg4[:, 0, :, :, 195:198])
                bprod = ap_.tile([128, 2, 2, 3], dt.float32, tag="bprod")
                nc.vector.tensor_mul(
                    bprod[:], badd[:],
                    qwb_sb[:, t, :].rearrange("p (a b) -> p a b", a=2)
                    .unsqueeze(3).broadcast_to((128, 2, 2, 3)))
                bt1 = ap_.tile([128, 2, 3], dt.float32, tag="bt1")
                nc.vector.tensor_add(bt1[:], bprod[:, 0], bprod[:, 1])
                base_t = ap_.tile([128, 3], dt.float32, tag="base")
                nc.vector.tensor_add(base_t[:], bt1[:, 0], bt1[:, 1])
                base_tiles[t] = base_t

                # -------- QK logits: 2 bulk muls + per-group binary trees ----
                # logits layout [128, g 2, x 7, s 4, h 8]; (g1, s3) = junk;
                # junk is masked to 0, exp's to 1, and the 7 spurious ones are
                # subtracted from the softmax denominator.
                logits = ap3.tile([128, 2, RR, 4, HEAD], dt.float32, tag="logits")
                for g in range(2):
                    e = ap3.tile([128, RR, 4, DIM], dt.bfloat16, tag="emul",
                                 bufs=1)
                    nc.vector.tensor_mul(
                        e[:], kvv[:, g, :, :, 0:DIM],
                        qb[:].unsqueeze(1).unsqueeze(2)
                        .broadcast_to((128, RR, 4, DIM)))
                    ev = e[:].rearrange("p x s (h d) -> p (x s) h d", h=HEAD)
                    l1 = ap3.tile([128, 28, HEAD, 12], dt.bfloat16, tag="l1",
                                  bufs=1)
                    nc.vector.tensor_add(l1[:], ev[:, :, :, 0:12], ev[:, :, :, 12:24])
                    l2 = ap3.tile([128, 28, HEAD, 6], dt.bfloat16, tag="l2",
                                  bufs=1)
                    nc.vector.tensor_add(l2[:], l1[:, :, :, 0:6], l1[:, :, :, 6:12])
                    l3 = ap3.tile([128, 28, HEAD, 3], dt.bfloat16, tag="l3",
                                  bufs=1)
                    nc.vector.tensor_add(l3[:], l2[:, :, :, 0:3], l2[:, :, :, 3:6])
                    l4 = ap3.tile([128, 28, HEAD, 1], dt.bfloat16, tag="l4",
                                  bufs=1)
                    nc.vector.tensor_add(l4[:], l3[:, :, :, 0:1], l3[:, :, :, 1:2])
                    nc.vector.tensor_add(
                        logits[:, g].rearrange("p x s h -> p (x s) h"),
                        l4[:].rearrange("p u h d -> p u (h d)"),
                        l3[:, :, :, 2:3].rearrange("p u h d -> p u (h d)"))

                lgv = logits[:].rearrange("p g x s h -> p (g x s) h")
                mask_bc = mk_sb[:, t, :].unsqueeze(2).broadcast_to((128, 56, HEAD))
                nc.vector.tensor_mul(lgv, lgv, mask_bc)
                nc.scalar.activation(lgv, lgv, AF.Exp)
                logits_st[t] = logits

        def stage_b(t):
            ti, gw, grp = gof[t]
            logits = logits_st.pop(t)
            lgv = logits[:].rearrange("p g x s h -> p (g x s) h")
            mask_bc = mk_sb[:, t, :].unsqueeze(2).broadcast_to((128, 56, HEAD))
            vg = pend.pop(t)[2]
            vvv = vg[:].rearrange("p g (x s e) -> p g x s e", x=RR, s=4)

            def vslice(dy):
                g, s = (0, dy) if dy < 4 else (1, dy - 4)
                return vvv[:, g, :, s, :]

            if True:
                ssum = ap_.tile([128, HEAD], dt.float32, tag="ssum")
                nc.vector.reduce_sum(
                    ssum[:], logits[:].rearrange("p g x s h -> p h (g x s)"),
                    axis=AX.X)
                nc.vector.tensor_scalar_add(ssum[:], ssum[:], -7.0)
                rec = ap_.tile([128, HEAD], dt.float32, tag="rec")
                nc.vector.reciprocal(rec[:], ssum[:])
                nc.vector.tensor_mul(
                    lgv, lgv, rec[:].unsqueeze(1).broadcast_to((128, 56, HEAD)))
                attnb = ap3.tile([128, 2, RR, 4, HEAD], dt.bfloat16, tag="attnb")
                nc.vector.tensor_mul(
                    attnb[:].rearrange("p g x s h -> p (g x s) h"), lgv, mask_bc)

                # ---- weighted v -> per-dy chunks; MLP0 accumulates per chunk
                if ti == 0:
                    mlp_state[0] = mp0.tile([128, 512], dt.float32, tag="mlp0ps",
                                            name="mlp0ps")
                    mlp_state[1] = [xkp.tile([128, DY_BLOCKS, 256], dt.bfloat16,
                                             tag=f"xc{dy}", name=f"xc{dy}")
                                    for dy in range(RR)]
                mlp_ps, xchunks = mlp_state

                def mlp0_chunk(dy):
                    chunk = xchunks[dy]
                    for m in range(2):
                        for b_ in range(DY_BLOCKS):
                            kb = dy * DY_BLOCKS + b_
                            nc.tensor.matmul(
                                mlp_ps[:, m * 256: m * 256 + gw * 128],
                                m0w_sb[:, kb, m * 128:(m + 1) * 128],
                                chunk[:, b_, 0: gw * 128], start=(kb == 0),
                                stop=(kb == KBLK - 1), skip_group_check=True)

                xcol = ti * 128
                for dy in range(RR):
                    wv = wvp.tile([128, DYW_P], dt.bfloat16, tag="wv")
                    nc.gpsimd.memset(wv[:, DYW:], 0.0)
                    weng = nc.gpsimd if dy >= 5 else nc.vector
                    # v channels are stored d-major (c' = d*8 + h) so that all
                    # three operands are innermost-contiguous -> DVE 2x mode
                    g_, s_ = (0, dy) if dy < 4 else (1, dy - 4)
                    weng.tensor_mul(
                        wv[:, 0:DYW].rearrange("p (r d h) -> p r d h", d=HD, h=HEAD),
                        vslice(dy).rearrange("p r (d h) -> p r d h", h=HEAD),
                        attnb[:, g_, :, s_, :].unsqueeze(2).broadcast_to(
                            (128, RR, HD, HEAD)))
                    for grp2, nb in ((0, 8), (1, 3)):
                        tps = tp_.tile([128, nb * 128], dt.bfloat16,
                                       tag=f"tps{grp2}")
                        for bi_ in range(nb):
                            blk = grp2 * 8 + bi_
                            nc.tensor.transpose(tps[:, bi_ * 128:(bi_ + 1) * 128],
                                                wv[:, blk * 128:(blk + 1) * 128],
                                                id_sb[:])
                        nc.scalar.copy(
                            xchunks[dy][:, grp2 * 8: grp2 * 8 + nb, xcol: xcol + 128],
                            tps[:].rearrange("p (a b) -> p a b", a=nb))
                    # emit chunk dy-1's MLP0 matmuls one dy late so the PE is
                    # not stalled on the PSUM->SBUF copy of the current chunk
                    if ti == gw - 1 and dy >= 1:
                        mlp0_chunk(dy - 1)

                # ---------------- MLP layers 0-3 + head (per tile group) -----
                if ti == gw - 1:
                    mlp0_chunk(RR - 1)
                    h0 = hp.tile([128, 2, 256], dt.bfloat16, tag="h")
                    for m in range(2):
                        nc.scalar.activation(h0[:, m, 0: gw * 128],
                                             mlp_ps[:, m * 256: m * 256 + gw * 128],
                                             AF.Relu, bias=bm_sb[:, m:m + 1])
                    cur = h0
                    for l in (1, 2, 3):
                        nxt = hp.tile([128, 2, 256], dt.bfloat16, tag="h")
                        for m in range(2):
                            ps = mp_.tile([128, 256], dt.float32, tag="mlpps")
                            for kc in range(2):
                                nc.tensor.matmul(
                                    ps[:, 0: gw * 128],
                                    m13_sb[:, (l - 1) * 2 + kc,
                                           m * 128:(m + 1) * 128],
                                    cur[:, kc, 0: gw * 128],
                                    start=(kc == 0), stop=(kc == 1))
                            nc.scalar.activation(nxt[:, m, 0: gw * 128],
                                                 ps[:, 0: gw * 128], AF.Relu,
                                                 bias=bm_sb[:, 2 * l + m: 2 * l + m + 1])
                        cur = nxt
                    for tt in range(gw):
                        psof = mp_.tile([128, 256], dt.float32, tag="mlpps")
                        pso = psof[:, 0:3]
                        for kc in range(2):
                            nc.tensor.matmul(pso,
                                             cur[:, kc, tt * 128: tt * 128 + 128],
                                             m4_sb[:, kc, :],
                                             start=(kc == 0), stop=(kc == 1))
                        o1 = ap_.tile([128, 3], dt.float32, tag="o1")
                        nc.vector.tensor_add(o1[:], pso, b4_sb[:])
                        nc.vector.tensor_add(out_sb[:, grp[tt], :], o1[:],
                                             base_tiles.pop(grp[tt])[:])

        # software pipeline: [gathers(t+1); stage A(t); stage B(t-1)] so the
        # DVE never blocks on the ACT exp or on gather completion
        for it in range(NT + 1):
            if it < NT:
                if it + 1 < NT:
                    pend[it + 1] = issue_gathers(it + 1)
                stage_a(it)
            if it >= 1:
                stage_b(it - 1)

        nc.sync.dma_start(
            out.rearrange("(t p) c -> p t c", p=128), out_sb[:])


# ============================ host preparation ==============================

def _host_prep(inputs):
    inp = np.asarray(inputs["inp"], f32)
    sc = np.asarray(inputs["sample_coord"], f32)
    cell = np.asarray(inputs["cell"], f32)

    enc_w = np.asarray(inputs["enc_w"], f32)
    ch_w = np.asarray(inputs["ch_w"], f32)

    w_enc = enc_w.transpose(1, 2, 3, 0).reshape(27, ENC).astype(bf16)
    w_chp = np.zeros((128, 3, DIM), bf16)
    w_ch2 = np.zeros((ENC, 3, DIM), bf16)
    for ky in range(3):
        w_chp[0:64, ky, :] = ch_w[:, :, ky, 0].T.astype(bf16)
        w_chp[64:128, ky, :] = ch_w[:, :, ky, 1].T.astype(bf16)
        w_ch2[:, ky, :] = ch_w[:, :, ky, 2].T.astype(bf16)

    # qkv weights M-packed: 576 output cols = [q 192 | k 192 | v 192]
    w_qkv0 = np.zeros((128, 9, 576), bf16)
    w_qkv1p = np.zeros((128, 3, 576), bf16)
    w_qkv1k2 = np.zeros((64, 3, 576), bf16)
    qkvb = np.zeros((128, 5), f32)
    # v output channels permuted d-major: device channel c' = d*8+h holds
    # reference channel h*24+d (lets the wv multiply run innermost-contiguous)
    cmap_v = (np.arange(192) % 8) * 24 + np.arange(192) // 8
    for ci, nm in enumerate(("q", "k", "v")):
        wt = np.asarray(inputs[f"{nm}_w"], f32)
        bt = np.asarray(inputs[f"{nm}_b"], f32)
        if nm == "v":
            wt = wt[cmap_v]
            bt = bt[cmap_v]
        cs_ = ci * 192
        for off in range(9):
            ky, kx = off // 3, off % 3
            w_qkv0[:, off, cs_: cs_ + 192] = wt[:, 0:128, ky, kx].T.astype(bf16)
        for ky in range(3):
            w_qkv1p[0:64, ky, cs_: cs_ + 192] = wt[:, 128:192, ky, 0].T.astype(bf16)
            w_qkv1p[64:128, ky, cs_: cs_ + 192] = wt[:, 128:192, ky, 1].T.astype(bf16)
            w_qkv1k2[:, ky, cs_: cs_ + 192] = wt[:, 128:192, ky, 2].T.astype(bf16)
        bfull = bt
        for g in range(5):
            msz = 128 if g < 4 else 64
            seg = np.arange(g * 128, g * 128 + msz)
            sel = (seg >= cs_) & (seg < cs_ + 192)
            if sel.any():
                qkvb[np.nonzero(sel)[0], g] = bfull[seg[sel] - cs_]

    # m0w rows permuted into 7 chunks of 1344 -> 1408 (zero padded); within a
    # chunk the device feature order is (dx, d, h): j = dx*192 + d*8 + h maps
    # to reference row (dy*7+dx)*192 + h*24 + d
    m0w_full = np.asarray(inputs["m0w"], f32)
    jj = np.arange(DYW)
    jdx, jc = jj // 192, jj % 192
    jd, jh = jc // 8, jc % 8
    ref_j = jdx * 192 + jh * 24 + jd
    perm = np.zeros((KBLK * 128, HID), f32)
    for i in range(RR):
        perm[i * DYW_P: i * DYW_P + DYW] = m0w_full[i * DYW + ref_j]
    m0w_dev = np.ascontiguousarray(
        perm.reshape(KBLK, 128, HID).transpose(1, 0, 2)).astype(bf16)

    m13w = np.zeros((128, 6, HID), bf16)
    for l in (1, 2, 3):
        wl = np.asarray(inputs[f"m{l}w"], f32)
        m13w[:, (l - 1) * 2 + 0, :] = wl[0:128].astype(bf16)
        m13w[:, (l - 1) * 2 + 1, :] = wl[128:256].astype(bf16)
    m4w_full = np.asarray(inputs["m4w"], f32)
    m4w = np.stack([m4w_full[0:128], m4w_full[128:256]], 1).astype(bf16)

    b4 = np.broadcast_to(np.asarray(inputs["m4b"], f32)[None, :], (128, 3)).copy()
    enc_bd = np.asarray(inputs["enc_b"], f32).reshape(ENC, 1)
    ch_bd = np.zeros((128, 2), f32)
    ch_bd[:, 0] = np.asarray(inputs["ch_b"], f32)[0:128]
    ch_bd[0:64, 1] = np.asarray(inputs["ch_b"], f32)[128:192]
    ident = np.eye(128, dtype=bf16)

    m0b = np.asarray(inputs["m0b"], f32)
    m0w_tail = m0w_full[RA * DIM: RA * DIM + 2]
    bias_rest = np.zeros((128, 8), f32)
    for l in (1, 2, 3):
        bl = np.asarray(inputs[f"m{l}b"], f32)
        bias_rest[:, 2 * l + 0] = bl[0:128]
        bias_rest[:, 2 * l + 1] = bl[128:256]

    # ---- ownership: core (bi, qc) owns queries with iy//16 == qc ----
    sqh = f32(1.0 / np.sqrt(HD))
    cy_all, cx_all = sc[..., 0], sc[..., 1]
    py_all = (cy_all + f32(1.0)) * f32(H * 0.5) - f32(0.5)
    px_all = (cx_all + f32(1.0)) * f32(W * 0.5) - f32(0.5)
    iy_all = np.clip(np.floor(py_all + f32(0.5)), 0, H - 1).astype(np.int64)
    ix_all = np.clip(np.floor(px_all + f32(0.5)), 0, W - 1).astype(np.int64)

    owners = []
    for core in range(N_CORES):
        bi, qc = core // 4, core % 4
        own = np.nonzero(iy_all[bi] // 16 == qc)[0]
        owners.append(own)
    NT = max((len(o) + 127) // 128 for o in owners)
    SLOTS = NT * 128

    batch_data = []
    for bi in range(B):
        x = inp[bi]
        xT = x.reshape(3, H * W).T
        hi = xT.astype(bf16).astype(f32)
        lo = (xT - hi).astype(bf16)
        hilo_full = np.concatenate([hi.astype(bf16), lo], 1)  # [4096, 6]

        rel_cell = cell[bi] * np.array([H, W], f32)
        b0 = m0b + rel_cell @ m0w_tail
        bm = bias_rest.copy()
        bm[:, 0] = b0[0:128]
        bm[:, 1] = b0[128:256]
        batch_data.append((x, hilo_full, bm))

    d = np.arange(-R, R + 1)
    percore = []
    for core in range(N_CORES):
        bi, qc = core // 4, core % 4
        R0 = 16 * qc
        own = owners[core]
        n = len(own)
        x, hilo_full, bm = batch_data[bi]

        # padded image: padded row p <-> image row R0 + p - 35
        xbig = np.zeros((CH_IN, 128, W + 2), f32)
        plo, phi = max(0, 35 - R0), min(128, 35 - R0 + H)
        xbig[:, plo:phi, 1:1 + W] = x[:, plo - 35 + R0: phi - 35 + R0, :]

        # im2col for enc strips 2..7 (rows 16..63)
        col = np.zeros((27, 6 * 512), bf16)
        for c in range(CH_IN):
            for ky in range(3):
                for kx in range(3):
                    col[c * 9 + ky * 3 + kx] = \
                        xbig[c, 15 + ky: 63 + ky, kx: kx + W] \
                        .reshape(-1).astype(bf16)
        tr = np.arange(16, 64) - 35 + R0  # image rows of enc output strips
        col.reshape(27, 48, W)[:, (tr < 0) | (tr >= H), :] = 0

        # ch rows 31..56 mask (image rows R0-4 .. R0+21)
        trf = np.arange(26) + R0 - 4
        maskfp = np.broadcast_to(
            ((trf >= 0) & (trf < H)).astype(bf16)[None, :], (128, 26)).copy()

        # hilo for local pixel rows 0..23 (image rows R0-3 .. R0+20)
        hl = np.zeros((NPX, 6), bf16)
        rlo, rhi = max(0, R0 - 3), min(H, R0 + 21)
        hl[(rlo - R0 + 3) * W: (rhi - R0 + 3) * W] = \
            hilo_full[rlo * W: rhi * W]
        hilo = np.ascontiguousarray(hl.reshape(12, 128, 6).transpose(1, 0, 2))

        # ---- per-query gather indices / weights ----
        iy, ix = iy_all[bi][own], ix_all[bi][own]
        py, px = py_all[bi][own], px_all[bi][own]

        dy_, dx_ = [a.reshape(-1) for a in np.meshgrid(d, d, indexing="ij")]
        yy = iy[:, None] + dy_[None, :]
        xx = ix[:, None] + dx_[None, :]
        valid = ((yy >= 0) & (yy < H) & (xx >= 0) & (xx < W)).astype(f32)

        # kv entry index for group g: entry (iy - R0 + 4g)*64 + ix - 3,
        # tensor row = HEADKV + entry = 192 + (iy-R0+4g)*64 + ix
        kvstart = np.zeros((n, 2), np.int64)
        for g in range(2):
            kvstart[:, g] = 192 + (iy - R0 + 4 * g) * 64 + ix

        y0 = np.floor(py)
        x0 = np.floor(px)
        wy, wx = py - y0, px - x0
        y0 = y0.astype(np.int64)
        x0 = x0.astype(np.int64)
        sy0 = np.clip(y0, 0, H - 2)
        sx0 = np.clip(x0, 0, W - 2)
        wq_eff = np.zeros((n, 2, 2), f32)
        wb_eff = np.zeros((n, 2, 2), f32)
        qq = np.arange(n)
        for ddy, syw in ((0, 1 - wy), (1, wy)):
            for ddx, sxw in ((0, 1 - wx), (1, wx)):
                w = (syw * sxw).astype(f32)
                yc, xc = y0 + ddy, x0 + ddx
                ly = np.clip(yc, 0, H - 1) - sy0
                lx = np.clip(xc, 0, W - 1) - sx0
                wb_eff[qq, ly, lx] += w
                vm = ((yc >= 0) & (yc < H) & (xc >= 0) & (xc < W))
                wq_eff[qq, ly, lx] += w * vm
        # qg4 layout is [x(sx), y(sy)] -> reorder weights to (lx, ly)
        wq4 = wq_eff.transpose(0, 2, 1).reshape(n, 4)
        wb4 = wb_eff.transpose(0, 2, 1).reshape(n, 4)
        qstart = HEADQ + (sy0 - R0 + 3) * 64 + sx0

        # pad to SLOTS
        def padto(a, fill):
            outp = np.full((SLOTS,) + a.shape[1:], fill, a.dtype)
            outp[:n] = a
            return outp

        kvstart_p = padto(kvstart, 192)
        qstart_p = padto(qstart, HEADQ + 128)
        valid_p = padto(valid, 0.0)
        wq4_p = padto(wq4.astype(f32), 0.0)
        wb4_p = padto(wb4, 0.0)

        kvidx = np.zeros((128, NT, 16), np.int16)
        qidx = np.zeros((128, NT, 8), np.int16)
        masktt = np.zeros((128, NT, 56), f32)
        qwt = np.zeros((128, NT, 4), bf16)
        qwbt = np.zeros((128, NT, 4), f32)
        # mask in device (g, x, s) order: u = g*28 + x*4 + s, dy = g*4+s, dx = x
        uu = np.arange(56)
        gu, xu, su = uu // 28, (uu % 28) // 4, uu % 4
        dyu = gu * 4 + su
        usel = dyu < 7
        for t in range(NT):
            ts = slice(t * 128, (t + 1) * 128)
            masktt[:, t, uu[usel]] = valid_p[ts][:, dyu[usel] * 7 + xu[usel]]
            qwt[:, t, :] = (wq4_p[ts] * sqh).astype(bf16)
            qwbt[:, t, :] = wb4_p[ts]
            flat = kvstart_p[ts].T.reshape(-1)  # j = g*128 + q
            kvidx[:, t, :] = np.tile(flat.reshape(-1, 16).T, (8, 1)).astype(np.int16)
            fq = qstart_p[ts]
            qidx[:, t, :] = np.tile(fq.reshape(-1, 16).T, (8, 1)).astype(np.int16)

        percore.append({
            "inp_col": col, "inp_hilo": hilo,
            "w_enc": w_enc, "w_chp": w_chp, "w_ch2": w_ch2,
            "w_qkv0": w_qkv0, "w_qkv1p": w_qkv1p, "w_qkv1k2": w_qkv1k2,
            "qkvb": qkvb, "maskfp": maskfp, "enc_b": enc_bd, "ch_b": ch_bd,
            "m0w": m0w_dev, "m13w": m13w, "m4w": m4w, "bmlp": bm, "b4": b4,
            "ident": ident, "kvidx": kvidx, "qidx": qidx, "maskt": masktt,
            "qwt": qwt, "qwbt": qwbt,
        })
    return percore, NT, owners


# ============================== entry point =================================

def _get_program(NT):
    if NT not in _PROGRAMS:
        _PROGRAMS[NT] = build_program(NT)
    return _PROGRAMS[NT]


def kernel(**inputs):
    from concourse import bass_utils
    in_maps, NT, owners = _host_prep(inputs)
    nc = _get_program(NT)
    res = bass_utils.run_bass_kernel_spmd(nc, in_maps, core_ids=list(range(N_CORES)))
    full = np.empty((B, Q, 3), f32)
    for core in range(N_CORES):
        bi = core // 4
        own = owners[core]
        full[bi, own] = res.results[core]["out"][:len(own)]
    return full


if __name__ == "__main__":
    import time
    t0 = time.time()
    nc = _get_program(9)
    print("built+compiled in", time.time() - t0, "s")
